# revision 7
# baseline (speedup 1.0000x reference)
"""Trainium2 Bass kernel for a pre-LN transformer block (MHA + FFN).

Data-parallel over batch: 8 NeuronCores, one batch element each.
All matmuls run as float32r (full PE rate at free-dim>=256), storage fp32.
"""
import sys

for _p in ("/opt/trn_rl_repo", "/root/.axon_site/_ro/trn_rl_repo"):
    if _p not in sys.path:
        sys.path.insert(0, _p)

import numpy as np
import concourse.bass as bass
import concourse.tile as tile
from concourse import bacc, mybir
from concourse.bass import ds, ts
from concourse.bass_utils import run_bass_kernel_spmd
from concourse.masks import make_identity

P = 128
N = 1024          # tokens per core (seq len)
D = 1024          # d_emb
H = 16            # heads
HS = 64           # head size
FF = 4096         # ffn hidden
NT = N // P       # 8 token tiles
DB = D // P       # 8 d blocks
EBS = D // P      # 8 e blocks (qkv out features)
NH = 2            # n halves of 512
LN_EPS = 1e-5

F32 = mybir.dt.float32
R = mybir.dt.float32r
AF = mybir.ActivationFunctionType
OP = mybir.AluOpType

_CACHED_NC = None


def build_nc():
    nc = bacc.Bacc("TRN2", target_bir_lowering=False, debug=False, num_devices=8)

    x_d = nc.dram_tensor("x", [N, D], F32, kind="ExternalInput").ap()
    wq_d = nc.dram_tensor("Wq", [H, D, HS], F32, kind="ExternalInput").ap()
    bq_d = nc.dram_tensor("bq", [H, HS], F32, kind="ExternalInput").ap()
    wk_d = nc.dram_tensor("Wk", [H, D, HS], F32, kind="ExternalInput").ap()
    bk_d = nc.dram_tensor("bk", [H, HS], F32, kind="ExternalInput").ap()
    wv_d = nc.dram_tensor("Wv", [H, D, HS], F32, kind="ExternalInput").ap()
    bv_d = nc.dram_tensor("bv", [H, HS], F32, kind="ExternalInput").ap()
    wp_d = nc.dram_tensor("Wproj", [H * HS, D], F32, kind="ExternalInput").ap()
    bp_d = nc.dram_tensor("bproj", [D], F32, kind="ExternalInput").ap()
    w1_d = nc.dram_tensor("W1", [D, FF], F32, kind="ExternalInput").ap()
    b1_d = nc.dram_tensor("b1", [FF], F32, kind="ExternalInput").ap()
    w2_d = nc.dram_tensor("W2", [FF, D], F32, kind="ExternalInput").ap()
    b2_d = nc.dram_tensor("b2", [D], F32, kind="ExternalInput").ap()
    g1_d = nc.dram_tensor("ln1_g", [D], F32, kind="ExternalInput").ap()
    c1_d = nc.dram_tensor("ln1_b", [D], F32, kind="ExternalInput").ap()
    g2_d = nc.dram_tensor("ln2_g", [D], F32, kind="ExternalInput").ap()
    c2_d = nc.dram_tensor("ln2_b", [D], F32, kind="ExternalInput").ap()
    out_d = nc.dram_tensor("out", [N, D], F32, kind="ExternalOutput").ap()
    x2pb_d = nc.dram_tensor("x2pb_scratch", [N, D], F32).ap()

    with tile.TileContext(nc) as tc:
        with tc.tile_pool(name="cn", bufs=1) as cp, \
             tc.tile_pool(name="big", bufs=1) as bp:
            # ---- constants / bias vectors (persistent, tiny) ----
            ident = cp.tile([P, P], F32)
            make_identity(nc, ident[:])
            ones_f = cp.tile([P, 1], F32)
            nc.vector.memset(ones_f[:], 1.0)
            ones64 = cp.tile([1, HS], R)
            nc.vector.tensor_copy(ones64[:],
                                  ones_f[0:1, :].to_broadcast([1, HS]))
            onesP = cp.tile([1, P], R)
            nc.vector.tensor_copy(onesP[:],
                                  ones_f[0:1, :].to_broadcast([1, P]))
            epsv = cp.tile([P, 1], F32)
            nc.vector.memset(epsv[:], LN_EPS)

            bqv = cp.tile([P, EBS], F32)
            nc.sync.dma_start(bqv[:], bq_d.rearrange("h s -> (h s)")
                              .rearrange("(b p) -> p b", p=P))
            bkv = cp.tile([P, EBS], F32)
            nc.sync.dma_start(bkv[:], bk_d.rearrange("h s -> (h s)")
                              .rearrange("(b p) -> p b", p=P))
            bvv = cp.tile([P, EBS], F32)
            nc.sync.dma_start(bvv[:], bv_d.rearrange("h s -> (h s)")
                              .rearrange("(b p) -> p b", p=P))
            g1v = cp.tile([P, DB], F32)
            nc.sync.dma_start(g1v[:], g1_d.rearrange("(b p) -> p b", p=P))
            c1v = cp.tile([P, DB], F32)
            nc.sync.dma_start(c1v[:], c1_d.rearrange("(b p) -> p b", p=P))
            g2v = cp.tile([P, DB], F32)
            nc.sync.dma_start(g2v[:], g2_d.rearrange("(b p) -> p b", p=P))
            c2v = cp.tile([P, DB], F32)
            nc.sync.dma_start(c2v[:], c2_d.rearrange("(b p) -> p b", p=P))
            b1v = cp.tile([P, FF // P], F32)
            nc.sync.dma_start(b1v[:], b1_d.rearrange("(b p) -> p b", p=P))

            # LN stats scratch (reused for LN2 by tag)
            st_sum = cp.tile([P, NT], F32)
            st_sq = cp.tile([P, NT], F32)
            st_mu = cp.tile([P, NT], F32)
            st_var = cp.tile([P, NT], F32)
            st_rs = cp.tile([P, NT], F32)
            st_nm = cp.tile([P, NT], F32)

            def layernorm_transpose(src, dst, gv, cv, psum_pool):
                """src: [P, NT, D] token layout (f32) -> dst [P, DB, N] f32r
                feature layout, with affine (gv, cv per-partition) folded into
                the transpose evacuation."""
                for tb in range(NT):
                    nc.vector.reduce_sum(st_sum[:, tb:tb + 1], src[:, tb, :],
                                         axis=mybir.AxisListType.X)
                    sq = bp.tile([P, D], F32, tag="qb", bufs=2,
                                 name=f"sq{tb}")
                    nc.scalar.activation(sq[:], src[:, tb, :], AF.Square,
                                         accum_out=st_sq[:, tb:tb + 1])
                nc.vector.tensor_scalar_mul(st_mu[:], st_sum[:], 1.0 / D)
                nc.vector.tensor_scalar_mul(st_var[:], st_sq[:], 1.0 / D)
                nc.vector.tensor_tensor(st_nm[:], st_mu[:], st_mu[:], OP.mult)
                nc.vector.tensor_tensor(st_var[:], st_var[:], st_nm[:],
                                        OP.subtract)
                nc.scalar.activation(st_rs[:], st_var[:], AF.Sqrt, bias=epsv[:])
                nc.vector.reciprocal(st_rs[:], st_rs[:])
                nc.vector.tensor_tensor(st_nm[:], st_mu[:], st_rs[:], OP.mult)
                nc.vector.tensor_scalar_mul(st_nm[:], st_nm[:], -1.0)
                # t = x*rstd - mu*rstd, reusing src's own slot is not possible
                # (in0 != out), so normalize into a per-tb temp then transpose.
                for tb in range(NT):
                    tnorm = bp.tile([P, D], F32, tag="kb", bufs=2,
                                    name=f"tn{tb}")
                    nc.vector.tensor_scalar(tnorm[:], src[:, tb, :],
                                            st_rs[:, tb:tb + 1],
                                            st_nm[:, tb:tb + 1],
                                            OP.mult, OP.add)
                    for db in range(DB):
                        pt = psum_pool.tile([P, P], F32, tag="tr", bufs=4,
                                            name=f"ptr{tb}_{db}")
                        nc.tensor.transpose(pt[:], tnorm[:, ts(db, P)],
                                            ident[:])
                        nc.vector.tensor_scalar(dst[:, db, ts(tb, P)], pt[:],
                                                gv[:, db:db + 1],
                                                cv[:, db:db + 1],
                                                OP.mult, OP.add)

            # ================= Phase A: load x, LN1, transpose =============
            xsb = bp.tile([P, NT, D], F32, tag="v", name="xsb")
            xr3 = x_d.rearrange("(t p) d -> p t d", p=P)
            for tb in range(NT):
                nc.sync.dma_start(xsb[:, tb, :], xr3[:, tb, :])
            HT = bp.tile([P, DB, N], R, tag="ht", name="HT")
            with tc.tile_pool(name="psA", bufs=1, space="PSUM") as psA:
                layernorm_transpose(xsb, HT, g1v, c1v, psA)

            # ================= Phase B0: V projection ======================
            # Vaug[p, tb, h, 0:64] = (h_ln @ Wv)[tok, h*64+s]; [..., 64] = 1.0
            Vaug = bp.tile([P, NT, H, HS + 1], R, tag="v", name="Vaug")
            nc.vector.tensor_copy(
                Vaug[:, :, :, HS:HS + 1],
                ones_f[:, None, :].to_broadcast([P, NT, H, 1]))
            with tc.tile_pool(name="psB", bufs=1, space="PSUM") as psB:
                for eh in range(2):
                    wvt = bp.tile([P, DB, 512], R, tag="se", bufs=2,
                                  name=f"wv{eh}")
                    for do in range(DB):
                        nc.sync.dma_start(
                            wvt[:, do].rearrange("p (h s) -> p h s", s=HS),
                            wv_d[eh * 8:(eh + 1) * 8, ds(do * P, P), :]
                            .rearrange("h dp s -> dp h s")
                            .bitcast(R))
                    for tb in range(NT):
                        pv = psB.tile([P, 512], F32, tag="qkv", bufs=4,
                                      name=f"pv{eh}_{tb}")
                        for db in range(DB):
                            nc.tensor.matmul(pv[:], HT[:, db, ts(tb, P)],
                                             wvt[:, db, :],
                                             start=(db == 0), stop=(db == DB - 1))
                        nc.vector.tensor_copy(
                            Vaug[:, tb, eh * 8:(eh + 1) * 8, 0:HS],
                            pv[:].rearrange("p (h s) -> p h s", s=HS))

            # ============ Phase BC: Q/K per e-block fused with attention ===
            attnT = bp.tile([P, EBS, N], R, tag="at", name="attnT")
            # prefetch Wproj during attention
            wpa = bp.tile([P, 4, D], R, tag="sd", bufs=2, name="wpa")
            wpb = bp.tile([P, 4, D], R, tag="sd", bufs=2, name="wpb")
            nc.sync.dma_start(
                wpa[:], wp_d[0:512].rearrange("(eo ep) d -> ep eo d", ep=P)
                .bitcast(R))
            nc.sync.dma_start(
                wpb[:], wp_d[512:1024].rearrange("(eo ep) d -> ep eo d", ep=P)
                .bitcast(R))

            with tc.tile_pool(name="psC", bufs=1, space="PSUM") as psC:
                for eb in range(EBS):
                    wqt = bp.tile([P, DB, P], R, tag="wqk", bufs=2,
                                  name=f"wq{eb}")
                    for do in range(DB):
                        nc.sync.dma_start(
                            wqt[:, do].rearrange("p (h s) -> p h s", s=HS),
                            wq_d[2 * eb:2 * eb + 2, ds(do * P, P), :]
                            .rearrange("h dp s -> dp h s")
                            .bitcast(R))
                    wkt = bp.tile([P, DB, P], R, tag="wqk", bufs=2,
                                  name=f"wk{eb}")
                    for do in range(DB):
                        nc.sync.dma_start(
                            wkt[:, do].rearrange("p (h s) -> p h s", s=HS),
                            wk_d[2 * eb:2 * eb + 2, ds(do * P, P), :]
                            .rearrange("h dp s -> dp h s")
                            .bitcast(R))
                    Qb = bp.tile([P, N], R, tag="qb", bufs=2, name=f"Qb{eb}")
                    Kb = bp.tile([P, N], R, tag="kb", bufs=2, name=f"Kb{eb}")
                    for nh in range(NH):
                        pq = psC.tile([P, 512], F32, tag="qk", bufs=2,
                                      name=f"pq{eb}_{nh}")
                        for db in range(DB):
                            nc.tensor.matmul(pq[:], wqt[:, db, :],
                                             HT[:, db, ds(nh * 512, 512)],
                                             start=(db == 0), stop=(db == DB - 1))
                        nc.vector.tensor_scalar_add(Qb[:, ds(nh * 512, 512)],
                                                    pq[:], bqv[:, eb:eb + 1])
                        pk = psC.tile([P, 512], F32, tag="qk", bufs=2,
                                      name=f"pk{eb}_{nh}")
                        for db in range(DB):
                            nc.tensor.matmul(pk[:], wkt[:, db, :],
                                             HT[:, db, ds(nh * 512, 512)],
                                             start=(db == 0), stop=(db == DB - 1))
                        nc.vector.tensor_scalar_add(Kb[:, ds(nh * 512, 512)],
                                                    pk[:], bkv[:, eb:eb + 1])

                    # attention for heads 2eb (partitions 0:64) and
                    # 2eb+1 (partitions 64:128), per n-half of 512
                    for nh in range(NH):
                        pts = [bp.tile([P, NT, 512], R, tag="se", bufs=2,
                                       name=f"PT{eb}_{nh}_{i}")
                               for i in range(2)]
                        # scoresT[m, n] = sum_s K[m,s] Q[n,s]; exp via ACT
                        for mt in range(NT):
                            for i in range(2):
                                base = i * HS
                                pss = psC.tile([P, 512], F32, tag="sc",
                                               bufs=2, name=f"ps{eb}{nh}{mt}{i}")
                                nc.tensor.matmul(
                                    pss[:],
                                    Kb[base:base + HS, ts(mt, P)],
                                    Qb[base:base + HS, ds(nh * 512, 512)],
                                    start=True, stop=True)
                                nc.scalar.activation(pts[i][:, mt, :], pss[:],
                                                     AF.Exp, scale=0.125)
                        pas = [psC.tile([HS + 1, 512], F32, tag="at65",
                                        bufs=2, name=f"pa{eb}_{nh}_{i}")
                               for i in range(2)]
                        for mb in range(NT):
                            for i in range(2):
                                nc.tensor.matmul(pas[i][:],
                                                 Vaug[:, mb, 2 * eb + i, :],
                                                 pts[i][:, mb, :],
                                                 start=(mb == 0),
                                                 stop=(mb == NT - 1))
                        for i in range(2):
                            base = i * HS
                            rec = bp.tile([1, 512], R, tag="rb", bufs=2,
                                          name=f"rc{eb}_{nh}_{i}")
                            with nc.allow_low_precision(
                                    reason="f32r is full fp32 bits here"):
                                nc.vector.reciprocal(rec[:],
                                                     pas[i][HS:HS + 1, :])
                            prb = psC.tile([HS, 512], F32, tag="rbp", bufs=2,
                                           name=f"prb{eb}_{nh}_{i}")
                            nc.tensor.matmul(prb[:], ones64[:], rec[:],
                                             start=True, stop=True)
                            rbs = bp.tile([HS, 512], F32, tag="rb", bufs=2,
                                          name=f"rb{eb}_{nh}_{i}")
                            nc.vector.tensor_copy(rbs[:], prb[:])
                            dstA = attnT[base:base + HS, eb,
                                         ds(nh * 512, 512)]
                            nc.vector.tensor_tensor(dstA, pas[i][0:HS, :],
                                                    rbs[:], OP.mult)
                            nc.vector.tensor_scalar_add(
                                dstA, dstA, bvv[base:base + HS, eb:eb + 1])

            # ================= Phase D: proj + residual ====================
            xr = bp.tile([P, NT, D], F32, tag="ht", name="xrl")
            for tb in range(NT):
                nc.sync.dma_start(xr[:, tb, :], xr3[:, tb, :])
            x2 = bp.tile([P, NT, D], F32, tag="v", name="x2")
            with tc.tile_pool(name="psD", bufs=1, space="PSUM") as psD:
                # broadcast bproj -> [P, D]
                bprow = bp.tile([1, D], R, tag="kb", bufs=2, name="bprow")
                nc.sync.dma_start(bprow[:], bp_d[None, :].bitcast(R))
                bpB = bp.tile([P, D], F32, tag="qb", bufs=2, name="bpB")
                for dh in range(2):
                    pbb = psD.tile([P, 512], F32, tag="pj", bufs=4,
                                   name=f"pbb{dh}")
                    nc.tensor.matmul(pbb[:], onesP[:],
                                     bprow[:, ds(dh * 512, 512)],
                                     start=True, stop=True)
                    nc.vector.tensor_copy(bpB[:, ds(dh * 512, 512)], pbb[:])
                for tb in range(NT):
                    nc.vector.tensor_tensor(xr[:, tb, :], xr[:, tb, :],
                                            bpB[:], OP.add)
                for tb in range(NT):
                    for dt in range(2):
                        pp = psD.tile([P, 512], F32, tag="pj", bufs=4,
                                      name=f"pp{tb}_{dt}")
                        for g in range(2):
                            wp = wpa if g == 0 else wpb
                            for eo in range(4):
                                nc.tensor.matmul(
                                    pp[:], attnT[:, g * 4 + eo, ts(tb, P)],
                                    wp[:, eo, ds(dt * 512, 512)],
                                    start=(g == 0 and eo == 0),
                                    stop=(g == 1 and eo == 3))
                        nc.vector.tensor_tensor(x2[:, tb, ds(dt * 512, 512)],
                                                pp[:],
                                                xr[:, tb, ds(dt * 512, 512)],
                                                OP.add)

            # ================= Phase E: LN2, transpose, stash x2+b2 ========
            H2T = bp.tile([P, DB, N], R, tag="ht", name="H2T")
            with tc.tile_pool(name="psE", bufs=1, space="PSUM") as psE:
                layernorm_transpose(x2, H2T, g2v, c2v, psE)
                # broadcast b2 -> [P, D]; x2 += b2B; stash to DRAM
                b2row = bp.tile([1, D], R, tag="kb", bufs=2, name="b2row")
                nc.sync.dma_start(b2row[:], b2_d[None, :].bitcast(R))
                b2B = bp.tile([P, D], F32, tag="qb", bufs=2, name="b2B")
                for dh in range(2):
                    pb2 = psE.tile([P, 512], F32, tag="trb", bufs=2,
                                   name=f"pb2{dh}")
                    nc.tensor.matmul(pb2[:], onesP[:],
                                     b2row[:, ds(dh * 512, 512)],
                                     start=True, stop=True)
                    nc.vector.tensor_copy(b2B[:, ds(dh * 512, 512)], pb2[:])
                x2r3 = x2pb_d.rearrange("(t p) d -> p t d", p=P)
                for tb in range(NT):
                    nc.vector.tensor_tensor(x2[:, tb, :], x2[:, tb, :],
                                            b2B[:], OP.add)
                    nc.sync.dma_start(x2r3[:, tb, :], x2[:, tb, :])

            # ================= Phase F: FFN ================================
            with tc.tile_pool(name="psF", bufs=1, space="PSUM") as psF:
                for nt in range(NH):
                    y1 = [bp.tile([P, 16, 512], R, tag=tg, name=f"y1{nt}{tg}")
                          for tg in ("v", "at")]
                    for ft in range(FF // 512):
                        for fc in range(4):
                            bf = ft * 4 + fc
                            w1c = bp.tile([P, DB, P], R, tag="wqk", bufs=2,
                                          name=f"w1_{nt}_{bf}")
                            nc.sync.dma_start(
                                w1c[:],
                                w1_d[:, ds(bf * P, P)]
                                .rearrange("(do dp) f -> dp do f", dp=P)
                                .bitcast(R))
                            p1 = psF.tile([P, 512], F32, tag="fp", bufs=8,
                                          name=f"p1_{nt}_{bf}")
                            for db in range(DB):
                                nc.tensor.matmul(
                                    p1[:], w1c[:, db, :],
                                    H2T[:, db, ds(nt * 512, 512)],
                                    start=(db == 0), stop=(db == DB - 1))
                            z = bp.tile([P, 512], F32, tag="qb", bufs=2,
                                        name=f"z{nt}_{bf}")
                            nc.scalar.activation(z[:], p1[:], AF.Identity,
                                                 bias=b1v[:, bf:bf + 1])
                            zs = bp.tile([P, 512], F32, tag="se", bufs=2,
                                         name=f"zs{nt}_{bf}")
                            nc.vector.tensor_scalar_mul(zs[:], z[:], 0.01)
                            nc.vector.tensor_tensor(y1[bf // 16][:, bf % 16, :],
                                                    z[:], zs[:], OP.max)
                    pf2 = [psF.tile([P, 512], F32, tag="fp", bufs=8,
                                    name=f"p2_{nt}_{j}") for j in range(8)]
                    for ft in range(FF // 512):
                        w2c = bp.tile([P, 4, D], R, tag="sd", bufs=2,
                                      name=f"w2_{nt}_{ft}")
                        nc.sync.dma_start(
                            w2c[:], w2_d[ds(ft * 512, 512), :]
                            .rearrange("(fo fp) d -> fp fo d", fp=P)
                            .bitcast(R))
                        for fc in range(4):
                            bf = ft * 4 + fc
                            ysrc = y1[bf // 16][:, bf % 16, :]
                            for tb in range(4):
                                for dt in range(2):
                                    nc.tensor.matmul(
                                        pf2[tb * 2 + dt][:],
                                        ysrc[:, ts(tb, P)],
                                        w2c[:, fc, ds(dt * 512, 512)],
                                        start=(ft == 0 and fc == 0),
                                        stop=(ft == 7 and fc == 3))
                    for tb in range(4):
                        for dt in range(2):
                            rows = ds(nt * 512 + tb * P, P)
                            xc = bp.tile([P, 512], F32, tag="kb", bufs=2,
                                         name=f"xc{nt}_{tb}_{dt}")
                            nc.sync.dma_start(xc[:],
                                              x2pb_d[rows, ds(dt * 512, 512)])
                            og = bp.tile([P, 512], F32, tag="rb", bufs=2,
                                         name=f"og{nt}_{tb}_{dt}")
                            nc.vector.tensor_tensor(og[:], pf2[tb * 2 + dt][:],
                                                    xc[:], OP.add)
                            nc.sync.dma_start(out_d[rows, ds(dt * 512, 512)],
                                              og[:])
    nc.compile()
    return nc


def get_nc():
    global _CACHED_NC
    if _CACHED_NC is None:
        _CACHED_NC = build_nc()
    return _CACHED_NC


def kernel(**inputs):
    nc = get_nc()
    x = np.ascontiguousarray(np.asarray(inputs["x"], dtype=np.float32))
    B = x.shape[0]
    weights = {k: np.ascontiguousarray(np.asarray(v, dtype=np.float32))
               for k, v in inputs.items() if k != "x"}
    in_maps = [dict(weights, x=x[b]) for b in range(B)]
    res = run_bass_kernel_spmd(nc, in_maps, list(range(B)))
    return np.stack([res.results[b]["out"] for b in range(B)], axis=0)


# revision 17
# speedup vs baseline: 10883.8241x; 10883.8241x over previous
"""Trainium2 Bass kernel for a pre-LN transformer block (MHA + FFN).

Data-parallel over batch: 8 NeuronCores, one batch element each.
All matmuls run as float32r (full PE rate at free-dim>=256), storage fp32.
"""
import sys

for _p in ("/opt/trn_rl_repo", "/root/.axon_site/_ro/trn_rl_repo"):
    if _p not in sys.path:
        sys.path.insert(0, _p)

import numpy as np
import concourse.bass as bass
import concourse.tile as tile
from concourse import bacc, mybir
from concourse.bass import ds, ts
from concourse.bass_utils import run_bass_kernel_spmd
from concourse.masks import make_identity

P = 128
N = 1024          # tokens per core (seq len)
D = 1024          # d_emb
H = 16            # heads
HS = 64           # head size
FF = 4096         # ffn hidden
NT = N // P       # 8 token tiles
DB = D // P       # 8 d blocks
EBS = D // P      # 8 e blocks (qkv out features)
NH = 2            # n halves of 512
LN_EPS = 1e-5

F32 = mybir.dt.float32
R = mybir.dt.float32r
AF = mybir.ActivationFunctionType
OP = mybir.AluOpType

_CACHED_NC = None


def build_nc(use_lrelu=True):
    nc = bacc.Bacc("TRN2", target_bir_lowering=False, debug=False, num_devices=8)

    x_d = nc.dram_tensor("x", [N, D], F32, kind="ExternalInput").ap()
    wq_d = nc.dram_tensor("Wq", [H, D, HS], F32, kind="ExternalInput").ap()
    bq_d = nc.dram_tensor("bq", [H, HS], F32, kind="ExternalInput").ap()
    wk_d = nc.dram_tensor("Wk", [H, D, HS], F32, kind="ExternalInput").ap()
    bk_d = nc.dram_tensor("bk", [H, HS], F32, kind="ExternalInput").ap()
    wv_d = nc.dram_tensor("Wv", [H, D, HS], F32, kind="ExternalInput").ap()
    bv_d = nc.dram_tensor("bv", [H, HS], F32, kind="ExternalInput").ap()
    wp_d = nc.dram_tensor("Wproj", [H * HS, D], F32, kind="ExternalInput").ap()
    bp_d = nc.dram_tensor("bproj", [D], F32, kind="ExternalInput").ap()
    w1_d = nc.dram_tensor("W1", [D, FF], F32, kind="ExternalInput").ap()
    b1_d = nc.dram_tensor("b1", [FF], F32, kind="ExternalInput").ap()
    w2_d = nc.dram_tensor("W2", [FF, D], F32, kind="ExternalInput").ap()
    b2_d = nc.dram_tensor("b2", [D], F32, kind="ExternalInput").ap()
    g1_d = nc.dram_tensor("ln1_g", [D], F32, kind="ExternalInput").ap()
    c1_d = nc.dram_tensor("ln1_b", [D], F32, kind="ExternalInput").ap()
    g2_d = nc.dram_tensor("ln2_g", [D], F32, kind="ExternalInput").ap()
    c2_d = nc.dram_tensor("ln2_b", [D], F32, kind="ExternalInput").ap()
    out_d = nc.dram_tensor("out", [N, D], F32, kind="ExternalOutput").ap()
    x2pb_d = nc.dram_tensor("x2pb_scratch", [P, NT, D], F32).ap()

    with tile.TileContext(nc) as tc:
        with tc.tile_pool(name="cn", bufs=1) as cp, \
             tc.tile_pool(name="big", bufs=1) as bp:
            # ---- constants / bias vectors (persistent, tiny) ----
            ident = cp.tile([P, P], F32)
            make_identity(nc, ident[:])
            ones_f = cp.tile([P, 1], F32)
            nc.vector.memset(ones_f[:], 1.0)
            ones64 = cp.tile([1, HS], R)
            nc.vector.tensor_copy(ones64[:],
                                  ones_f[0:1, :].to_broadcast([1, HS]))
            onesP = cp.tile([1, P], R)
            nc.vector.tensor_copy(onesP[:],
                                  ones_f[0:1, :].to_broadcast([1, P]))
            epsv = cp.tile([P, 1], F32)
            nc.vector.memset(epsv[:], LN_EPS)

            # x load first so the big DMA isn't stuck behind the
            # scattered little bias loads
            xsb = bp.tile([P, NT, D], F32, tag="v", name="xsb")
            xr3 = x_d.rearrange("(t p) d -> p t d", p=P)
            for tb in range(NT):
                nc.sync.dma_start(xsb[:, tb, :], xr3[:, tb, :])

            bqv = cp.tile([P, EBS], F32)
            nc.sync.dma_start(bqv[:], bq_d.rearrange("h s -> (h s)")
                              .rearrange("(b p) -> p b", p=P))
            bkv = cp.tile([P, EBS], F32)
            nc.sync.dma_start(bkv[:], bk_d.rearrange("h s -> (h s)")
                              .rearrange("(b p) -> p b", p=P))
            bvv = cp.tile([P, EBS], F32)
            nc.sync.dma_start(bvv[:], bv_d.rearrange("h s -> (h s)")
                              .rearrange("(b p) -> p b", p=P))
            g1v = cp.tile([P, DB], F32)
            nc.sync.dma_start(g1v[:], g1_d.rearrange("(b p) -> p b", p=P))
            c1v = cp.tile([P, DB], F32)
            nc.sync.dma_start(c1v[:], c1_d.rearrange("(b p) -> p b", p=P))
            g2v = cp.tile([P, DB], F32)
            nc.sync.dma_start(g2v[:], g2_d.rearrange("(b p) -> p b", p=P))
            c2v = cp.tile([P, DB], F32)
            nc.sync.dma_start(c2v[:], c2_d.rearrange("(b p) -> p b", p=P))
            b1v = cp.tile([P, FF // P], F32)
            nc.sync.dma_start(b1v[:], b1_d.rearrange("(b p) -> p b", p=P))

            # LN stats scratch (reused for LN2 by tag)
            st_sum = cp.tile([P, NT], F32)
            st_sq = cp.tile([P, NT], F32)
            st_mu = cp.tile([P, NT], F32)
            st_var = cp.tile([P, NT], F32)
            st_rs = cp.tile([P, NT], F32)
            st_nm = cp.tile([P, NT], F32)
            st_vh = cp.tile([P, NT], F32)
            st_t = cp.tile([P, NT], F32)
            st_ih = cp.tile([P, NT], mybir.dt.int32)

            def layernorm_transpose(src, dst, gv, cv, pfx, pspool, trbufs):
                """src: [P, NT, D] token layout (f32) -> dst [P, DB, N] f32r
                feature layout, with affine (gv, cv per-partition) folded into
                the transpose evacuation. Fully per-tb so each token tile
                flows stats -> normalize -> transpose independently."""
                for tb in range(NT):
                    t1 = (tb, tb + 1)
                    nc.vector.reduce_sum(st_sum[:, t1[0]:t1[1]], src[:, tb, :],
                                         axis=mybir.AxisListType.X)
                    sq = bp.tile([P, D], F32, tag="qb", bufs=2,
                                 name=f"sq{tb}")
                    nc.scalar.activation(sq[:], src[:, tb, :], AF.Square,
                                         accum_out=st_sq[:, t1[0]:t1[1]])
                    sm = st_sum[:, t1[0]:t1[1]]
                    var = st_var[:, t1[0]:t1[1]]
                    rs = st_rs[:, t1[0]:t1[1]]
                    nm = st_nm[:, t1[0]:t1[1]]
                    ih = st_ih[:, t1[0]:t1[1]]
                    vh = st_vh[:, t1[0]:t1[1]]
                    tt = st_t[:, t1[0]:t1[1]]
                    i32 = mybir.dt.int32
                    # var = sq/D - (sum/D)^2 + eps   (depth-3 chain)
                    nc.vector.tensor_tensor(tt, sm, sm, OP.mult)
                    nc.vector.tensor_scalar(var, tt, -1.0 / (D * D), LN_EPS,
                                            OP.mult, OP.add)
                    nc.vector.tensor_scalar(var, st_sq[:, t1[0]:t1[1]],
                                            1.0 / D, var, OP.mult, OP.add)
                    # rstd = rsqrt(var), DVE-only (bit hack + 2 Newton steps)
                    # so the ACT engine never needs the sqrt table set
                    nc.vector.tensor_scalar(ih, var.bitcast(i32), 1, None,
                                            OP.arith_shift_right)
                    nc.vector.tensor_scalar(rs.bitcast(i32), ih, -1,
                                            0x5F3759DF, OP.mult, OP.add)
                    nc.vector.tensor_scalar_mul(vh, var, 0.5)
                    for _ in range(2):
                        nc.vector.tensor_tensor(tt, rs, rs, OP.mult)
                        nc.vector.tensor_tensor(tt, tt, vh, OP.mult)
                        nc.vector.tensor_scalar(tt, tt, -1.0, 1.5,
                                                OP.mult, OP.add)
                        nc.vector.tensor_tensor(rs, rs, tt, OP.mult)
                    # nm = -(sum/D)*rstd
                    nc.vector.tensor_tensor(nm, sm, rs, OP.mult)
                    nc.vector.tensor_scalar_mul(nm, nm, -1.0 / D)
                    tnorm = bp.tile([P, D], F32, tag="kb", bufs=2,
                                    name=f"tn{tb}")
                    nc.vector.tensor_scalar(tnorm[:], src[:, tb, :],
                                            rs, nm, OP.mult, OP.add)
                    for db in range(DB):
                        pt = pspool.tile([P, P], F32, tag="tr", bufs=trbufs,
                                         name=f"ptr{pfx}_{tb}_{db}")
                        nc.tensor.transpose(pt[:], tnorm[:, ts(db, P)],
                                            ident[:])
                        nc.vector.tensor_scalar(dst[:, db, ts(tb, P)], pt[:],
                                                gv[:, db:db + 1],
                                                cv[:, db:db + 1],
                                                OP.mult, OP.add)

            # ================= Phase A: LN1 + transpose ====================
            HT = bp.tile([P, DB, N], R, tag="ht", name="HT")
            psAB_cm = tc.tile_pool(name="psAB", bufs=1, space="PSUM")
            psAB = psAB_cm.__enter__()
            layernorm_transpose(xsb, HT, g1v, c1v, "a", psAB, 4)

            # ================= Phase B0: V projection ======================
            # Vaug[p, tb, h, 0:64] = (h_ln @ Wv)[tok, h*64+s]; [..., 64] = 1.0
            Vaug = bp.tile([P, NT, H, HS + 1], R, tag="v", name="Vaug")
            nc.vector.tensor_copy(
                Vaug[:, :, :, HS:HS + 1],
                ones_f[:, None, :].to_broadcast([P, NT, H, 1]))
            if True:
                for eh in range(2):
                    wvt = bp.tile([P, DB, 512], R, tag="se", bufs=2,
                                  name=f"wv{eh}")
                    for do in range(DB):
                        nc.sync.dma_start(
                            wvt[:, do].rearrange("p (h s) -> p h s", s=HS),
                            wv_d[eh * 8:(eh + 1) * 8, ds(do * P, P), :]
                            .rearrange("h dp s -> dp h s")
                            .bitcast(R))
                    for tb in range(NT):
                        pv = psAB.tile([P, 512], F32, tag="qkv", bufs=4,
                                       name=f"pv{eh}_{tb}")
                        for db in range(DB):
                            nc.tensor.matmul(pv[:], HT[:, db, ts(tb, P)],
                                             wvt[:, db, :],
                                             start=(db == 0), stop=(db == DB - 1))
                        nc.vector.tensor_copy(
                            Vaug[:, tb, eh * 8:(eh + 1) * 8, 0:HS],
                            pv[:].rearrange("p (h s) -> p h s", s=HS))

            # ============ Phase BC: Q/K per e-block fused with attention ===
            psAB_cm.__exit__(None, None, None)
            psBC_cm = tc.tile_pool(name="psBC", bufs=1, space="PSUM")
            psBC = psBC_cm.__enter__()
            attnT = bp.tile([P, EBS, N], R, tag="at", name="attnT")
            # prefetch Wproj during attention
            wpa = bp.tile([P, 4, D], R, tag="sd", bufs=2, name="wpa")
            wpb = bp.tile([P, 4, D], R, tag="sd", bufs=2, name="wpb")
            nc.sync.dma_start(
                wpa[:], wp_d[0:512].rearrange("(eo ep) d -> ep eo d", ep=P)
                .bitcast(R))
            nc.sync.dma_start(
                wpb[:], wp_d[512:1024].rearrange("(eo ep) d -> ep eo d", ep=P)
                .bitcast(R))

            if True:
                for eb in range(EBS):
                    wqt = bp.tile([P, DB, P], R, tag="wqk", bufs=2,
                                  name=f"wq{eb}")
                    for do in range(DB):
                        nc.sync.dma_start(
                            wqt[:, do].rearrange("p (h s) -> p h s", s=HS),
                            wq_d[2 * eb:2 * eb + 2, ds(do * P, P), :]
                            .rearrange("h dp s -> dp h s")
                            .bitcast(R))
                    wkt = bp.tile([P, DB, P], R, tag="wqk", bufs=2,
                                  name=f"wk{eb}")
                    for do in range(DB):
                        nc.sync.dma_start(
                            wkt[:, do].rearrange("p (h s) -> p h s", s=HS),
                            wk_d[2 * eb:2 * eb + 2, ds(do * P, P), :]
                            .rearrange("h dp s -> dp h s")
                            .bitcast(R))
                    Qb = bp.tile([P, N], R, tag="qb", bufs=2, name=f"Qb{eb}")
                    Kb = bp.tile([P, N], R, tag="kb", bufs=2, name=f"Kb{eb}")
                    for nh in range(NH):
                        pq = psBC.tile([P, 512], F32, tag="qk", bufs=2,
                                       name=f"pq{eb}_{nh}")
                        for db in range(DB):
                            nc.tensor.matmul(pq[:], wqt[:, db, :],
                                             HT[:, db, ds(nh * 512, 512)],
                                             start=(db == 0), stop=(db == DB - 1))
                        nc.vector.tensor_scalar_add(Qb[:, ds(nh * 512, 512)],
                                                    pq[:], bqv[:, eb:eb + 1])
                        pk = psBC.tile([P, 512], F32, tag="qk", bufs=2,
                                       name=f"pk{eb}_{nh}")
                        for db in range(DB):
                            nc.tensor.matmul(pk[:], wkt[:, db, :],
                                             HT[:, db, ds(nh * 512, 512)],
                                             start=(db == 0), stop=(db == DB - 1))
                        nc.vector.tensor_scalar_add(Kb[:, ds(nh * 512, 512)],
                                                    pk[:], bkv[:, eb:eb + 1])

                    # attention for heads 2eb (partitions 0:64) and
                    # 2eb+1 (partitions 64:128), per n-half of 512
                    for nh in range(NH):
                        pts = [bp.tile([P, NT, 512], R, tag="se", bufs=2,
                                       name=f"PT{eb}_{nh}_{i}")
                               for i in range(2)]
                        # scoresT[m, n] = sum_s K[m,s] Q[n,s]; exp via ACT
                        for mt in range(NT):
                            for i in range(2):
                                base = i * HS
                                pss = psBC.tile([P, 512], F32, tag="sc",
                                                bufs=2, name=f"ps{eb}{nh}{mt}{i}")
                                nc.tensor.matmul(
                                    pss[:],
                                    Kb[base:base + HS, ts(mt, P)],
                                    Qb[base:base + HS, ds(nh * 512, 512)],
                                    start=True, stop=True)
                                nc.scalar.activation(pts[i][:, mt, :], pss[:],
                                                     AF.Exp, scale=0.125)
                        pas = [psBC.tile([HS + 1, 512], F32, tag="at65",
                                         bufs=2, name=f"pa{eb}_{nh}_{i}")
                               for i in range(2)]
                        for mb in range(NT):
                            for i in range(2):
                                nc.tensor.matmul(pas[i][:],
                                                 Vaug[:, mb, 2 * eb + i, :],
                                                 pts[i][:, mb, :],
                                                 start=(mb == 0),
                                                 stop=(mb == NT - 1))
                        for i in range(2):
                            base = i * HS
                            rec = bp.tile([1, 512], R, tag="rb", bufs=2,
                                          name=f"rc{eb}_{nh}_{i}")
                            with nc.allow_low_precision(
                                    reason="f32r is full fp32 bits here"):
                                nc.vector.reciprocal(rec[:],
                                                     pas[i][HS:HS + 1, :])
                            prb = psBC.tile([HS, 512], F32, tag="rbp", bufs=2,
                                            name=f"prb{eb}_{nh}_{i}")
                            nc.tensor.matmul(prb[:], ones64[:], rec[:],
                                             start=True, stop=True)
                            rbs = bp.tile([HS, 512], F32, tag="rb", bufs=2,
                                          name=f"rb{eb}_{nh}_{i}")
                            nc.vector.tensor_copy(rbs[:], prb[:])
                            dstA = attnT[base:base + HS, eb,
                                         ds(nh * 512, 512)]
                            nc.vector.tensor_tensor(dstA, pas[i][0:HS, :],
                                                    rbs[:], OP.mult)
                            nc.vector.tensor_scalar_add(
                                dstA, dstA, bvv[base:base + HS, eb:eb + 1])

            # ================= Phase D: proj + residual ====================
            psBC_cm.__exit__(None, None, None)
            psDE_cm = tc.tile_pool(name="psDE", bufs=1, space="PSUM")
            psDE = psDE_cm.__enter__()
            x2 = bp.tile([P, NT, D], F32, tag="v", name="x2")
            xr = bp.tile([P, NT, D], F32, tag="ht", name="xrl")
            if True:
                # broadcast bproj -> [P, D]
                bprow = bp.tile([1, D], R, tag="kb", bufs=2, name="bprow")
                nc.sync.dma_start(bprow[:], bp_d[None, :].bitcast(R))
                bpB = bp.tile([P, D], F32, tag="qb", bufs=2, name="bpB")
                for dh in range(2):
                    pbb = psDE.tile([P, 512], F32, tag="trb", bufs=1,
                                    name=f"pbb{dh}")
                    nc.tensor.matmul(pbb[:], onesP[:],
                                     bprow[:, ds(dh * 512, 512)],
                                     start=True, stop=True)
                    nc.vector.tensor_copy(bpB[:, ds(dh * 512, 512)], pbb[:])
                for tb in range(NT):
                    nc.sync.dma_start(xr[:, tb, :], xr3[:, tb, :])
                    nc.vector.tensor_tensor(xr[:, tb, :], xr[:, tb, :],
                                            bpB[:], OP.add)
                for tb in range(NT):
                    for dt in range(2):
                        pp = psDE.tile([P, 512], F32, tag="pj", bufs=4,
                                       name=f"pp{tb}_{dt}")
                        for g in range(2):
                            wp = wpa if g == 0 else wpb
                            for eo in range(4):
                                nc.tensor.matmul(
                                    pp[:], attnT[:, g * 4 + eo, ts(tb, P)],
                                    wp[:, eo, ds(dt * 512, 512)],
                                    start=(g == 0 and eo == 0),
                                    stop=(g == 1 and eo == 3))
                        nc.vector.tensor_tensor(x2[:, tb, ds(dt * 512, 512)],
                                                pp[:],
                                                xr[:, tb, ds(dt * 512, 512)],
                                                OP.add)

            # ================= Phase E: LN2, transpose, stash x2+b2 ========
            H2T = bp.tile([P, DB, N], R, tag="ht", name="H2T")
            if True:
                layernorm_transpose(x2, H2T, g2v, c2v, "e", psDE, 3)
                # broadcast b2 -> [P, D]; x2 += b2B; stash to DRAM
                b2row = bp.tile([1, D], R, tag="kb", bufs=2, name="b2row")
                nc.sync.dma_start(b2row[:], b2_d[None, :].bitcast(R))
                b2B = bp.tile([P, D], F32, tag="qb", bufs=2, name="b2B")
                for dh in range(2):
                    pb2 = psDE.tile([P, 512], F32, tag="trb", bufs=1,
                                    name=f"pb2{dh}")
                    nc.tensor.matmul(pb2[:], onesP[:],
                                     b2row[:, ds(dh * 512, 512)],
                                     start=True, stop=True)
                    nc.vector.tensor_copy(b2B[:, ds(dh * 512, 512)], pb2[:])
                for tb in range(NT):
                    nc.vector.tensor_tensor(x2[:, tb, :], x2[:, tb, :],
                                            b2B[:], OP.add)
                    nc.sync.dma_start(x2pb_d[:, tb, :], x2[:, tb, :])

            # ================= Phase F: FFN ================================
            psDE_cm.__exit__(None, None, None)
            psF_cm = tc.tile_pool(name="psF", bufs=1, space="PSUM")
            psF = psF_cm.__enter__()
            if True:
                w1pre = bp.tile([P, DB, 512], R, tag="se", bufs=2,
                                name="w1pre")
                nc.sync.dma_start(
                    w1pre[:],
                    w1_d[:, ds(0, 512)]
                    .rearrange("(do dp) f -> dp do f", dp=P)
                    .bitcast(R))
                for nt in range(NH):
                    y1 = [bp.tile([P, 16, 512], R, tag=tg, name=f"y1{nt}{tg}")
                          for tg in ("v", "at")]
                    xcf = bp.tile([P, 4, D], F32, tag="se", bufs=2,
                                  name=f"xcf{nt}")
                    nc.sync.dma_start(xcf[:], x2pb_d[:, nt * 4:(nt + 1) * 4, :])
                    for ft in range(FF // 512):
                        if ft == 0:
                            w1c = w1pre
                        else:
                            w1c = bp.tile([P, DB, 512], R, tag="sd", bufs=2,
                                          name=f"w1_{nt}_{ft}")
                            nc.sync.dma_start(
                                w1c[:],
                                w1_d[:, ds(ft * 512, 512)]
                                .rearrange("(do dp) f -> dp do f", dp=P)
                                .bitcast(R))
                        for fc in range(4):
                            bf = ft * 4 + fc
                            p1 = psF.tile([P, 512], F32, tag="fp", bufs=8,
                                          name=f"p1_{nt}_{bf}")
                            for db in range(DB):
                                nc.tensor.matmul(
                                    p1[:], w1c[:, db, ds(fc * P, P)],
                                    H2T[:, db, ds(nt * 512, 512)],
                                    start=(db == 0), stop=(db == DB - 1))
                            ydst = y1[bf // 16][:, bf % 16, :]
                            if use_lrelu:
                                nc.scalar.activation(ydst, p1[:], AF.Prelu,
                                                     bias=b1v[:, bf:bf + 1],
                                                     alpha=0.01)
                            else:
                                z = bp.tile([P, 512], F32, tag="qb", bufs=2,
                                            name=f"z{nt}_{bf}")
                                nc.scalar.activation(z[:], p1[:], AF.Identity,
                                                     bias=b1v[:, bf:bf + 1])
                                zs = bp.tile([P, 512], F32, tag="rb", bufs=2,
                                             name=f"zs{nt}_{bf}")
                                nc.vector.tensor_scalar_mul(zs[:], z[:], 0.01)
                                nc.vector.tensor_tensor(ydst, z[:], zs[:],
                                                        OP.max)
                    pf2 = [psF.tile([P, 512], F32, tag="fp", bufs=8,
                                    name=f"p2_{nt}_{j}") for j in range(8)]
                    for ft in range(FF // 512):
                        w2c = bp.tile([P, 4, D], R, tag="sd", bufs=2,
                                      name=f"w2_{nt}_{ft}")
                        nc.sync.dma_start(
                            w2c[:], w2_d[ds(ft * 512, 512), :]
                            .rearrange("(fo fp) d -> fp fo d", fp=P)
                            .bitcast(R))
                        for fc in range(4):
                            bf = ft * 4 + fc
                            ysrc = y1[bf // 16][:, bf % 16, :]
                            for tb in range(4):
                                for dt in range(2):
                                    nc.tensor.matmul(
                                        pf2[tb * 2 + dt][:],
                                        ysrc[:, ts(tb, P)],
                                        w2c[:, fc, ds(dt * 512, 512)],
                                        start=(ft == 0 and fc == 0),
                                        stop=(ft == 7 and fc == 3))
                    for tb in range(4):
                        for dt in range(2):
                            rows = ds(nt * 512 + tb * P, P)
                            og = bp.tile([P, 512], F32, tag="rb", bufs=2,
                                         name=f"og{nt}_{tb}_{dt}")
                            nc.vector.tensor_tensor(og[:], pf2[tb * 2 + dt][:],
                                                    xcf[:, tb, ds(dt * 512, 512)],
                                                    OP.add)
                            nc.sync.dma_start(out_d[rows, ds(dt * 512, 512)],
                                              og[:])
            psF_cm.__exit__(None, None, None)
    nc.compile()
    return nc


def get_nc():
    global _CACHED_NC
    if _CACHED_NC is None:
        _CACHED_NC = build_nc()
    return _CACHED_NC


def kernel(**inputs):
    nc = get_nc()
    x = np.ascontiguousarray(np.asarray(inputs["x"], dtype=np.float32))
    B = x.shape[0]
    weights = {k: np.ascontiguousarray(np.asarray(v, dtype=np.float32))
               for k, v in inputs.items() if k != "x"}
    in_maps = [dict(weights, x=x[b]) for b in range(B)]
    res = run_bass_kernel_spmd(nc, in_maps, list(range(B)))
    return np.stack([res.results[b]["out"] for b in range(B)], axis=0)


# revision 19
# speedup vs baseline: 10958.2666x; 1.0068x over previous
"""Trainium2 Bass kernel for a pre-LN transformer block (MHA + FFN).

Data-parallel over batch: 8 NeuronCores, one batch element each.
All matmuls run as float32r (full PE rate at free-dim>=256), storage fp32.
"""
import sys

for _p in ("/opt/trn_rl_repo", "/root/.axon_site/_ro/trn_rl_repo"):
    if _p not in sys.path:
        sys.path.insert(0, _p)

import numpy as np
import concourse.bass as bass
import concourse.tile as tile
from concourse import bacc, mybir
from concourse.bass import ds, ts
from concourse.bass_utils import run_bass_kernel_spmd
from concourse.masks import make_identity

P = 128
N = 1024          # tokens per core (seq len)
D = 1024          # d_emb
H = 16            # heads
HS = 64           # head size
FF = 4096         # ffn hidden
NT = N // P       # 8 token tiles
DB = D // P       # 8 d blocks
EBS = D // P      # 8 e blocks (qkv out features)
NH = 2            # n halves of 512
LN_EPS = 1e-5

F32 = mybir.dt.float32
R = mybir.dt.float32r
AF = mybir.ActivationFunctionType
OP = mybir.AluOpType

_CACHED_NC = None


def build_nc(use_lrelu=True):
    nc = bacc.Bacc("TRN2", target_bir_lowering=False, debug=False, num_devices=8)

    x_d = nc.dram_tensor("x", [N, D], F32, kind="ExternalInput").ap()
    wq_d = nc.dram_tensor("Wq", [H, D, HS], F32, kind="ExternalInput").ap()
    bq_d = nc.dram_tensor("bq", [H, HS], F32, kind="ExternalInput").ap()
    wk_d = nc.dram_tensor("Wk", [H, D, HS], F32, kind="ExternalInput").ap()
    bk_d = nc.dram_tensor("bk", [H, HS], F32, kind="ExternalInput").ap()
    wv_d = nc.dram_tensor("Wv", [H, D, HS], F32, kind="ExternalInput").ap()
    bv_d = nc.dram_tensor("bv", [H, HS], F32, kind="ExternalInput").ap()
    wp_d = nc.dram_tensor("Wproj", [H * HS, D], F32, kind="ExternalInput").ap()
    bp_d = nc.dram_tensor("bproj", [D], F32, kind="ExternalInput").ap()
    w1_d = nc.dram_tensor("W1", [D, FF], F32, kind="ExternalInput").ap()
    b1_d = nc.dram_tensor("b1", [FF], F32, kind="ExternalInput").ap()
    w2_d = nc.dram_tensor("W2", [FF, D], F32, kind="ExternalInput").ap()
    b2_d = nc.dram_tensor("b2", [D], F32, kind="ExternalInput").ap()
    g1_d = nc.dram_tensor("ln1_g", [D], F32, kind="ExternalInput").ap()
    c1_d = nc.dram_tensor("ln1_b", [D], F32, kind="ExternalInput").ap()
    g2_d = nc.dram_tensor("ln2_g", [D], F32, kind="ExternalInput").ap()
    c2_d = nc.dram_tensor("ln2_b", [D], F32, kind="ExternalInput").ap()
    out_d = nc.dram_tensor("out", [N, D], F32, kind="ExternalOutput").ap()
    x2pb_d = nc.dram_tensor("x2pb_scratch", [P, NT, D], F32).ap()

    with tile.TileContext(nc) as tc:
        with tc.tile_pool(name="cn", bufs=1) as cp, \
             tc.tile_pool(name="big", bufs=1) as bp:
            # ---- constants / bias vectors (persistent, tiny) ----
            ident = cp.tile([P, P], F32)
            make_identity(nc, ident[:])
            ones_f = cp.tile([P, 1], F32)
            nc.vector.memset(ones_f[:], 1.0)
            ones64 = cp.tile([1, HS], R)
            nc.vector.tensor_copy(ones64[:],
                                  ones_f[0:1, :].to_broadcast([1, HS]))
            onesP = cp.tile([1, P], R)
            nc.vector.tensor_copy(onesP[:],
                                  ones_f[0:1, :].to_broadcast([1, P]))
            epsv = cp.tile([P, 1], F32)
            nc.vector.memset(epsv[:], LN_EPS)

            # x load first so the big DMA isn't stuck behind the
            # scattered little bias loads
            xsb = bp.tile([P, NT, D], F32, tag="at", name="xsb")
            xr3 = x_d.rearrange("(t p) d -> p t d", p=P)
            for tb in range(NT):
                nc.sync.dma_start(xsb[:, tb, :], xr3[:, tb, :])

            bqv = cp.tile([P, EBS], F32)
            nc.sync.dma_start(bqv[:], bq_d.rearrange("h s -> (h s)")
                              .rearrange("(b p) -> p b", p=P))
            bkv = cp.tile([P, EBS], F32)
            nc.sync.dma_start(bkv[:], bk_d.rearrange("h s -> (h s)")
                              .rearrange("(b p) -> p b", p=P))
            bvv = cp.tile([P, EBS], F32)
            nc.sync.dma_start(bvv[:], bv_d.rearrange("h s -> (h s)")
                              .rearrange("(b p) -> p b", p=P))
            g1v = cp.tile([P, DB], F32)
            nc.sync.dma_start(g1v[:], g1_d.rearrange("(b p) -> p b", p=P))
            c1v = cp.tile([P, DB], F32)
            nc.sync.dma_start(c1v[:], c1_d.rearrange("(b p) -> p b", p=P))
            g2v = cp.tile([P, DB], F32)
            nc.sync.dma_start(g2v[:], g2_d.rearrange("(b p) -> p b", p=P))
            c2v = cp.tile([P, DB], F32)
            nc.sync.dma_start(c2v[:], c2_d.rearrange("(b p) -> p b", p=P))
            b1v = cp.tile([P, FF // P], F32)
            nc.sync.dma_start(b1v[:], b1_d.rearrange("(b p) -> p b", p=P))

            # LN stats scratch (reused for LN2 by tag)
            st_sum = cp.tile([P, NT], F32)
            st_sq = cp.tile([P, NT], F32)
            st_mu = cp.tile([P, NT], F32)
            st_var = cp.tile([P, NT], F32)
            st_rs = cp.tile([P, NT], F32)
            st_nm = cp.tile([P, NT], F32)
            st_vh = cp.tile([P, NT], F32)
            st_t = cp.tile([P, NT], F32)
            st_ih = cp.tile([P, NT], mybir.dt.int32)

            def layernorm_transpose(src, dst, gv, cv, pfx, pspool, trbufs,
                                    after_tb=None):
                """src: [P, NT, D] token layout (f32) -> dst [P, DB, N] f32r
                feature layout, with affine (gv, cv per-partition) folded into
                the transpose evacuation. Fully per-tb so each token tile
                flows stats -> normalize -> transpose independently."""
                for tb in range(NT):
                    t1 = (tb, tb + 1)
                    nc.vector.reduce_sum(st_sum[:, t1[0]:t1[1]], src[:, tb, :],
                                         axis=mybir.AxisListType.X)
                    sq = bp.tile([P, D], F32, tag="qb", bufs=2,
                                 name=f"sq{tb}")
                    nc.scalar.activation(sq[:], src[:, tb, :], AF.Square,
                                         accum_out=st_sq[:, t1[0]:t1[1]])
                    sm = st_sum[:, t1[0]:t1[1]]
                    var = st_var[:, t1[0]:t1[1]]
                    rs = st_rs[:, t1[0]:t1[1]]
                    nm = st_nm[:, t1[0]:t1[1]]
                    ih = st_ih[:, t1[0]:t1[1]]
                    vh = st_vh[:, t1[0]:t1[1]]
                    tt = st_t[:, t1[0]:t1[1]]
                    i32 = mybir.dt.int32
                    # var = sq/D - (sum/D)^2 + eps   (depth-3 chain)
                    nc.vector.tensor_tensor(tt, sm, sm, OP.mult)
                    nc.vector.tensor_scalar(var, tt, -1.0 / (D * D), LN_EPS,
                                            OP.mult, OP.add)
                    nc.vector.tensor_scalar(var, st_sq[:, t1[0]:t1[1]],
                                            1.0 / D, var, OP.mult, OP.add)
                    # rstd = rsqrt(var), DVE-only (bit hack + 2 Newton steps)
                    # so the ACT engine never needs the sqrt table set
                    nc.vector.tensor_scalar(ih, var.bitcast(i32), 1, None,
                                            OP.arith_shift_right)
                    nc.vector.tensor_scalar(rs.bitcast(i32), ih, -1,
                                            0x5F3759DF, OP.mult, OP.add)
                    nc.vector.tensor_scalar_mul(vh, var, 0.5)
                    for _ in range(2):
                        nc.vector.tensor_tensor(tt, rs, rs, OP.mult)
                        nc.vector.tensor_tensor(tt, tt, vh, OP.mult)
                        nc.vector.tensor_scalar(tt, tt, -1.0, 1.5,
                                                OP.mult, OP.add)
                        nc.vector.tensor_tensor(rs, rs, tt, OP.mult)
                    # nm = -(sum/D)*rstd
                    nc.vector.tensor_tensor(nm, sm, rs, OP.mult)
                    nc.vector.tensor_scalar_mul(nm, nm, -1.0 / D)
                    tnorm = bp.tile([P, D], F32, tag="kb", bufs=2,
                                    name=f"tn{tb}")
                    nc.vector.tensor_scalar(tnorm[:], src[:, tb, :],
                                            rs, nm, OP.mult, OP.add)
                    for db in range(DB):
                        pt = pspool.tile([P, P], F32, tag="tr", bufs=trbufs,
                                         name=f"ptr{pfx}_{tb}_{db}")
                        nc.tensor.transpose(pt[:], tnorm[:, ts(db, P)],
                                            ident[:])
                        nc.vector.tensor_scalar(dst[:, db, ts(tb, P)], pt[:],
                                                gv[:, db:db + 1],
                                                cv[:, db:db + 1],
                                                OP.mult, OP.add)
                    if after_tb is not None:
                        after_tb(tb)

            # ================= Phase A: LN1 + transpose ====================
            HT = bp.tile([P, DB, N], R, tag="ht", name="HT")
            psAB_cm = tc.tile_pool(name="psAB", bufs=1, space="PSUM")
            psAB = psAB_cm.__enter__()
            layernorm_transpose(xsb, HT, g1v, c1v, "a", psAB, 4)

            # ================= Phase B0: V projection ======================
            Vaug = bp.tile([P, NT, H, HS + 1], R, tag="v", name="Vaug")
            nc.vector.tensor_copy(
                Vaug[:, :, :, HS:HS + 1],
                ones_f[:, None, :].to_broadcast([P, NT, H, 1]))
            if True:
                for eh in range(2):
                    wvt = bp.tile([P, DB, 512], R, tag="se", bufs=2,
                                  name=f"wv{eh}")
                    for do in range(DB):
                        nc.sync.dma_start(
                            wvt[:, do].rearrange("p (h s) -> p h s", s=HS),
                            wv_d[eh * 8:(eh + 1) * 8, ds(do * P, P), :]
                            .rearrange("h dp s -> dp h s")
                            .bitcast(R))
                    for tb in range(NT):
                        pv = psAB.tile([P, 512], F32, tag="qkv", bufs=4,
                                       name=f"pv{eh}_{tb}")
                        for db in range(DB):
                            nc.tensor.matmul(pv[:], HT[:, db, ts(tb, P)],
                                             wvt[:, db, :],
                                             start=(db == 0), stop=(db == DB - 1))
                        nc.scalar.activation(
                            Vaug[:, tb, eh * 8:(eh + 1) * 8, 0:HS],
                            pv[:].rearrange("p (h s) -> p h s", s=HS), AF.Copy)

            # ============ Phase BC: Q/K per e-block fused with attention ===
            psAB_cm.__exit__(None, None, None)
            psBC_cm = tc.tile_pool(name="psBC", bufs=1, space="PSUM")
            psBC = psBC_cm.__enter__()
            attnT = bp.tile([P, EBS, N], R, tag="at", name="attnT")
            # prefetch Wproj during attention
            wpa = bp.tile([P, 4, D], R, tag="sd", bufs=2, name="wpa")
            wpb = bp.tile([P, 4, D], R, tag="sd", bufs=2, name="wpb")
            nc.sync.dma_start(
                wpa[:], wp_d[0:512].rearrange("(eo ep) d -> ep eo d", ep=P)
                .bitcast(R))
            nc.sync.dma_start(
                wpb[:], wp_d[512:1024].rearrange("(eo ep) d -> ep eo d", ep=P)
                .bitcast(R))

            if True:
                for eb in range(EBS):
                    wqt = bp.tile([P, DB, P], R, tag="wqk", bufs=2,
                                  name=f"wq{eb}")
                    for do in range(DB):
                        nc.sync.dma_start(
                            wqt[:, do].rearrange("p (h s) -> p h s", s=HS),
                            wq_d[2 * eb:2 * eb + 2, ds(do * P, P), :]
                            .rearrange("h dp s -> dp h s")
                            .bitcast(R))
                    wkt = bp.tile([P, DB, P], R, tag="wqk", bufs=2,
                                  name=f"wk{eb}")
                    for do in range(DB):
                        nc.sync.dma_start(
                            wkt[:, do].rearrange("p (h s) -> p h s", s=HS),
                            wk_d[2 * eb:2 * eb + 2, ds(do * P, P), :]
                            .rearrange("h dp s -> dp h s")
                            .bitcast(R))
                    Qb = bp.tile([P, N], R, tag="qb", bufs=2, name=f"Qb{eb}")
                    Kb = bp.tile([P, N], R, tag="kb", bufs=2, name=f"Kb{eb}")
                    for nh in range(NH):
                        pq = psBC.tile([P, 512], F32, tag="qk", bufs=2,
                                       name=f"pq{eb}_{nh}")
                        for db in range(DB):
                            nc.tensor.matmul(pq[:], wqt[:, db, :],
                                             HT[:, db, ds(nh * 512, 512)],
                                             start=(db == 0), stop=(db == DB - 1))
                        nc.vector.tensor_scalar_add(Qb[:, ds(nh * 512, 512)],
                                                    pq[:], bqv[:, eb:eb + 1])
                        pk = psBC.tile([P, 512], F32, tag="qk", bufs=2,
                                       name=f"pk{eb}_{nh}")
                        for db in range(DB):
                            nc.tensor.matmul(pk[:], wkt[:, db, :],
                                             HT[:, db, ds(nh * 512, 512)],
                                             start=(db == 0), stop=(db == DB - 1))
                        nc.vector.tensor_scalar_add(Kb[:, ds(nh * 512, 512)],
                                                    pk[:], bkv[:, eb:eb + 1])

                    # attention for heads 2eb (partitions 0:64) and
                    # 2eb+1 (partitions 64:128), per n-half of 512
                    for nh in range(NH):
                        pts = [bp.tile([P, NT, 512], R, tag="se", bufs=2,
                                       name=f"PT{eb}_{nh}_{i}")
                               for i in range(2)]
                        # scoresT[m, n] = sum_s K[m,s] Q[n,s]; exp via ACT
                        for mt in range(NT):
                            for i in range(2):
                                base = i * HS
                                pss = psBC.tile([P, 512], F32, tag="sc",
                                                bufs=2, name=f"ps{eb}{nh}{mt}{i}")
                                nc.tensor.matmul(
                                    pss[:],
                                    Kb[base:base + HS, ts(mt, P)],
                                    Qb[base:base + HS, ds(nh * 512, 512)],
                                    start=True, stop=True)
                                nc.scalar.activation(pts[i][:, mt, :], pss[:],
                                                     AF.Exp, scale=0.125)
                        pas = [psBC.tile([HS + 1, 512], F32, tag="at65",
                                         bufs=2, name=f"pa{eb}_{nh}_{i}")
                               for i in range(2)]
                        for mb in range(NT):
                            for i in range(2):
                                nc.tensor.matmul(pas[i][:],
                                                 Vaug[:, mb, 2 * eb + i, :],
                                                 pts[i][:, mb, :],
                                                 start=(mb == 0),
                                                 stop=(mb == NT - 1))
                        for i in range(2):
                            base = i * HS
                            rec = bp.tile([1, 512], R, tag="rb", bufs=2,
                                          name=f"rc{eb}_{nh}_{i}")
                            with nc.allow_low_precision(
                                    reason="f32r is full fp32 bits here"):
                                nc.vector.reciprocal(rec[:],
                                                     pas[i][HS:HS + 1, :])
                            prb = psBC.tile([HS, 512], F32, tag="rbp", bufs=2,
                                            name=f"prb{eb}_{nh}_{i}")
                            nc.tensor.matmul(prb[:], ones64[:], rec[:],
                                             start=True, stop=True)
                            rbs = bp.tile([HS, 512], F32, tag="rb", bufs=2,
                                          name=f"rb{eb}_{nh}_{i}")
                            nc.vector.tensor_copy(rbs[:], prb[:])
                            dstA = attnT[base:base + HS, eb,
                                         ds(nh * 512, 512)]
                            nc.vector.tensor_tensor(dstA, pas[i][0:HS, :],
                                                    rbs[:], OP.mult)
                            nc.vector.tensor_scalar_add(
                                dstA, dstA, bvv[base:base + HS, eb:eb + 1])

            # w1(ft0) prefetch into "se" (frees at end of attention)
            w1pre = bp.tile([P, DB, 512], R, tag="se", bufs=2,
                            name="w1pre")
            nc.sync.dma_start(
                w1pre[:],
                w1_d[:, ds(0, 512)]
                .rearrange("(do dp) f -> dp do f", dp=P)
                .bitcast(R))

            # ================= Phase D: proj + residual ====================
            psBC_cm.__exit__(None, None, None)
            psDE_cm = tc.tile_pool(name="psDE", bufs=1, space="PSUM")
            psDE = psDE_cm.__enter__()
            x2 = bp.tile([P, NT, D], F32, tag="v", name="x2")
            xr = bp.tile([P, NT, D], F32, tag="ht", name="xrl")
            if True:
                # broadcast bproj -> [P, D]
                bprow = bp.tile([1, D], R, tag="kb", bufs=2, name="bprow")
                nc.sync.dma_start(bprow[:], bp_d[None, :].bitcast(R))
                bpB = bp.tile([P, D], F32, tag="qb", bufs=2, name="bpB")
                for dh in range(2):
                    pbb = psDE.tile([P, 512], F32, tag="trb", bufs=1,
                                    name=f"pbb{dh}")
                    nc.tensor.matmul(pbb[:], onesP[:],
                                     bprow[:, ds(dh * 512, 512)],
                                     start=True, stop=True)
                    nc.vector.tensor_copy(bpB[:, ds(dh * 512, 512)], pbb[:])
                for tb in range(NT):
                    nc.sync.dma_start(xr[:, tb, :], xr3[:, tb, :])
                    nc.vector.tensor_tensor(xr[:, tb, :], xr[:, tb, :],
                                            bpB[:], OP.add)
                for tb in range(NT):
                    for dt in range(2):
                        pp = psDE.tile([P, 512], F32, tag="pj", bufs=4,
                                       name=f"pp{tb}_{dt}")
                        for g in range(2):
                            wp = wpa if g == 0 else wpb
                            for eo in range(4):
                                nc.tensor.matmul(
                                    pp[:], attnT[:, g * 4 + eo, ts(tb, P)],
                                    wp[:, eo, ds(dt * 512, 512)],
                                    start=(g == 0 and eo == 0),
                                    stop=(g == 1 and eo == 3))
                        nc.vector.tensor_tensor(x2[:, tb, ds(dt * 512, 512)],
                                                pp[:],
                                                xr[:, tb, ds(dt * 512, 512)],
                                                OP.add)

            # ================= Phase E: LN2, transpose, stash x2+b2 ========
            H2T = bp.tile([P, DB, N], R, tag="ht", name="H2T")
            if True:
                layernorm_transpose(x2, H2T, g2v, c2v, "e", psDE, 3)
                # broadcast b2 -> [P, D]; x2 += b2B; stash to DRAM
                b2row = bp.tile([1, D], R, tag="kb", bufs=2, name="b2row")
                nc.sync.dma_start(b2row[:], b2_d[None, :].bitcast(R))
                b2B = bp.tile([P, D], F32, tag="qb", bufs=2, name="b2B")
                for dh in range(2):
                    pb2 = psDE.tile([P, 512], F32, tag="trb", bufs=1,
                                    name=f"pb2{dh}")
                    nc.tensor.matmul(pb2[:], onesP[:],
                                     b2row[:, ds(dh * 512, 512)],
                                     start=True, stop=True)
                    nc.vector.tensor_copy(b2B[:, ds(dh * 512, 512)], pb2[:])
                for tb in range(NT):
                    nc.vector.tensor_tensor(x2[:, tb, :], x2[:, tb, :],
                                            b2B[:], OP.add)
                    nc.sync.dma_start(x2pb_d[:, tb, :], x2[:, tb, :])

            # ================= Phase F: FFN ================================
            psDE_cm.__exit__(None, None, None)
            psF_cm = tc.tile_pool(name="psF", bufs=1, space="PSUM")
            psF = psF_cm.__enter__()
            if True:
                for nt in range(NH):
                    y1 = [bp.tile([P, 16, 512], R, tag=tg, name=f"y1{nt}{tg}")
                          for tg in ("v", "at")]
                    xcf = bp.tile([P, 4, D], F32, tag="se", bufs=2,
                                  name=f"xcf{nt}")
                    nc.sync.dma_start(xcf[:], x2pb_d[:, nt * 4:(nt + 1) * 4, :])
                    for ft in range(FF // 512):
                        if ft == 0:
                            w1c = w1pre
                        else:
                            w1c = bp.tile([P, DB, 512], R, tag="sd", bufs=2,
                                          name=f"w1_{nt}_{ft}")
                            nc.sync.dma_start(
                                w1c[:],
                                w1_d[:, ds(ft * 512, 512)]
                                .rearrange("(do dp) f -> dp do f", dp=P)
                                .bitcast(R))
                        for fc in range(4):
                            bf = ft * 4 + fc
                            p1 = psF.tile([P, 512], F32, tag="fp", bufs=8,
                                          name=f"p1_{nt}_{bf}")
                            for db in range(DB):
                                nc.tensor.matmul(
                                    p1[:], w1c[:, db, ds(fc * P, P)],
                                    H2T[:, db, ds(nt * 512, 512)],
                                    start=(db == 0), stop=(db == DB - 1))
                            ydst = y1[bf // 16][:, bf % 16, :]
                            if use_lrelu:
                                nc.scalar.activation(ydst, p1[:], AF.Prelu,
                                                     bias=b1v[:, bf:bf + 1],
                                                     alpha=0.01)
                            else:
                                z = bp.tile([P, 512], F32, tag="qb", bufs=2,
                                            name=f"z{nt}_{bf}")
                                nc.scalar.activation(z[:], p1[:], AF.Identity,
                                                     bias=b1v[:, bf:bf + 1])
                                zs = bp.tile([P, 512], F32, tag="rb", bufs=2,
                                             name=f"zs{nt}_{bf}")
                                nc.vector.tensor_scalar_mul(zs[:], z[:], 0.01)
                                nc.vector.tensor_tensor(ydst, z[:], zs[:],
                                                        OP.max)
                    pf2 = [psF.tile([P, 512], F32, tag="fp", bufs=8,
                                    name=f"p2_{nt}_{j}") for j in range(8)]
                    for ft in range(FF // 512):
                        w2c = bp.tile([P, 4, D], R, tag="sd", bufs=2,
                                      name=f"w2_{nt}_{ft}")
                        nc.sync.dma_start(
                            w2c[:], w2_d[ds(ft * 512, 512), :]
                            .rearrange("(fo fp) d -> fp fo d", fp=P)
                            .bitcast(R))
                        for fc in range(4):
                            bf = ft * 4 + fc
                            ysrc = y1[bf // 16][:, bf % 16, :]
                            for tb in range(4):
                                for dt in range(2):
                                    nc.tensor.matmul(
                                        pf2[tb * 2 + dt][:],
                                        ysrc[:, ts(tb, P)],
                                        w2c[:, fc, ds(dt * 512, 512)],
                                        start=(ft == 0 and fc == 0),
                                        stop=(ft == 7 and fc == 3))
                    for tb in range(4):
                        for dt in range(2):
                            rows = ds(nt * 512 + tb * P, P)
                            og = bp.tile([P, 512], F32, tag="rb", bufs=2,
                                         name=f"og{nt}_{tb}_{dt}")
                            nc.vector.tensor_tensor(og[:], pf2[tb * 2 + dt][:],
                                                    xcf[:, tb, ds(dt * 512, 512)],
                                                    OP.add)
                            nc.sync.dma_start(out_d[rows, ds(dt * 512, 512)],
                                              og[:])
            psF_cm.__exit__(None, None, None)
    nc.compile()
    return nc


def get_nc():
    global _CACHED_NC
    if _CACHED_NC is None:
        _CACHED_NC = build_nc()
    return _CACHED_NC


def kernel(**inputs):
    nc = get_nc()
    x = np.ascontiguousarray(np.asarray(inputs["x"], dtype=np.float32))
    B = x.shape[0]
    weights = {k: np.ascontiguousarray(np.asarray(v, dtype=np.float32))
               for k, v in inputs.items() if k != "x"}
    in_maps = [dict(weights, x=x[b]) for b in range(B)]
    res = run_bass_kernel_spmd(nc, in_maps, list(range(B)))
    return np.stack([res.results[b]["out"] for b in range(B)], axis=0)


# revision 20
# speedup vs baseline: 11025.6690x; 1.0062x over previous
"""Trainium2 Bass kernel for a pre-LN transformer block (MHA + FFN).

Data-parallel over batch: 8 NeuronCores, one batch element each.
All matmuls run as float32r (full PE rate at free-dim>=256), storage fp32.
"""
import sys

for _p in ("/opt/trn_rl_repo", "/root/.axon_site/_ro/trn_rl_repo"):
    if _p not in sys.path:
        sys.path.insert(0, _p)

import numpy as np
import concourse.bass as bass
import concourse.tile as tile
from concourse import bacc, mybir
from concourse.bass import ds, ts
from concourse.bass_utils import run_bass_kernel_spmd
from concourse.masks import make_identity

P = 128
N = 1024          # tokens per core (seq len)
D = 1024          # d_emb
H = 16            # heads
HS = 64           # head size
FF = 4096         # ffn hidden
NT = N // P       # 8 token tiles
DB = D // P       # 8 d blocks
EBS = D // P      # 8 e blocks (qkv out features)
NH = 2            # n halves of 512
LN_EPS = 1e-5

F32 = mybir.dt.float32
R = mybir.dt.float32r
AF = mybir.ActivationFunctionType
OP = mybir.AluOpType

_CACHED_NC = None


def build_nc(use_lrelu=True):
    nc = bacc.Bacc("TRN2", target_bir_lowering=False, debug=False, num_devices=8)

    x_d = nc.dram_tensor("x", [N, D], F32, kind="ExternalInput").ap()
    wq_d = nc.dram_tensor("Wq", [H, D, HS], F32, kind="ExternalInput").ap()
    bq_d = nc.dram_tensor("bq", [H, HS], F32, kind="ExternalInput").ap()
    wk_d = nc.dram_tensor("Wk", [H, D, HS], F32, kind="ExternalInput").ap()
    bk_d = nc.dram_tensor("bk", [H, HS], F32, kind="ExternalInput").ap()
    wv_d = nc.dram_tensor("Wv", [H, D, HS], F32, kind="ExternalInput").ap()
    bv_d = nc.dram_tensor("bv", [H, HS], F32, kind="ExternalInput").ap()
    wp_d = nc.dram_tensor("Wproj", [H * HS, D], F32, kind="ExternalInput").ap()
    bp_d = nc.dram_tensor("bproj", [D], F32, kind="ExternalInput").ap()
    w1_d = nc.dram_tensor("W1", [D, FF], F32, kind="ExternalInput").ap()
    b1_d = nc.dram_tensor("b1", [FF], F32, kind="ExternalInput").ap()
    w2_d = nc.dram_tensor("W2", [FF, D], F32, kind="ExternalInput").ap()
    b2_d = nc.dram_tensor("b2", [D], F32, kind="ExternalInput").ap()
    g1_d = nc.dram_tensor("ln1_g", [D], F32, kind="ExternalInput").ap()
    c1_d = nc.dram_tensor("ln1_b", [D], F32, kind="ExternalInput").ap()
    g2_d = nc.dram_tensor("ln2_g", [D], F32, kind="ExternalInput").ap()
    c2_d = nc.dram_tensor("ln2_b", [D], F32, kind="ExternalInput").ap()
    out_d = nc.dram_tensor("out", [N, D], F32, kind="ExternalOutput").ap()
    x2pb_d = nc.dram_tensor("x2pb_scratch", [P, NT, D], F32).ap()

    with tile.TileContext(nc) as tc:
        with tc.tile_pool(name="cn", bufs=1) as cp, \
             tc.tile_pool(name="big", bufs=1) as bp:
            # ---- constants / bias vectors (persistent, tiny) ----
            ident = cp.tile([P, P], F32)
            make_identity(nc, ident[:])
            ones_f = cp.tile([P, 1], F32)
            nc.vector.memset(ones_f[:], 1.0)
            ones64 = cp.tile([1, HS], R)
            nc.vector.tensor_copy(ones64[:],
                                  ones_f[0:1, :].to_broadcast([1, HS]))
            onesP = cp.tile([1, P], R)
            nc.vector.tensor_copy(onesP[:],
                                  ones_f[0:1, :].to_broadcast([1, P]))
            epsv = cp.tile([P, 1], F32)
            nc.vector.memset(epsv[:], LN_EPS)

            # x load first so the big DMA isn't stuck behind the
            # scattered little bias loads
            xsb = bp.tile([P, NT, D], F32, tag="at", name="xsb")
            xr3 = x_d.rearrange("(t p) d -> p t d", p=P)
            for tb in range(NT):
                nc.sync.dma_start(xsb[:, tb, :], xr3[:, tb, :])

            bqv = cp.tile([P, EBS], F32)
            nc.sync.dma_start(bqv[:], bq_d.rearrange("h s -> (h s)")
                              .rearrange("(b p) -> p b", p=P))
            bkv = cp.tile([P, EBS], F32)
            nc.sync.dma_start(bkv[:], bk_d.rearrange("h s -> (h s)")
                              .rearrange("(b p) -> p b", p=P))
            bvv = cp.tile([P, EBS], F32)
            nc.sync.dma_start(bvv[:], bv_d.rearrange("h s -> (h s)")
                              .rearrange("(b p) -> p b", p=P))
            g1v = cp.tile([P, DB], F32)
            nc.sync.dma_start(g1v[:], g1_d.rearrange("(b p) -> p b", p=P))
            c1v = cp.tile([P, DB], F32)
            nc.sync.dma_start(c1v[:], c1_d.rearrange("(b p) -> p b", p=P))
            g2v = cp.tile([P, DB], F32)
            nc.sync.dma_start(g2v[:], g2_d.rearrange("(b p) -> p b", p=P))
            c2v = cp.tile([P, DB], F32)
            nc.sync.dma_start(c2v[:], c2_d.rearrange("(b p) -> p b", p=P))
            b1v = cp.tile([P, FF // P], F32)
            nc.sync.dma_start(b1v[:], b1_d.rearrange("(b p) -> p b", p=P))

            # LN stats scratch (reused for LN2 by tag)
            st_sum = cp.tile([P, NT], F32)
            st_sq = cp.tile([P, NT], F32)
            st_mu = cp.tile([P, NT], F32)
            st_var = cp.tile([P, NT], F32)
            st_rs = cp.tile([P, NT], F32)
            st_nm = cp.tile([P, NT], F32)
            st_vh = cp.tile([P, NT], F32)
            st_t = cp.tile([P, NT], F32)
            st_ih = cp.tile([P, NT], mybir.dt.int32)

            def layernorm_transpose(src, dst, gv, cv, pfx, pspool, trbufs,
                                    after_tb=None):
                """src: [P, NT, D] token layout (f32) -> dst [P, DB, N] f32r
                feature layout, with affine (gv, cv per-partition) folded into
                the transpose evacuation. Fully per-tb so each token tile
                flows stats -> normalize -> transpose independently."""
                for tb in range(NT):
                    t1 = (tb, tb + 1)
                    nc.vector.reduce_sum(st_sum[:, t1[0]:t1[1]], src[:, tb, :],
                                         axis=mybir.AxisListType.X)
                    sq = bp.tile([P, D], F32, tag="qb", bufs=2,
                                 name=f"sq{tb}")
                    nc.scalar.activation(sq[:], src[:, tb, :], AF.Square,
                                         accum_out=st_sq[:, t1[0]:t1[1]])
                    sm = st_sum[:, t1[0]:t1[1]]
                    var = st_var[:, t1[0]:t1[1]]
                    rs = st_rs[:, t1[0]:t1[1]]
                    nm = st_nm[:, t1[0]:t1[1]]
                    ih = st_ih[:, t1[0]:t1[1]]
                    vh = st_vh[:, t1[0]:t1[1]]
                    tt = st_t[:, t1[0]:t1[1]]
                    i32 = mybir.dt.int32
                    # var = sq/D - (sum/D)^2 + eps   (depth-3 chain)
                    nc.vector.tensor_tensor(tt, sm, sm, OP.mult)
                    nc.vector.tensor_scalar(var, tt, -1.0 / (D * D), LN_EPS,
                                            OP.mult, OP.add)
                    nc.vector.tensor_scalar(var, st_sq[:, t1[0]:t1[1]],
                                            1.0 / D, var, OP.mult, OP.add)
                    # rstd = rsqrt(var), DVE-only (bit hack + 2 Newton steps)
                    # so the ACT engine never needs the sqrt table set
                    nc.vector.tensor_scalar(ih, var.bitcast(i32), 1, None,
                                            OP.arith_shift_right)
                    nc.vector.tensor_scalar(rs.bitcast(i32), ih, -1,
                                            0x5F3759DF, OP.mult, OP.add)
                    nc.vector.tensor_scalar_mul(vh, var, 0.5)
                    for _ in range(2):
                        nc.vector.tensor_tensor(tt, rs, rs, OP.mult)
                        nc.vector.tensor_tensor(tt, tt, vh, OP.mult)
                        nc.vector.tensor_scalar(tt, tt, -1.0, 1.5,
                                                OP.mult, OP.add)
                        nc.vector.tensor_tensor(rs, rs, tt, OP.mult)
                    # nm = -(sum/D)*rstd
                    nc.vector.tensor_tensor(nm, sm, rs, OP.mult)
                    nc.vector.tensor_scalar_mul(nm, nm, -1.0 / D)
                    tnorm = bp.tile([P, D], F32, tag="kb", bufs=2,
                                    name=f"tn{tb}")
                    nc.vector.tensor_scalar(tnorm[:], src[:, tb, :],
                                            rs, nm, OP.mult, OP.add)
                    for db in range(DB):
                        pt = pspool.tile([P, P], F32, tag="tr", bufs=trbufs,
                                         name=f"ptr{pfx}_{tb}_{db}")
                        nc.tensor.transpose(pt[:], tnorm[:, ts(db, P)],
                                            ident[:])
                        nc.vector.tensor_scalar(dst[:, db, ts(tb, P)], pt[:],
                                                gv[:, db:db + 1],
                                                cv[:, db:db + 1],
                                                OP.mult, OP.add)
                    if after_tb is not None:
                        after_tb(tb)

            # ================= Phase A: LN1 + transpose ====================
            HT = bp.tile([P, DB, N], R, tag="ht", name="HT")
            psAB_cm = tc.tile_pool(name="psAB", bufs=1, space="PSUM")
            psAB = psAB_cm.__enter__()
            layernorm_transpose(xsb, HT, g1v, c1v, "a", psAB, 4)

            # ================= Phase B0: V projection ======================
            Vaug = bp.tile([P, NT, H, HS + 1], R, tag="v", name="Vaug")
            nc.vector.tensor_copy(
                Vaug[:, :, :, HS:HS + 1],
                ones_f[:, None, :].to_broadcast([P, NT, H, 1]))
            if True:
                for eh in range(2):
                    wvt = bp.tile([P, DB, 512], R, tag="se", bufs=2,
                                  name=f"wv{eh}")
                    for do in range(DB):
                        nc.sync.dma_start(
                            wvt[:, do].rearrange("p (h s) -> p h s", s=HS),
                            wv_d[eh * 8:(eh + 1) * 8, ds(do * P, P), :]
                            .rearrange("h dp s -> dp h s")
                            .bitcast(R))
                    for tb in range(NT):
                        pv = psAB.tile([P, 512], F32, tag="qkv", bufs=4,
                                       name=f"pv{eh}_{tb}")
                        for db in range(DB):
                            nc.tensor.matmul(pv[:], HT[:, db, ts(tb, P)],
                                             wvt[:, db, :],
                                             start=(db == 0), stop=(db == DB - 1))
                        nc.scalar.activation(
                            Vaug[:, tb, eh * 8:(eh + 1) * 8, 0:HS],
                            pv[:].rearrange("p (h s) -> p h s", s=HS), AF.Copy)

            # ============ Phase BC: Q/K per e-block fused with attention ===
            psAB_cm.__exit__(None, None, None)
            psBC_cm = tc.tile_pool(name="psBC", bufs=1, space="PSUM")
            psBC = psBC_cm.__enter__()
            attnT = bp.tile([P, EBS, N], R, tag="at", name="attnT")
            # prefetch Wproj during attention
            wpa = bp.tile([P, 4, D], R, tag="sd", bufs=2, name="wpa")
            wpb = bp.tile([P, 4, D], R, tag="sd", bufs=2, name="wpb")
            nc.sync.dma_start(
                wpa[:], wp_d[0:512].rearrange("(eo ep) d -> ep eo d", ep=P)
                .bitcast(R))
            nc.sync.dma_start(
                wpb[:], wp_d[512:1024].rearrange("(eo ep) d -> ep eo d", ep=P)
                .bitcast(R))

            if True:
                for eb in range(EBS):
                    wqt = bp.tile([P, DB, P], R, tag="wqk", bufs=2,
                                  name=f"wq{eb}")
                    for do in range(DB):
                        nc.sync.dma_start(
                            wqt[:, do].rearrange("p (h s) -> p h s", s=HS),
                            wq_d[2 * eb:2 * eb + 2, ds(do * P, P), :]
                            .rearrange("h dp s -> dp h s")
                            .bitcast(R))
                    wkt = bp.tile([P, DB, P], R, tag="wqk", bufs=2,
                                  name=f"wk{eb}")
                    for do in range(DB):
                        nc.sync.dma_start(
                            wkt[:, do].rearrange("p (h s) -> p h s", s=HS),
                            wk_d[2 * eb:2 * eb + 2, ds(do * P, P), :]
                            .rearrange("h dp s -> dp h s")
                            .bitcast(R))
                    Qb = bp.tile([P, N], R, tag="qb", bufs=2, name=f"Qb{eb}")
                    Kb = bp.tile([P, N], R, tag="kb", bufs=2, name=f"Kb{eb}")
                    for nh in range(NH):
                        pq = psBC.tile([P, 512], F32, tag="qk", bufs=2,
                                       name=f"pq{eb}_{nh}")
                        for db in range(DB):
                            nc.tensor.matmul(pq[:], wqt[:, db, :],
                                             HT[:, db, ds(nh * 512, 512)],
                                             start=(db == 0), stop=(db == DB - 1))
                        nc.vector.tensor_scalar_add(Qb[:, ds(nh * 512, 512)],
                                                    pq[:], bqv[:, eb:eb + 1])
                        pk = psBC.tile([P, 512], F32, tag="qk", bufs=2,
                                       name=f"pk{eb}_{nh}")
                        for db in range(DB):
                            nc.tensor.matmul(pk[:], wkt[:, db, :],
                                             HT[:, db, ds(nh * 512, 512)],
                                             start=(db == 0), stop=(db == DB - 1))
                        nc.vector.tensor_scalar_add(Kb[:, ds(nh * 512, 512)],
                                                    pk[:], bkv[:, eb:eb + 1])

                    # attention for heads 2eb (partitions 0:64) and
                    # 2eb+1 (partitions 64:128), per n-half of 512
                    for nh in range(NH):
                        pts = [bp.tile([P, NT, 512], R, tag="se", bufs=2,
                                       name=f"PT{eb}_{nh}_{i}")
                               for i in range(2)]
                        # scoresT[m, n] = sum_s K[m,s] Q[n,s]; exp via ACT
                        for mt in range(NT):
                            for i in range(2):
                                base = i * HS
                                pss = psBC.tile([P, 512], F32, tag="sc",
                                                bufs=4, name=f"ps{eb}{nh}{mt}{i}")
                                nc.tensor.matmul(
                                    pss[:],
                                    Kb[base:base + HS, ts(mt, P)],
                                    Qb[base:base + HS, ds(nh * 512, 512)],
                                    start=True, stop=True)
                                nc.scalar.activation(pts[i][:, mt, :], pss[:],
                                                     AF.Exp, scale=0.125)
                        pas = [psBC.tile([HS + 1, 512], F32, tag="at65",
                                         bufs=2, name=f"pa{eb}_{nh}_{i}")
                               for i in range(2)]
                        for mb in range(NT):
                            for i in range(2):
                                nc.tensor.matmul(pas[i][:],
                                                 Vaug[:, mb, 2 * eb + i, :],
                                                 pts[i][:, mb, :],
                                                 start=(mb == 0),
                                                 stop=(mb == NT - 1))
                        for i in range(2):
                            base = i * HS
                            rec = bp.tile([1, 512], F32, tag="rb", bufs=2,
                                          name=f"rc{eb}_{nh}_{i}")
                            nc.vector.reciprocal(rec[:],
                                                 pas[i][HS:HS + 1, :])
                            rbs = bp.tile([HS, 512], F32, tag="rb", bufs=2,
                                          name=f"rb{eb}_{nh}_{i}")
                            nc.gpsimd.partition_broadcast(rbs[:], rec[:])
                            dstA = attnT[base:base + HS, eb,
                                         ds(nh * 512, 512)]
                            nc.vector.tensor_tensor(dstA, pas[i][0:HS, :],
                                                    rbs[:], OP.mult)
                            nc.vector.tensor_scalar_add(
                                dstA, dstA, bvv[base:base + HS, eb:eb + 1])

            # w1(ft0) prefetch into "se" (frees at end of attention)
            w1pre = bp.tile([P, DB, 512], R, tag="se", bufs=2,
                            name="w1pre")
            nc.sync.dma_start(
                w1pre[:],
                w1_d[:, ds(0, 512)]
                .rearrange("(do dp) f -> dp do f", dp=P)
                .bitcast(R))

            # ================= Phase D: proj + residual ====================
            psBC_cm.__exit__(None, None, None)
            psDE_cm = tc.tile_pool(name="psDE", bufs=1, space="PSUM")
            psDE = psDE_cm.__enter__()
            x2 = bp.tile([P, NT, D], F32, tag="v", name="x2")
            xr = bp.tile([P, NT, D], F32, tag="ht", name="xrl")
            if True:
                # broadcast bproj -> [P, D]
                bprow = bp.tile([1, D], R, tag="kb", bufs=2, name="bprow")
                nc.sync.dma_start(bprow[:], bp_d[None, :].bitcast(R))
                bpB = bp.tile([P, D], F32, tag="qb", bufs=2, name="bpB")
                for dh in range(2):
                    pbb = psDE.tile([P, 512], F32, tag="trb", bufs=1,
                                    name=f"pbb{dh}")
                    nc.tensor.matmul(pbb[:], onesP[:],
                                     bprow[:, ds(dh * 512, 512)],
                                     start=True, stop=True)
                    nc.vector.tensor_copy(bpB[:, ds(dh * 512, 512)], pbb[:])
                for tb in range(NT):
                    nc.sync.dma_start(xr[:, tb, :], xr3[:, tb, :])
                    nc.vector.tensor_tensor(xr[:, tb, :], xr[:, tb, :],
                                            bpB[:], OP.add)
                for tb in range(NT):
                    for dt in range(2):
                        pp = psDE.tile([P, 512], F32, tag="pj", bufs=4,
                                       name=f"pp{tb}_{dt}")
                        for g in range(2):
                            wp = wpa if g == 0 else wpb
                            for eo in range(4):
                                nc.tensor.matmul(
                                    pp[:], attnT[:, g * 4 + eo, ts(tb, P)],
                                    wp[:, eo, ds(dt * 512, 512)],
                                    start=(g == 0 and eo == 0),
                                    stop=(g == 1 and eo == 3))
                        nc.vector.tensor_tensor(x2[:, tb, ds(dt * 512, 512)],
                                                pp[:],
                                                xr[:, tb, ds(dt * 512, 512)],
                                                OP.add)

            # ================= Phase E: LN2, transpose, stash x2+b2 ========
            H2T = bp.tile([P, DB, N], R, tag="ht", name="H2T")
            if True:
                layernorm_transpose(x2, H2T, g2v, c2v, "e", psDE, 3)
                # broadcast b2 -> [P, D]; x2 += b2B; stash to DRAM
                b2row = bp.tile([1, D], R, tag="kb", bufs=2, name="b2row")
                nc.sync.dma_start(b2row[:], b2_d[None, :].bitcast(R))
                b2B = bp.tile([P, D], F32, tag="qb", bufs=2, name="b2B")
                for dh in range(2):
                    pb2 = psDE.tile([P, 512], F32, tag="trb", bufs=1,
                                    name=f"pb2{dh}")
                    nc.tensor.matmul(pb2[:], onesP[:],
                                     b2row[:, ds(dh * 512, 512)],
                                     start=True, stop=True)
                    nc.vector.tensor_copy(b2B[:, ds(dh * 512, 512)], pb2[:])
                for tb in range(NT):
                    nc.vector.tensor_tensor(x2[:, tb, :], x2[:, tb, :],
                                            b2B[:], OP.add)
                    nc.sync.dma_start(x2pb_d[:, tb, :], x2[:, tb, :])

            # ================= Phase F: FFN ================================
            psDE_cm.__exit__(None, None, None)
            psF_cm = tc.tile_pool(name="psF", bufs=1, space="PSUM")
            psF = psF_cm.__enter__()
            if True:
                for nt in range(NH):
                    y1 = [bp.tile([P, 16, 512], R, tag=tg, name=f"y1{nt}{tg}")
                          for tg in ("v", "at")]
                    xcf = bp.tile([P, 4, D], F32, tag="se", bufs=2,
                                  name=f"xcf{nt}")
                    nc.sync.dma_start(xcf[:], x2pb_d[:, nt * 4:(nt + 1) * 4, :])
                    for ft in range(FF // 512):
                        if ft == 0:
                            w1c = w1pre
                        else:
                            w1c = bp.tile([P, DB, 512], R, tag="sd", bufs=2,
                                          name=f"w1_{nt}_{ft}")
                            nc.sync.dma_start(
                                w1c[:],
                                w1_d[:, ds(ft * 512, 512)]
                                .rearrange("(do dp) f -> dp do f", dp=P)
                                .bitcast(R))
                        for fc in range(4):
                            bf = ft * 4 + fc
                            p1 = psF.tile([P, 512], F32, tag="fp", bufs=8,
                                          name=f"p1_{nt}_{bf}")
                            for db in range(DB):
                                nc.tensor.matmul(
                                    p1[:], w1c[:, db, ds(fc * P, P)],
                                    H2T[:, db, ds(nt * 512, 512)],
                                    start=(db == 0), stop=(db == DB - 1))
                            ydst = y1[bf // 16][:, bf % 16, :]
                            if use_lrelu:
                                nc.scalar.activation(ydst, p1[:], AF.Prelu,
                                                     bias=b1v[:, bf:bf + 1],
                                                     alpha=0.01)
                            else:
                                z = bp.tile([P, 512], F32, tag="qb", bufs=2,
                                            name=f"z{nt}_{bf}")
                                nc.scalar.activation(z[:], p1[:], AF.Identity,
                                                     bias=b1v[:, bf:bf + 1])
                                zs = bp.tile([P, 512], F32, tag="rb", bufs=2,
                                             name=f"zs{nt}_{bf}")
                                nc.vector.tensor_scalar_mul(zs[:], z[:], 0.01)
                                nc.vector.tensor_tensor(ydst, z[:], zs[:],
                                                        OP.max)
                    pf2 = [psF.tile([P, 512], F32, tag="fp", bufs=8,
                                    name=f"p2_{nt}_{j}") for j in range(8)]
                    for ft in range(FF // 512):
                        w2c = bp.tile([P, 4, D], R, tag="sd", bufs=2,
                                      name=f"w2_{nt}_{ft}")
                        nc.sync.dma_start(
                            w2c[:], w2_d[ds(ft * 512, 512), :]
                            .rearrange("(fo fp) d -> fp fo d", fp=P)
                            .bitcast(R))
                        for fc in range(4):
                            bf = ft * 4 + fc
                            ysrc = y1[bf // 16][:, bf % 16, :]
                            for tb in range(4):
                                for dt in range(2):
                                    nc.tensor.matmul(
                                        pf2[tb * 2 + dt][:],
                                        ysrc[:, ts(tb, P)],
                                        w2c[:, fc, ds(dt * 512, 512)],
                                        start=(ft == 0 and fc == 0),
                                        stop=(ft == 7 and fc == 3))
                    for tb in range(4):
                        for dt in range(2):
                            rows = ds(nt * 512 + tb * P, P)
                            og = bp.tile([P, 512], F32, tag="rb", bufs=2,
                                         name=f"og{nt}_{tb}_{dt}")
                            nc.vector.tensor_tensor(og[:], pf2[tb * 2 + dt][:],
                                                    xcf[:, tb, ds(dt * 512, 512)],
                                                    OP.add)
                            nc.sync.dma_start(out_d[rows, ds(dt * 512, 512)],
                                              og[:])
            psF_cm.__exit__(None, None, None)
    nc.compile()
    return nc


def get_nc():
    global _CACHED_NC
    if _CACHED_NC is None:
        _CACHED_NC = build_nc()
    return _CACHED_NC


def kernel(**inputs):
    nc = get_nc()
    x = np.ascontiguousarray(np.asarray(inputs["x"], dtype=np.float32))
    B = x.shape[0]
    weights = {k: np.ascontiguousarray(np.asarray(v, dtype=np.float32))
               for k, v in inputs.items() if k != "x"}
    in_maps = [dict(weights, x=x[b]) for b in range(B)]
    res = run_bass_kernel_spmd(nc, in_maps, list(range(B)))
    return np.stack([res.results[b]["out"] for b in range(B)], axis=0)


# revision 25
# speedup vs baseline: 11123.3928x; 1.0089x over previous
"""Trainium2 Bass kernel for a pre-LN transformer block (MHA + FFN).

Data-parallel over batch: 8 NeuronCores, one batch element each.
All matmuls run as float32r (full PE rate at free-dim>=256), storage fp32.
"""
import sys

for _p in ("/opt/trn_rl_repo", "/root/.axon_site/_ro/trn_rl_repo"):
    if _p not in sys.path:
        sys.path.insert(0, _p)

import numpy as np
import concourse.bass as bass
import concourse.tile as tile
from concourse import bacc, mybir
from concourse.bass import ds, ts
from concourse.bass_utils import run_bass_kernel_spmd
from concourse.masks import make_identity

P = 128
N = 1024          # tokens per core (seq len)
D = 1024          # d_emb
H = 16            # heads
HS = 64           # head size
FF = 4096         # ffn hidden
NT = N // P       # 8 token tiles
DB = D // P       # 8 d blocks
EBS = D // P      # 8 e blocks (qkv out features)
NH = 2            # n halves of 512
LN_EPS = 1e-5

F32 = mybir.dt.float32
R = mybir.dt.float32r
AF = mybir.ActivationFunctionType
OP = mybir.AluOpType

_CACHED_NC = None


def build_nc(use_lrelu=True):
    nc = bacc.Bacc("TRN2", target_bir_lowering=False, debug=False, num_devices=8)

    x_d = nc.dram_tensor("x", [N, D], F32, kind="ExternalInput").ap()
    wq_d = nc.dram_tensor("Wq", [H, D, HS], F32, kind="ExternalInput").ap()
    bq_d = nc.dram_tensor("bq", [H, HS], F32, kind="ExternalInput").ap()
    wk_d = nc.dram_tensor("Wk", [H, D, HS], F32, kind="ExternalInput").ap()
    bk_d = nc.dram_tensor("bk", [H, HS], F32, kind="ExternalInput").ap()
    wv_d = nc.dram_tensor("Wv", [H, D, HS], F32, kind="ExternalInput").ap()
    bv_d = nc.dram_tensor("bv", [H, HS], F32, kind="ExternalInput").ap()
    wp_d = nc.dram_tensor("Wproj", [H * HS, D], F32, kind="ExternalInput").ap()
    bp_d = nc.dram_tensor("bproj", [D], F32, kind="ExternalInput").ap()
    w1_d = nc.dram_tensor("W1", [D, FF], F32, kind="ExternalInput").ap()
    b1_d = nc.dram_tensor("b1", [FF], F32, kind="ExternalInput").ap()
    w2_d = nc.dram_tensor("W2", [FF, D], F32, kind="ExternalInput").ap()
    b2_d = nc.dram_tensor("b2", [D], F32, kind="ExternalInput").ap()
    g1_d = nc.dram_tensor("ln1_g", [D], F32, kind="ExternalInput").ap()
    c1_d = nc.dram_tensor("ln1_b", [D], F32, kind="ExternalInput").ap()
    g2_d = nc.dram_tensor("ln2_g", [D], F32, kind="ExternalInput").ap()
    c2_d = nc.dram_tensor("ln2_b", [D], F32, kind="ExternalInput").ap()
    out_d = nc.dram_tensor("out", [N, D], F32, kind="ExternalOutput").ap()
    x2pb_d = nc.dram_tensor("x2pb_scratch", [P, NT, D], F32).ap()

    with tile.TileContext(nc) as tc:
        with tc.tile_pool(name="cn", bufs=1) as cp, \
             tc.tile_pool(name="big", bufs=1) as bp:
            # ---- constants / bias vectors (persistent, tiny) ----
            ident = cp.tile([P, P], F32)
            make_identity(nc, ident[:])
            ones_f = cp.tile([P, 1], F32)
            nc.vector.memset(ones_f[:], 1.0)
            ones64 = cp.tile([1, HS], R)
            nc.vector.tensor_copy(ones64[:],
                                  ones_f[0:1, :].to_broadcast([1, HS]))
            onesP = cp.tile([1, P], R)
            nc.vector.tensor_copy(onesP[:],
                                  ones_f[0:1, :].to_broadcast([1, P]))
            epsv = cp.tile([P, 1], F32)
            nc.vector.memset(epsv[:], LN_EPS)

            # x load first so the big DMA isn't stuck behind the
            # scattered little bias loads
            xsb = bp.tile([P, NT, D], F32, tag="at", name="xsb")
            xr3 = x_d.rearrange("(t p) d -> p t d", p=P)
            for tb in range(NT):
                nc.sync.dma_start(xsb[:, tb, :], xr3[:, tb, :])

            bqv = cp.tile([P, EBS], F32)
            nc.sync.dma_start(bqv[:], bq_d.rearrange("h s -> (h s)")
                              .rearrange("(b p) -> p b", p=P))
            bkv = cp.tile([P, EBS], F32)
            nc.sync.dma_start(bkv[:], bk_d.rearrange("h s -> (h s)")
                              .rearrange("(b p) -> p b", p=P))
            bvv = cp.tile([P, EBS], F32)
            nc.sync.dma_start(bvv[:], bv_d.rearrange("h s -> (h s)")
                              .rearrange("(b p) -> p b", p=P))
            g1v = cp.tile([P, DB], F32)
            nc.sync.dma_start(g1v[:], g1_d.rearrange("(b p) -> p b", p=P))
            c1v = cp.tile([P, DB], F32)
            nc.sync.dma_start(c1v[:], c1_d.rearrange("(b p) -> p b", p=P))
            g2v = cp.tile([P, DB], F32)
            nc.sync.dma_start(g2v[:], g2_d.rearrange("(b p) -> p b", p=P))
            c2v = cp.tile([P, DB], F32)
            nc.sync.dma_start(c2v[:], c2_d.rearrange("(b p) -> p b", p=P))
            b1v = cp.tile([P, FF // P], F32)
            nc.sync.dma_start(b1v[:], b1_d.rearrange("(b p) -> p b", p=P))

            # LN stats scratch (reused for LN2 by tag)
            st_sum = cp.tile([P, NT], F32)
            st_sq = cp.tile([P, NT], F32)
            st_mu = cp.tile([P, NT], F32)
            st_var = cp.tile([P, NT], F32)
            st_rs = cp.tile([P, NT], F32)
            st_nm = cp.tile([P, NT], F32)
            st_vh = cp.tile([P, NT], F32)
            st_t = cp.tile([P, NT], F32)
            st_ih = cp.tile([P, NT], mybir.dt.int32)

            def layernorm_transpose(src, dst, gv, cv, pfx, pspool, trbufs,
                                    after_tb=None, tbs=None):
                """src: [P, NT, D] token layout (f32) -> dst [P, DB, N] f32r
                feature layout, with affine (gv, cv per-partition) folded into
                the transpose evacuation. Fully per-tb so each token tile
                flows stats -> normalize -> transpose independently."""
                for tb in (range(NT) if tbs is None else tbs):
                    t1 = (tb, tb + 1)
                    nc.vector.reduce_sum(st_sum[:, t1[0]:t1[1]], src[:, tb, :],
                                         axis=mybir.AxisListType.X)
                    sq = bp.tile([P, D], F32, tag="qb", bufs=2,
                                 name=f"sq{tb}")
                    nc.scalar.activation(sq[:], src[:, tb, :], AF.Square,
                                         accum_out=st_sq[:, t1[0]:t1[1]])
                    sm = st_sum[:, t1[0]:t1[1]]
                    var = st_var[:, t1[0]:t1[1]]
                    rs = st_rs[:, t1[0]:t1[1]]
                    nm = st_nm[:, t1[0]:t1[1]]
                    ih = st_ih[:, t1[0]:t1[1]]
                    vh = st_vh[:, t1[0]:t1[1]]
                    tt = st_t[:, t1[0]:t1[1]]
                    i32 = mybir.dt.int32
                    # var = sq/D - (sum/D)^2 + eps   (depth-3 chain)
                    nc.vector.tensor_tensor(tt, sm, sm, OP.mult)
                    nc.vector.tensor_scalar(var, tt, -1.0 / (D * D), LN_EPS,
                                            OP.mult, OP.add)
                    nc.vector.tensor_scalar(var, st_sq[:, t1[0]:t1[1]],
                                            1.0 / D, var, OP.mult, OP.add)
                    # rstd = rsqrt(var), DVE-only (bit hack + 2 Newton steps)
                    # so the ACT engine never needs the sqrt table set
                    nc.vector.tensor_scalar(ih, var.bitcast(i32), 1, None,
                                            OP.arith_shift_right)
                    nc.vector.tensor_scalar(rs.bitcast(i32), ih, -1,
                                            0x5F3759DF, OP.mult, OP.add)
                    nc.vector.tensor_scalar_mul(vh, var, 0.5)
                    for _ in range(2):
                        nc.vector.tensor_tensor(tt, rs, rs, OP.mult)
                        nc.vector.tensor_tensor(tt, tt, vh, OP.mult)
                        nc.vector.tensor_scalar(tt, tt, -1.0, 1.5,
                                                OP.mult, OP.add)
                        nc.vector.tensor_tensor(rs, rs, tt, OP.mult)
                    # nm = -(sum/D)*rstd
                    nc.vector.tensor_tensor(nm, sm, rs, OP.mult)
                    nc.vector.tensor_scalar_mul(nm, nm, -1.0 / D)
                    tnorm = bp.tile([P, D], F32, tag="kb", bufs=2,
                                    name=f"tn{tb}")
                    nc.vector.tensor_scalar(tnorm[:], src[:, tb, :],
                                            rs, nm, OP.mult, OP.add)
                    for db in range(DB):
                        pt = pspool.tile([P, P], F32, tag="tr", bufs=trbufs,
                                         name=f"ptr{pfx}_{tb}_{db}")
                        nc.tensor.transpose(pt[:], tnorm[:, ts(db, P)],
                                            ident[:])
                        nc.vector.tensor_scalar(dst[:, db, ts(tb, P)], pt[:],
                                                gv[:, db:db + 1],
                                                cv[:, db:db + 1],
                                                OP.mult, OP.add)
                    if after_tb is not None:
                        after_tb(tb)

            # ================= Phase A: LN1 + transpose ====================
            HT = bp.tile([P, DB, N], R, tag="ht", name="HT")
            psAB_cm = tc.tile_pool(name="psAB", bufs=1, space="PSUM")
            psAB = psAB_cm.__enter__()
            layernorm_transpose(xsb, HT, g1v, c1v, "a", psAB, 4)

            # ================= Phase B0: V projection ======================
            Vaug = bp.tile([P, NT, H, HS + 1], R, tag="v", name="Vaug")
            nc.vector.tensor_copy(
                Vaug[:, :, :, HS:HS + 1],
                ones_f[:, None, :].to_broadcast([P, NT, H, 1]))
            if True:
                for eh in range(2):
                    wvt = bp.tile([P, DB, 512], R, tag="se", bufs=2,
                                  name=f"wv{eh}")
                    for do in range(DB):
                        nc.sync.dma_start(
                            wvt[:, do].rearrange("p (h s) -> p h s", s=HS),
                            wv_d[eh * 8:(eh + 1) * 8, ds(do * P, P), :]
                            .rearrange("h dp s -> dp h s")
                            .bitcast(R))
                    for tb in range(NT):
                        pv = psAB.tile([P, 512], F32, tag="qkv", bufs=4,
                                       name=f"pv{eh}_{tb}")
                        for db in range(DB):
                            nc.tensor.matmul(pv[:], HT[:, db, ts(tb, P)],
                                             wvt[:, db, :],
                                             start=(db == 0), stop=(db == DB - 1))
                        nc.scalar.activation(
                            Vaug[:, tb, eh * 8:(eh + 1) * 8, 0:HS],
                            pv[:].rearrange("p (h s) -> p h s", s=HS), AF.Copy)

            # ============ Phase BC: Q/K per e-block fused with attention ===
            psAB_cm.__exit__(None, None, None)
            psBC_cm = tc.tile_pool(name="psBC", bufs=1, space="PSUM")
            psBC = psBC_cm.__enter__()
            attnT = bp.tile([P, EBS, N], R, tag="at", name="attnT")
            # prefetch Wproj during attention
            wpa = bp.tile([P, 4, D], R, tag="sd", bufs=2, name="wpa")
            wpb = bp.tile([P, 4, D], R, tag="sd", bufs=2, name="wpb")
            nc.sync.dma_start(
                wpa[:], wp_d[0:512].rearrange("(eo ep) d -> ep eo d", ep=P)
                .bitcast(R))
            nc.sync.dma_start(
                wpb[:], wp_d[512:1024].rearrange("(eo ep) d -> ep eo d", ep=P)
                .bitcast(R))

            if True:
                for eb in range(EBS):
                    wqt = bp.tile([P, DB, P], R, tag="wqk", bufs=2,
                                  name=f"wq{eb}")
                    for do in range(DB):
                        nc.sync.dma_start(
                            wqt[:, do].rearrange("p (h s) -> p h s", s=HS),
                            wq_d[2 * eb:2 * eb + 2, ds(do * P, P), :]
                            .rearrange("h dp s -> dp h s")
                            .bitcast(R))
                    wkt = bp.tile([P, DB, P], R, tag="wqk", bufs=2,
                                  name=f"wk{eb}")
                    for do in range(DB):
                        nc.sync.dma_start(
                            wkt[:, do].rearrange("p (h s) -> p h s", s=HS),
                            wk_d[2 * eb:2 * eb + 2, ds(do * P, P), :]
                            .rearrange("h dp s -> dp h s")
                            .bitcast(R))
                    Qb = bp.tile([P, N], R, tag="qb", bufs=2, name=f"Qb{eb}")
                    Kb = bp.tile([P, N], R, tag="kb", bufs=2, name=f"Kb{eb}")
                    for nh in range(NH):
                        pq = psBC.tile([P, 512], F32, tag="qk", bufs=2,
                                       name=f"pq{eb}_{nh}")
                        for db in range(DB):
                            nc.tensor.matmul(pq[:], wqt[:, db, :],
                                             HT[:, db, ds(nh * 512, 512)],
                                             start=(db == 0), stop=(db == DB - 1))
                        nc.vector.tensor_scalar_add(Qb[:, ds(nh * 512, 512)],
                                                    pq[:], bqv[:, eb:eb + 1])
                        pk = psBC.tile([P, 512], F32, tag="qk", bufs=2,
                                       name=f"pk{eb}_{nh}")
                        for db in range(DB):
                            nc.tensor.matmul(pk[:], wkt[:, db, :],
                                             HT[:, db, ds(nh * 512, 512)],
                                             start=(db == 0), stop=(db == DB - 1))
                        nc.vector.tensor_scalar_add(Kb[:, ds(nh * 512, 512)],
                                                    pk[:], bkv[:, eb:eb + 1])

                    # attention for heads 2eb (partitions 0:64) and
                    # 2eb+1 (partitions 64:128), per n-half of 512
                    for nh in range(NH):
                        pts = [bp.tile([P, NT, 512], R, tag="se", bufs=2,
                                       name=f"PT{eb}_{nh}_{i}")
                               for i in range(2)]
                        # scoresT[m, n] = sum_s K[m,s] Q[n,s]; exp via ACT
                        for mt in range(NT):
                            for i in range(2):
                                base = i * HS
                                pss = psBC.tile([P, 512], F32, tag="sc",
                                                bufs=4, name=f"ps{eb}{nh}{mt}{i}")
                                nc.tensor.matmul(
                                    pss[:],
                                    Kb[base:base + HS, ts(mt, P)],
                                    Qb[base:base + HS, ds(nh * 512, 512)],
                                    start=True, stop=True)
                                nc.scalar.activation(pts[i][:, mt, :], pss[:],
                                                     AF.Exp, scale=0.125)
                        pas = [psBC.tile([HS + 1, 512], F32, tag="at65",
                                         bufs=2, name=f"pa{eb}_{nh}_{i}")
                               for i in range(2)]
                        for mb in range(NT):
                            for i in range(2):
                                nc.tensor.matmul(pas[i][:],
                                                 Vaug[:, mb, 2 * eb + i, :],
                                                 pts[i][:, mb, :],
                                                 start=(mb == 0),
                                                 stop=(mb == NT - 1))
                        for i in range(2):
                            base = i * HS
                            rec = bp.tile([1, 512], F32, tag="rb", bufs=2,
                                          name=f"rc{eb}_{nh}_{i}")
                            nc.vector.reciprocal(rec[:],
                                                 pas[i][HS:HS + 1, :])
                            rbs = bp.tile([HS, 512], F32, tag="rb", bufs=2,
                                          name=f"rb{eb}_{nh}_{i}")
                            nc.gpsimd.partition_broadcast(rbs[:], rec[:])
                            dstA = attnT[base:base + HS, eb,
                                         ds(nh * 512, 512)]
                            nc.vector.tensor_tensor(dstA, pas[i][0:HS, :],
                                                    rbs[:], OP.mult)
                            nc.vector.tensor_scalar_add(
                                dstA, dstA, bvv[base:base + HS, eb:eb + 1])

            # w1(ft0) prefetch into "se" (frees at end of attention);
            # high priority so the DMA issues as soon as the slot frees
            w1pre = bp.tile([P, DB, 512], R, tag="se", bufs=2,
                            name="w1pre")
            with tc.high_priority():
                nc.sync.dma_start(
                    w1pre[:],
                    w1_d[:, ds(0, 512)]
                    .rearrange("(do dp) f -> dp do f", dp=P)
                    .bitcast(R))

            # ================= Phase D: proj + residual ====================
            psBC_cm.__exit__(None, None, None)
            psDE_cm = tc.tile_pool(name="psDE", bufs=1, space="PSUM")
            psDE = psDE_cm.__enter__()
            x2 = bp.tile([P, NT, D], F32, tag="v", name="x2")
            xr = bp.tile([P, NT, D], F32, tag="ht", name="xrl")
            if True:
                # broadcast bproj -> [P, D]
                bprow = bp.tile([1, D], R, tag="kb", bufs=2, name="bprow")
                nc.sync.dma_start(bprow[:], bp_d[None, :].bitcast(R))
                bpB = bp.tile([P, D], F32, tag="qb", bufs=2, name="bpB")
                for dh in range(2):
                    pbb = psDE.tile([P, 512], F32, tag="trb", bufs=1,
                                    name=f"pbb{dh}")
                    nc.tensor.matmul(pbb[:], onesP[:],
                                     bprow[:, ds(dh * 512, 512)],
                                     start=True, stop=True)
                    nc.vector.tensor_copy(bpB[:, ds(dh * 512, 512)], pbb[:])
                for tb in range(NT):
                    nc.sync.dma_start(xr[:, tb, :], xr3[:, tb, :])
                    nc.vector.tensor_tensor(xr[:, tb, :], xr[:, tb, :],
                                            bpB[:], OP.add)
                for tb in range(NT):
                    for dt in range(2):
                        pp = psDE.tile([P, 512], F32, tag="pj", bufs=4,
                                       name=f"pp{tb}_{dt}")
                        for g in range(2):
                            wp = wpa if g == 0 else wpb
                            for eo in range(4):
                                nc.tensor.matmul(
                                    pp[:], attnT[:, g * 4 + eo, ts(tb, P)],
                                    wp[:, eo, ds(dt * 512, 512)],
                                    start=(g == 0 and eo == 0),
                                    stop=(g == 1 and eo == 3))
                        nc.vector.tensor_tensor(x2[:, tb, ds(dt * 512, 512)],
                                                pp[:],
                                                xr[:, tb, ds(dt * 512, 512)],
                                                OP.add)

            # ================= Phase E: LN2, transpose, stash x2+b2 ========
            H2T = bp.tile([P, DB, N], R, tag="ht", name="H2T")
            if True:
                def ffn1_group(nt, ft, fc, w1c, pool, ptag, pbufs, ydst):
                    p1 = pool.tile([P, 512], F32, tag=ptag, bufs=pbufs,
                                   name=f"p1_{nt}_{ft}_{fc}")
                    for db in range(DB):
                        nc.tensor.matmul(
                            p1[:], w1c[:, db, ds(fc * P, P)],
                            H2T[:, db, ds(nt * 512, 512)],
                            start=(db == 0), stop=(db == DB - 1))
                    bf = ft * 4 + fc
                    if use_lrelu:
                        nc.scalar.activation(ydst, p1[:], AF.Prelu,
                                             bias=b1v[:, bf:bf + 1],
                                             alpha=0.01)
                    else:
                        z = bp.tile([P, 512], F32, tag="qb", bufs=2,
                                    name=f"z{nt}_{bf}")
                        nc.scalar.activation(z[:], p1[:], AF.Identity,
                                             bias=b1v[:, bf:bf + 1])
                        zs = bp.tile([P, 512], F32, tag="rb", bufs=2,
                                     name=f"zs{nt}_{bf}")
                        nc.vector.tensor_scalar_mul(zs[:], z[:], 0.01)
                        nc.vector.tensor_tensor(ydst, z[:], zs[:], OP.max)

                layernorm_transpose(x2, H2T, g2v, c2v, "e", psDE, 3)
                # broadcast b2 -> [P, D]; x2 += b2B; stash to DRAM
                b2row = bp.tile([1, D], R, tag="kb", bufs=2, name="b2row")
                nc.sync.dma_start(b2row[:], b2_d[None, :].bitcast(R))
                b2B = bp.tile([P, D], F32, tag="qb", bufs=2, name="b2B")
                for dh in range(2):
                    pb2 = psDE.tile([P, 512], F32, tag="trb", bufs=1,
                                    name=f"pb2{dh}")
                    nc.tensor.matmul(pb2[:], onesP[:],
                                     b2row[:, ds(dh * 512, 512)],
                                     start=True, stop=True)
                    nc.vector.tensor_copy(b2B[:, ds(dh * 512, 512)], pb2[:])
                for tb in range(NT):
                    nc.vector.tensor_tensor(x2[:, tb, :], x2[:, tb, :],
                                            b2B[:], OP.add)
                    nc.sync.dma_start(x2pb_d[:, tb, :], x2[:, tb, :])

            # ================= Phase F: FFN ================================
            psDE_cm.__exit__(None, None, None)
            psF_cm = tc.tile_pool(name="psF", bufs=1, space="PSUM")
            psF = psF_cm.__enter__()
            if True:
                for nt in range(NH):
                    y1 = [bp.tile([P, 16, 512], R, tag=tg,
                                  name=f"y1{nt}{tg}")
                          for tg in ("at", "v")]
                    xcf = bp.tile([P, 4, D], F32, tag="se", bufs=2,
                                  name=f"xcf{nt}")
                    nc.sync.dma_start(xcf[:], x2pb_d[:, nt * 4:(nt + 1) * 4, :])
                    for ft in range(FF // 512):
                        if ft == 0:
                            w1c = w1pre
                        else:
                            w1c = bp.tile([P, DB, 512], R, tag="sd", bufs=2,
                                          name=f"w1_{nt}_{ft}")
                            nc.sync.dma_start(
                                w1c[:],
                                w1_d[:, ds(ft * 512, 512)]
                                .rearrange("(do dp) f -> dp do f", dp=P)
                                .bitcast(R))
                        for fc in range(4):
                            bf = ft * 4 + fc
                            ffn1_group(nt, ft, fc, w1c, psF, "fp", 8,
                                       y1[bf // 16][:, bf % 16, :])
                    pf2 = [psF.tile([P, 512], F32, tag="fp", bufs=8,
                                    name=f"p2_{nt}_{j}") for j in range(8)]
                    NFT = FF // 512
                    for ft in range(NFT - 1):
                        w2c = bp.tile([P, 4, D], R, tag="sd", bufs=2,
                                      name=f"w2_{nt}_{ft}")
                        nc.sync.dma_start(
                            w2c[:], w2_d[ds(ft * 512, 512), :]
                            .rearrange("(fo fp) d -> fp fo d", fp=P)
                            .bitcast(R))
                        for fc in range(4):
                            bf = ft * 4 + fc
                            ysrc = y1[bf // 16][:, bf % 16, :]
                            for tb in range(4):
                                for dt in range(2):
                                    nc.tensor.matmul(
                                        pf2[tb * 2 + dt][:],
                                        ysrc[:, ts(tb, P)],
                                        w2c[:, fc, ds(dt * 512, 512)],
                                        start=(ft == 0 and fc == 0),
                                        stop=False)
                    # last f-tile: close each psum group in turn so its evac
                    # and output DMA overlap the remaining groups' matmuls
                    ftl = NFT - 1
                    w2c = bp.tile([P, 4, D], R, tag="sd", bufs=2,
                                  name=f"w2_{nt}_{ftl}")
                    nc.sync.dma_start(
                        w2c[:], w2_d[ds(ftl * 512, 512), :]
                        .rearrange("(fo fp) d -> fp fo d", fp=P)
                        .bitcast(R))
                    for tb in range(4):
                        for dt in range(2):
                            for fc in range(4):
                                bf = ftl * 4 + fc
                                ysrc = y1[bf // 16][:, bf % 16, :]
                                nc.tensor.matmul(
                                    pf2[tb * 2 + dt][:],
                                    ysrc[:, ts(tb, P)],
                                    w2c[:, fc, ds(dt * 512, 512)],
                                    start=False, stop=(fc == 3))
                            rows = ds(nt * 512 + tb * P, P)
                            og = bp.tile([P, 512], F32, tag="rb", bufs=2,
                                         name=f"og{nt}_{tb}_{dt}")
                            nc.vector.tensor_tensor(og[:], pf2[tb * 2 + dt][:],
                                                    xcf[:, tb, ds(dt * 512, 512)],
                                                    OP.add)
                            nc.sync.dma_start(out_d[rows, ds(dt * 512, 512)],
                                              og[:])
            psF_cm.__exit__(None, None, None)
    nc.compile()
    return nc


def get_nc():
    global _CACHED_NC
    if _CACHED_NC is None:
        _CACHED_NC = build_nc()
    return _CACHED_NC


def kernel(**inputs):
    nc = get_nc()
    x = np.ascontiguousarray(np.asarray(inputs["x"], dtype=np.float32))
    B = x.shape[0]
    weights = {k: np.ascontiguousarray(np.asarray(v, dtype=np.float32))
               for k, v in inputs.items() if k != "x"}
    in_maps = [dict(weights, x=x[b]) for b in range(B)]
    res = run_bass_kernel_spmd(nc, in_maps, list(range(B)))
    return np.stack([res.results[b]["out"] for b in range(B)], axis=0)


# revision 28
# speedup vs baseline: 11908.2224x; 1.0706x over previous
"""Trainium2 Bass kernel for a pre-LN transformer block (MHA + FFN).

Data-parallel over batch: 8 NeuronCores, one batch element each.
All matmuls run as float32r (full PE rate at free-dim>=256), storage fp32.
"""
import sys

for _p in ("/opt/trn_rl_repo", "/root/.axon_site/_ro/trn_rl_repo"):
    if _p not in sys.path:
        sys.path.insert(0, _p)

import numpy as np
import concourse.bass as bass
import concourse.tile as tile
from concourse import bacc, mybir
from concourse.bass import ds, ts
from concourse.bass_utils import run_bass_kernel_spmd
from concourse.masks import make_identity

P = 128
N = 1024          # tokens per core (seq len)
D = 1024          # d_emb
H = 16            # heads
HS = 64           # head size
FF = 4096         # ffn hidden
NT = N // P       # 8 token tiles
DB = D // P       # 8 d blocks
EBS = D // P      # 8 e blocks (qkv out features)
NH = 2            # n halves of 512
LN_EPS = 1e-5

F32 = mybir.dt.float32
R = mybir.dt.float32r
AF = mybir.ActivationFunctionType
OP = mybir.AluOpType

_CACHED_NC = None


def build_nc(use_lrelu=True):
    nc = bacc.Bacc("TRN2", target_bir_lowering=False, debug=False, num_devices=8)

    x_d = nc.dram_tensor("x", [N, D], F32, kind="ExternalInput").ap()
    wq_d = nc.dram_tensor("Wq", [H, D, HS], F32, kind="ExternalInput").ap()
    bq_d = nc.dram_tensor("bq", [H, HS], F32, kind="ExternalInput").ap()
    wk_d = nc.dram_tensor("Wk", [H, D, HS], F32, kind="ExternalInput").ap()
    bk_d = nc.dram_tensor("bk", [H, HS], F32, kind="ExternalInput").ap()
    wv_d = nc.dram_tensor("Wv", [H, D, HS], F32, kind="ExternalInput").ap()
    bv_d = nc.dram_tensor("bv", [H, HS], F32, kind="ExternalInput").ap()
    wp_d = nc.dram_tensor("Wproj", [H * HS, D], F32, kind="ExternalInput").ap()
    bp_d = nc.dram_tensor("bproj", [D], F32, kind="ExternalInput").ap()
    w1_d = nc.dram_tensor("W1", [D, FF], F32, kind="ExternalInput").ap()
    b1_d = nc.dram_tensor("b1", [FF], F32, kind="ExternalInput").ap()
    w2_d = nc.dram_tensor("W2", [FF, D], F32, kind="ExternalInput").ap()
    b2_d = nc.dram_tensor("b2", [D], F32, kind="ExternalInput").ap()
    g1_d = nc.dram_tensor("ln1_g", [D], F32, kind="ExternalInput").ap()
    c1_d = nc.dram_tensor("ln1_b", [D], F32, kind="ExternalInput").ap()
    g2_d = nc.dram_tensor("ln2_g", [D], F32, kind="ExternalInput").ap()
    c2_d = nc.dram_tensor("ln2_b", [D], F32, kind="ExternalInput").ap()
    out_d = nc.dram_tensor("out", [N, D], F32, kind="ExternalOutput").ap()
    x2pb_d = nc.dram_tensor("x2pb_scratch", [P, NT, D], F32).ap()

    with tile.TileContext(nc) as tc:
        with tc.tile_pool(name="cn", bufs=1) as cp, \
             tc.tile_pool(name="big", bufs=1) as bp:
            # ---- constants / bias vectors (persistent, tiny) ----
            ident = cp.tile([P, P], F32)
            make_identity(nc, ident[:])
            ones_f = cp.tile([P, 1], F32)
            nc.vector.memset(ones_f[:], 1.0)
            ones64 = cp.tile([1, HS], R)
            nc.vector.tensor_copy(ones64[:],
                                  ones_f[0:1, :].to_broadcast([1, HS]))
            onesP = cp.tile([1, P], R)
            nc.vector.tensor_copy(onesP[:],
                                  ones_f[0:1, :].to_broadcast([1, P]))
            epsv = cp.tile([P, 1], F32)
            nc.vector.memset(epsv[:], LN_EPS)

            # x load first so the big DMA isn't stuck behind the
            # scattered little bias loads
            xsb = bp.tile([P, NT, D], F32, tag="at", name="xsb")
            xr3 = x_d.rearrange("(t p) d -> p t d", p=P)
            for tb in range(NT):
                nc.sync.dma_start(xsb[:, tb, :], xr3[:, tb, :])

            bqv = cp.tile([P, EBS], F32)
            nc.sync.dma_start(bqv[:], bq_d.rearrange("h s -> (h s)")
                              .rearrange("(b p) -> p b", p=P))
            bkv = cp.tile([P, EBS], F32)
            nc.sync.dma_start(bkv[:], bk_d.rearrange("h s -> (h s)")
                              .rearrange("(b p) -> p b", p=P))
            bvv = cp.tile([P, EBS], F32)
            nc.sync.dma_start(bvv[:], bv_d.rearrange("h s -> (h s)")
                              .rearrange("(b p) -> p b", p=P))
            g1v = cp.tile([P, DB], F32)
            nc.sync.dma_start(g1v[:], g1_d.rearrange("(b p) -> p b", p=P))
            c1v = cp.tile([P, DB], F32)
            nc.sync.dma_start(c1v[:], c1_d.rearrange("(b p) -> p b", p=P))
            g2v = cp.tile([P, DB], F32)
            nc.sync.dma_start(g2v[:], g2_d.rearrange("(b p) -> p b", p=P))
            c2v = cp.tile([P, DB], F32)
            nc.sync.dma_start(c2v[:], c2_d.rearrange("(b p) -> p b", p=P))
            b1v = cp.tile([P, FF // P], F32)
            nc.sync.dma_start(b1v[:], b1_d.rearrange("(b p) -> p b", p=P))

            # LN stats scratch (reused for LN2 by tag)
            st_sum = cp.tile([P, NT], F32)
            st_sq = cp.tile([P, NT], F32)
            st_mu = cp.tile([P, NT], F32)
            st_var = cp.tile([P, NT], F32)
            st_rs = cp.tile([P, NT], F32)
            st_nm = cp.tile([P, NT], F32)
            st_vh = cp.tile([P, NT], F32)
            st_t = cp.tile([P, NT], F32)
            st_ih = cp.tile([P, NT], mybir.dt.int32)

            def layernorm_transpose(src, dst, gv, cv, pfx, pspool, trbufs,
                                    after_tb=None, tbs=None):
                """src: [P, NT, D] token layout (f32) -> dst [P, DB, N] f32r
                feature layout, with affine (gv, cv per-partition) folded into
                the transpose evacuation. Fully per-tb so each token tile
                flows stats -> normalize -> transpose independently."""
                for tb in (range(NT) if tbs is None else tbs):
                    t1 = (tb, tb + 1)
                    nc.vector.reduce_sum(st_sum[:, t1[0]:t1[1]], src[:, tb, :],
                                         axis=mybir.AxisListType.X)
                    sq = bp.tile([P, D], F32, tag="qb", bufs=2,
                                 name=f"sq{tb}")
                    nc.scalar.activation(sq[:], src[:, tb, :], AF.Square,
                                         accum_out=st_sq[:, t1[0]:t1[1]])
                    sm = st_sum[:, t1[0]:t1[1]]
                    var = st_var[:, t1[0]:t1[1]]
                    rs = st_rs[:, t1[0]:t1[1]]
                    nm = st_nm[:, t1[0]:t1[1]]
                    ih = st_ih[:, t1[0]:t1[1]]
                    vh = st_vh[:, t1[0]:t1[1]]
                    tt = st_t[:, t1[0]:t1[1]]
                    i32 = mybir.dt.int32
                    # var = sq/D - (sum/D)^2 + eps   (depth-3 chain)
                    nc.vector.tensor_tensor(tt, sm, sm, OP.mult)
                    nc.vector.tensor_scalar(var, tt, -1.0 / (D * D), LN_EPS,
                                            OP.mult, OP.add)
                    nc.vector.tensor_scalar(var, st_sq[:, t1[0]:t1[1]],
                                            1.0 / D, var, OP.mult, OP.add)
                    # rstd = rsqrt(var), DVE-only (bit hack + 2 Newton steps)
                    # so the ACT engine never needs the sqrt table set
                    nc.vector.tensor_scalar(ih, var.bitcast(i32), 1, None,
                                            OP.arith_shift_right)
                    nc.vector.tensor_scalar(rs.bitcast(i32), ih, -1,
                                            0x5F3759DF, OP.mult, OP.add)
                    nc.vector.tensor_scalar_mul(vh, var, 0.5)
                    for _ in range(2):
                        nc.vector.tensor_tensor(tt, rs, rs, OP.mult)
                        nc.vector.tensor_tensor(tt, tt, vh, OP.mult)
                        nc.vector.tensor_scalar(tt, tt, -1.0, 1.5,
                                                OP.mult, OP.add)
                        nc.vector.tensor_tensor(rs, rs, tt, OP.mult)
                    # nm = -(sum/D)*rstd
                    nc.vector.tensor_tensor(nm, sm, rs, OP.mult)
                    nc.vector.tensor_scalar_mul(nm, nm, -1.0 / D)
                    tnorm = bp.tile([P, D], F32, tag="kb", bufs=2,
                                    name=f"tn{tb}")
                    nc.vector.tensor_scalar(tnorm[:], src[:, tb, :],
                                            rs, nm, OP.mult, OP.add)
                    for db in range(DB):
                        pt = pspool.tile([P, P], F32, tag="tr", bufs=trbufs,
                                         name=f"ptr{pfx}_{tb}_{db}")
                        nc.tensor.transpose(pt[:], tnorm[:, ts(db, P)],
                                            ident[:])
                        nc.vector.tensor_scalar(dst[:, db, ts(tb, P)], pt[:],
                                                gv[:, db:db + 1],
                                                cv[:, db:db + 1],
                                                OP.mult, OP.add)
                    if after_tb is not None:
                        after_tb(tb)

            # ================= Phase A: LN1 + transpose ====================
            HT = bp.tile([P, DB, N], R, tag="ht", name="HT")
            psAB_cm = tc.tile_pool(name="psAB", bufs=1, space="PSUM")
            psAB = psAB_cm.__enter__()
            layernorm_transpose(xsb, HT, g1v, c1v, "a", psAB, 4)

            # ================= Phase B0: V projection ======================
            Vaug = bp.tile([P, NT, H, HS + 1], R, tag="v", name="Vaug")
            nc.vector.tensor_copy(
                Vaug[:, :, :, HS:HS + 1],
                ones_f[:, None, :].to_broadcast([P, NT, H, 1]))
            if True:
                for eh in range(2):
                    wvt = bp.tile([P, DB, 512], R, tag="se", bufs=2,
                                  name=f"wv{eh}")
                    for do in range(DB):
                        nc.sync.dma_start(
                            wvt[:, do].rearrange("p (h s) -> p h s", s=HS),
                            wv_d[eh * 8:(eh + 1) * 8, ds(do * P, P), :]
                            .rearrange("h dp s -> dp h s")
                            .bitcast(R))
                    for tb in range(NT):
                        pv = psAB.tile([P, 512], F32, tag="qkv", bufs=4,
                                       name=f"pv{eh}_{tb}")
                        for db in range(DB):
                            nc.tensor.matmul(pv[:], HT[:, db, ts(tb, P)],
                                             wvt[:, db, :],
                                             start=(db == 0), stop=(db == DB - 1))
                        nc.scalar.activation(
                            Vaug[:, tb, eh * 8:(eh + 1) * 8, 0:HS],
                            pv[:].rearrange("p (h s) -> p h s", s=HS), AF.Copy)

            # ============ Phase BC: Q/K per e-block fused with attention ===
            psAB_cm.__exit__(None, None, None)
            psBC_cm = tc.tile_pool(name="psBC", bufs=1, space="PSUM")
            psBC = psBC_cm.__enter__()
            attnT = bp.tile([P, EBS, N], R, tag="at", name="attnT")
            # prefetch Wproj during attention (4 x 1MB quarter-tiles)
            wp4 = []
            for g4 in range(4):
                wpt = bp.tile([P, 2, D], R, tag="sh", bufs=4, name=f"wp{g4}")
                nc.sync.dma_start(
                    wpt[:], wp_d[ds(g4 * 256, 256)]
                    .rearrange("(eo ep) d -> ep eo d", ep=P)
                    .bitcast(R))
                wp4.append(wpt)

            if True:
                for eb in range(EBS):
                    wqt = bp.tile([P, DB, P], R, tag="wqk", bufs=2,
                                  name=f"wq{eb}")
                    for do in range(DB):
                        nc.sync.dma_start(
                            wqt[:, do].rearrange("p (h s) -> p h s", s=HS),
                            wq_d[2 * eb:2 * eb + 2, ds(do * P, P), :]
                            .rearrange("h dp s -> dp h s")
                            .bitcast(R))
                    wkt = bp.tile([P, DB, P], R, tag="wqk", bufs=2,
                                  name=f"wk{eb}")
                    for do in range(DB):
                        nc.sync.dma_start(
                            wkt[:, do].rearrange("p (h s) -> p h s", s=HS),
                            wk_d[2 * eb:2 * eb + 2, ds(do * P, P), :]
                            .rearrange("h dp s -> dp h s")
                            .bitcast(R))
                    Qb = bp.tile([P, N], R, tag="qb", bufs=2, name=f"Qb{eb}")
                    Kb = bp.tile([P, N], R, tag="kb", bufs=2, name=f"Kb{eb}")
                    for nh in range(NH):
                        pq = psBC.tile([P, 512], F32, tag="qk", bufs=2,
                                       name=f"pq{eb}_{nh}")
                        for db in range(DB):
                            nc.tensor.matmul(pq[:], wqt[:, db, :],
                                             HT[:, db, ds(nh * 512, 512)],
                                             start=(db == 0), stop=(db == DB - 1))
                        nc.vector.tensor_scalar_add(Qb[:, ds(nh * 512, 512)],
                                                    pq[:], bqv[:, eb:eb + 1])
                        pk = psBC.tile([P, 512], F32, tag="qk", bufs=2,
                                       name=f"pk{eb}_{nh}")
                        for db in range(DB):
                            nc.tensor.matmul(pk[:], wkt[:, db, :],
                                             HT[:, db, ds(nh * 512, 512)],
                                             start=(db == 0), stop=(db == DB - 1))
                        nc.vector.tensor_scalar_add(Kb[:, ds(nh * 512, 512)],
                                                    pk[:], bkv[:, eb:eb + 1])

                    # attention for heads 2eb (partitions 0:64) and
                    # 2eb+1 (partitions 64:128), per n-half of 512
                    for nh in range(NH):
                        pts = [bp.tile([P, NT, 512], R, tag="se", bufs=2,
                                       name=f"PT{eb}_{nh}_{i}")
                               for i in range(2)]
                        # scoresT[m, n] = sum_s K[m,s] Q[n,s]; exp via ACT
                        for mt in range(NT):
                            for i in range(2):
                                base = i * HS
                                pss = psBC.tile([P, 512], F32, tag="sc",
                                                bufs=4, name=f"ps{eb}{nh}{mt}{i}")
                                nc.tensor.matmul(
                                    pss[:],
                                    Kb[base:base + HS, ts(mt, P)],
                                    Qb[base:base + HS, ds(nh * 512, 512)],
                                    start=True, stop=True)
                                nc.scalar.activation(pts[i][:, mt, :], pss[:],
                                                     AF.Exp, scale=0.125)
                        pas = [psBC.tile([HS + 1, 512], F32, tag="at65",
                                         bufs=2, name=f"pa{eb}_{nh}_{i}")
                               for i in range(2)]
                        for mb in range(NT):
                            for i in range(2):
                                nc.tensor.matmul(pas[i][:],
                                                 Vaug[:, mb, 2 * eb + i, :],
                                                 pts[i][:, mb, :],
                                                 start=(mb == 0),
                                                 stop=(mb == NT - 1))
                        for i in range(2):
                            base = i * HS
                            rec = bp.tile([1, 512], F32, tag="rb", bufs=2,
                                          name=f"rc{eb}_{nh}_{i}")
                            nc.vector.reciprocal(rec[:],
                                                 pas[i][HS:HS + 1, :])
                            rbs = bp.tile([HS, 512], F32, tag="rb", bufs=2,
                                          name=f"rb{eb}_{nh}_{i}")
                            nc.gpsimd.partition_broadcast(rbs[:], rec[:])
                            dstA = attnT[base:base + HS, eb,
                                         ds(nh * 512, 512)]
                            nc.vector.tensor_tensor(dstA, pas[i][0:HS, :],
                                                    rbs[:], OP.mult)
                            nc.vector.tensor_scalar_add(
                                dstA, dstA, bvv[base:base + HS, eb:eb + 1])

            # w1(ft0) prefetch into "se" (frees at end of attention);
            # high priority so the DMA issues as soon as the slot frees
            w1pre = bp.tile([P, DB, 512], R, tag="se", bufs=2,
                            name="w1pre")
            with tc.high_priority():
                nc.sync.dma_start(
                    w1pre[:],
                    w1_d[:, ds(0, 512)]
                    .rearrange("(do dp) f -> dp do f", dp=P)
                    .bitcast(R))

            # ================= Phase D: proj + residual ====================
            psBC_cm.__exit__(None, None, None)
            psDE_cm = tc.tile_pool(name="psDE", bufs=1, space="PSUM")
            psDE = psDE_cm.__enter__()
            x2 = bp.tile([P, NT, D], F32, tag="v", name="x2")
            xr = bp.tile([P, NT, D], F32, tag="ht", name="xrl")
            if True:
                # broadcast bproj -> [P, D]
                bprow = bp.tile([1, D], R, tag="kb", bufs=2, name="bprow")
                nc.sync.dma_start(bprow[:], bp_d[None, :].bitcast(R))
                bpB = bp.tile([P, D], F32, tag="qb", bufs=2, name="bpB")
                for dh in range(2):
                    pbb = psDE.tile([P, 512], F32, tag="trb", bufs=1,
                                    name=f"pbb{dh}")
                    nc.tensor.matmul(pbb[:], onesP[:],
                                     bprow[:, ds(dh * 512, 512)],
                                     start=True, stop=True)
                    nc.vector.tensor_copy(bpB[:, ds(dh * 512, 512)], pbb[:])
                for tb in range(NT):
                    nc.sync.dma_start(xr[:, tb, :], xr3[:, tb, :])
                    nc.vector.tensor_tensor(xr[:, tb, :], xr[:, tb, :],
                                            bpB[:], OP.add)
                for tb in range(NT):
                    for dt in range(2):
                        pp = psDE.tile([P, 512], F32, tag="pj", bufs=4,
                                       name=f"pp{tb}_{dt}")
                        for g4 in range(4):
                            for eo in range(2):
                                nc.tensor.matmul(
                                    pp[:], attnT[:, g4 * 2 + eo, ts(tb, P)],
                                    wp4[g4][:, eo, ds(dt * 512, 512)],
                                    start=(g4 == 0 and eo == 0),
                                    stop=(g4 == 3 and eo == 1))
                        nc.vector.tensor_tensor(x2[:, tb, ds(dt * 512, 512)],
                                                pp[:],
                                                xr[:, tb, ds(dt * 512, 512)],
                                                OP.add)

            # ================= Phase E: LN2, transpose, stash x2+b2 ========
            H2T = bp.tile([P, DB, N], R, tag="ht", name="H2T")
            if True:
                def ffn1_group(nt, ft, fc, _unused, pool, ptag, pbufs,
                               ydst, w1t, w1o):
                    p1 = pool.tile([P, 512], F32, tag=ptag, bufs=pbufs,
                                   name=f"p1_{nt}_{ft}_{fc}")
                    for db in range(DB):
                        nc.tensor.matmul(
                            p1[:], w1t[:, db, ds(w1o, P)],
                            H2T[:, db, ds(nt * 512, 512)],
                            start=(db == 0), stop=(db == DB - 1))
                    bf = ft * 4 + fc
                    if use_lrelu:
                        nc.scalar.activation(ydst, p1[:], AF.Prelu,
                                             bias=b1v[:, bf:bf + 1],
                                             alpha=0.01)
                    else:
                        z = bp.tile([P, 512], F32, tag="qb", bufs=2,
                                    name=f"z{nt}_{bf}")
                        nc.scalar.activation(z[:], p1[:], AF.Identity,
                                             bias=b1v[:, bf:bf + 1])
                        zs = bp.tile([P, 512], F32, tag="rb", bufs=2,
                                     name=f"zs{nt}_{bf}")
                        nc.vector.tensor_scalar_mul(zs[:], z[:], 0.01)
                        nc.vector.tensor_tensor(ydst, z[:], zs[:], OP.max)

                layernorm_transpose(x2, H2T, g2v, c2v, "e", psDE, 3)
                # broadcast b2 -> [P, D]; x2 += b2B; stash to DRAM
                b2row = bp.tile([1, D], R, tag="kb", bufs=2, name="b2row")
                nc.sync.dma_start(b2row[:], b2_d[None, :].bitcast(R))
                b2B = bp.tile([P, D], F32, tag="qb", bufs=2, name="b2B")
                for dh in range(2):
                    pb2 = psDE.tile([P, 512], F32, tag="trb", bufs=1,
                                    name=f"pb2{dh}")
                    nc.tensor.matmul(pb2[:], onesP[:],
                                     b2row[:, ds(dh * 512, 512)],
                                     start=True, stop=True)
                    nc.vector.tensor_copy(b2B[:, ds(dh * 512, 512)], pb2[:])
                for tb in range(NT):
                    nc.vector.tensor_tensor(x2[:, tb, :], x2[:, tb, :],
                                            b2B[:], OP.add)
                    nc.sync.dma_start(x2pb_d[:, tb, :], x2[:, tb, :])

            # ================= Phase F: FFN ================================
            psDE_cm.__exit__(None, None, None)
            psF_cm = tc.tile_pool(name="psF", bufs=1, space="PSUM")
            psF = psF_cm.__enter__()
            if True:
                for nt in range(NH):
                    y1 = [bp.tile([P, 16, 512], R, tag=tg,
                                  name=f"y1{nt}{tg}")
                          for tg in ("at", "v")]
                    xcf = bp.tile([P, 4, D], F32, tag="se", bufs=2,
                                  name=f"xcf{nt}")
                    nc.sync.dma_start(xcf[:], x2pb_d[:, nt * 4:(nt + 1) * 4, :])
                    for ft in range(FF // 512):
                        if ft == 0:
                            w1h = [w1pre, w1pre]
                            w1off = [0, 256]
                        else:
                            w1h = []
                            w1off = [0, 0]
                            for hh in range(2):
                                w1t = bp.tile([P, DB, 256], R, tag="sh",
                                              bufs=4, name=f"w1_{nt}_{ft}_{hh}")
                                nc.sync.dma_start(
                                    w1t[:],
                                    w1_d[:, ds(ft * 512 + hh * 256, 256)]
                                    .rearrange("(do dp) f -> dp do f", dp=P)
                                    .bitcast(R))
                                w1h.append(w1t)
                        for fc in range(4):
                            bf = ft * 4 + fc
                            ffn1_group(nt, ft, fc, None, psF, "fp", 8,
                                       y1[bf // 16][:, bf % 16, :],
                                       w1h[fc // 2],
                                       w1off[fc // 2] + (fc % 2) * P)
                    pf2 = [psF.tile([P, 512], F32, tag="fp", bufs=8,
                                    name=f"p2_{nt}_{j}") for j in range(8)]
                    NFT = FF // 512
                    def w2_halves(nt, ft):
                        hs = []
                        for hh in range(2):
                            w2t = bp.tile([P, 2, D], R, tag="sh", bufs=4,
                                          name=f"w2_{nt}_{ft}_{hh}")
                            nc.sync.dma_start(
                                w2t[:],
                                w2_d[ds(ft * 512 + hh * 256, 256), :]
                                .rearrange("(fo fp) d -> fp fo d", fp=P)
                                .bitcast(R))
                            hs.append(w2t)
                        return hs
                    for ft in range(NFT - 1):
                        w2h = w2_halves(nt, ft)
                        for fc in range(4):
                            bf = ft * 4 + fc
                            ysrc = y1[bf // 16][:, bf % 16, :]
                            for tb in range(4):
                                for dt in range(2):
                                    nc.tensor.matmul(
                                        pf2[tb * 2 + dt][:],
                                        ysrc[:, ts(tb, P)],
                                        w2h[fc // 2][:, fc % 2,
                                                     ds(dt * 512, 512)],
                                        start=(ft == 0 and fc == 0),
                                        stop=False)
                    # last f-tile: close each psum group in turn so its evac
                    # and output DMA overlap the remaining groups' matmuls
                    ftl = NFT - 1
                    w2h = w2_halves(nt, ftl)
                    for tb in range(4):
                        for dt in range(2):
                            for fc in range(4):
                                bf = ftl * 4 + fc
                                ysrc = y1[bf // 16][:, bf % 16, :]
                                nc.tensor.matmul(
                                    pf2[tb * 2 + dt][:],
                                    ysrc[:, ts(tb, P)],
                                    w2h[fc // 2][:, fc % 2,
                                                 ds(dt * 512, 512)],
                                    start=False, stop=(fc == 3))
                            rows = ds(nt * 512 + tb * P, P)
                            og = bp.tile([P, 512], F32, tag="rb", bufs=2,
                                         name=f"og{nt}_{tb}_{dt}")
                            nc.vector.tensor_tensor(og[:], pf2[tb * 2 + dt][:],
                                                    xcf[:, tb, ds(dt * 512, 512)],
                                                    OP.add)
                            nc.sync.dma_start(out_d[rows, ds(dt * 512, 512)],
                                              og[:])
            psF_cm.__exit__(None, None, None)
    nc.compile()
    return nc


def get_nc():
    global _CACHED_NC
    if _CACHED_NC is None:
        _CACHED_NC = build_nc()
    return _CACHED_NC


def kernel(**inputs):
    nc = get_nc()
    x = np.ascontiguousarray(np.asarray(inputs["x"], dtype=np.float32))
    B = x.shape[0]
    weights = {k: np.ascontiguousarray(np.asarray(v, dtype=np.float32))
               for k, v in inputs.items() if k != "x"}
    in_maps = [dict(weights, x=x[b]) for b in range(B)]
    res = run_bass_kernel_spmd(nc, in_maps, list(range(B)))
    return np.stack([res.results[b]["out"] for b in range(B)], axis=0)


# revision 32
# speedup vs baseline: 11943.9534x; 1.0030x over previous
"""Trainium2 Bass kernel for a pre-LN transformer block (MHA + FFN).

Data-parallel over batch: 8 NeuronCores, one batch element each.
All matmuls run as float32r (full PE rate at free-dim>=256), storage fp32.
"""
import sys

for _p in ("/opt/trn_rl_repo", "/root/.axon_site/_ro/trn_rl_repo"):
    if _p not in sys.path:
        sys.path.insert(0, _p)

import numpy as np
import concourse.bass as bass
import concourse.tile as tile
from concourse import bacc, mybir
from concourse.bass import ds, ts
from concourse.bass_utils import run_bass_kernel_spmd
from concourse.masks import make_identity

P = 128
N = 1024          # tokens per core (seq len)
D = 1024          # d_emb
H = 16            # heads
HS = 64           # head size
FF = 4096         # ffn hidden
NT = N // P       # 8 token tiles
DB = D // P       # 8 d blocks
EBS = D // P      # 8 e blocks (qkv out features)
NH = 2            # n halves of 512
LN_EPS = 1e-5

F32 = mybir.dt.float32
R = mybir.dt.float32r
AF = mybir.ActivationFunctionType
OP = mybir.AluOpType

_CACHED_NC = None


def build_nc(use_lrelu=True):
    nc = bacc.Bacc("TRN2", target_bir_lowering=False, debug=False, num_devices=8)

    x_d = nc.dram_tensor("x", [N, D], F32, kind="ExternalInput").ap()
    wq_d = nc.dram_tensor("Wq", [H, D, HS], F32, kind="ExternalInput").ap()
    bq_d = nc.dram_tensor("bq", [H, HS], F32, kind="ExternalInput").ap()
    wk_d = nc.dram_tensor("Wk", [H, D, HS], F32, kind="ExternalInput").ap()
    bk_d = nc.dram_tensor("bk", [H, HS], F32, kind="ExternalInput").ap()
    wv_d = nc.dram_tensor("Wv", [H, D, HS], F32, kind="ExternalInput").ap()
    bv_d = nc.dram_tensor("bv", [H, HS], F32, kind="ExternalInput").ap()
    wp_d = nc.dram_tensor("Wproj", [H * HS, D], F32, kind="ExternalInput").ap()
    bp_d = nc.dram_tensor("bproj", [D], F32, kind="ExternalInput").ap()
    w1_d = nc.dram_tensor("W1", [D, FF], F32, kind="ExternalInput").ap()
    b1_d = nc.dram_tensor("b1", [FF], F32, kind="ExternalInput").ap()
    w2_d = nc.dram_tensor("W2", [FF, D], F32, kind="ExternalInput").ap()
    b2_d = nc.dram_tensor("b2", [D], F32, kind="ExternalInput").ap()
    g1_d = nc.dram_tensor("ln1_g", [D], F32, kind="ExternalInput").ap()
    c1_d = nc.dram_tensor("ln1_b", [D], F32, kind="ExternalInput").ap()
    g2_d = nc.dram_tensor("ln2_g", [D], F32, kind="ExternalInput").ap()
    c2_d = nc.dram_tensor("ln2_b", [D], F32, kind="ExternalInput").ap()
    out_d = nc.dram_tensor("out", [N, D], F32, kind="ExternalOutput").ap()
    x2pb_d = nc.dram_tensor("x2pb_scratch", [P, NT, D], F32).ap()

    with tile.TileContext(nc) as tc:
        with tc.tile_pool(name="cn", bufs=1) as cp, \
             tc.tile_pool(name="big", bufs=1) as bp:
            # ---- constants / bias vectors (persistent, tiny) ----
            ident = cp.tile([P, P], F32)
            make_identity(nc, ident[:])
            ones_f = cp.tile([P, 1], F32)
            nc.vector.memset(ones_f[:], 1.0)
            ones64 = cp.tile([1, HS], R)
            nc.vector.tensor_copy(ones64[:],
                                  ones_f[0:1, :].to_broadcast([1, HS]))
            onesP = cp.tile([1, P], R)
            nc.vector.tensor_copy(onesP[:],
                                  ones_f[0:1, :].to_broadcast([1, P]))
            epsv = cp.tile([P, 1], F32)
            nc.vector.memset(epsv[:], LN_EPS)
            identR = cp.tile([P, P], R)
            nc.vector.tensor_copy(identR[:], ident[:])

            # x load first so the big DMA isn't stuck behind the
            # scattered little bias loads
            xsb = bp.tile([P, NT, D], F32, tag="at", name="xsb")
            xr3 = x_d.rearrange("(t p) d -> p t d", p=P)
            for tb in range(NT):
                nc.sync.dma_start(xsb[:, tb, :], xr3[:, tb, :])

            bqv = cp.tile([P, EBS], F32)
            nc.sync.dma_start(bqv[:], bq_d.rearrange("h s -> (h s)")
                              .rearrange("(b p) -> p b", p=P))
            bkv = cp.tile([P, EBS], F32)
            nc.sync.dma_start(bkv[:], bk_d.rearrange("h s -> (h s)")
                              .rearrange("(b p) -> p b", p=P))
            bvv = cp.tile([P, EBS], F32)
            nc.sync.dma_start(bvv[:], bv_d.rearrange("h s -> (h s)")
                              .rearrange("(b p) -> p b", p=P))
            g1v = cp.tile([P, DB], F32)
            nc.sync.dma_start(g1v[:], g1_d.rearrange("(b p) -> p b", p=P))
            c1v = cp.tile([P, DB], F32)
            nc.sync.dma_start(c1v[:], c1_d.rearrange("(b p) -> p b", p=P))
            g2v = cp.tile([P, DB], F32)
            nc.sync.dma_start(g2v[:], g2_d.rearrange("(b p) -> p b", p=P))
            c2v = cp.tile([P, DB], F32)
            nc.sync.dma_start(c2v[:], c2_d.rearrange("(b p) -> p b", p=P))
            b1v = cp.tile([P, FF // P], F32)
            nc.sync.dma_start(b1v[:], b1_d.rearrange("(b p) -> p b", p=P))

            # LN stats scratch (reused for LN2 by tag)
            st_sum = cp.tile([P, NT], F32)
            st_sq = cp.tile([P, NT], F32)
            st_mu = cp.tile([P, NT], F32)
            st_var = cp.tile([P, NT], F32)
            st_rs = cp.tile([P, NT], F32)
            st_nm = cp.tile([P, NT], F32)
            st_vh = cp.tile([P, NT], F32)
            st_t = cp.tile([P, NT], F32)
            st_ih = cp.tile([P, NT], mybir.dt.int32)

            def layernorm_transpose(src, dst, gv, cv, pfx, pspool, trbufs,
                                    after_tb=None, tbs=None):
                """src: [P, NT, D] token layout (f32) -> dst [P, DB, N] f32r
                feature layout, with affine (gv, cv per-partition) folded into
                the transpose evacuation. Fully per-tb so each token tile
                flows stats -> normalize -> transpose independently."""
                for tb in (range(NT) if tbs is None else tbs):
                    t1 = (tb, tb + 1)
                    nc.vector.reduce_sum(st_sum[:, t1[0]:t1[1]], src[:, tb, :],
                                         axis=mybir.AxisListType.X)
                    sq = bp.tile([P, D], F32, tag="qb", bufs=2,
                                 name=f"sq{tb}")
                    nc.scalar.activation(sq[:], src[:, tb, :], AF.Square,
                                         accum_out=st_sq[:, t1[0]:t1[1]])
                    sm = st_sum[:, t1[0]:t1[1]]
                    var = st_var[:, t1[0]:t1[1]]
                    rs = st_rs[:, t1[0]:t1[1]]
                    nm = st_nm[:, t1[0]:t1[1]]
                    ih = st_ih[:, t1[0]:t1[1]]
                    vh = st_vh[:, t1[0]:t1[1]]
                    tt = st_t[:, t1[0]:t1[1]]
                    i32 = mybir.dt.int32
                    # var = sq/D - (sum/D)^2 + eps   (depth-3 chain)
                    nc.vector.tensor_tensor(tt, sm, sm, OP.mult)
                    nc.vector.tensor_scalar(var, tt, -1.0 / (D * D), LN_EPS,
                                            OP.mult, OP.add)
                    nc.vector.tensor_scalar(var, st_sq[:, t1[0]:t1[1]],
                                            1.0 / D, var, OP.mult, OP.add)
                    # rstd = rsqrt(var), DVE-only (bit hack + 2 Newton steps)
                    # so the ACT engine never needs the sqrt table set
                    nc.vector.tensor_scalar(ih, var.bitcast(i32), 1, None,
                                            OP.arith_shift_right)
                    nc.vector.tensor_scalar(rs.bitcast(i32), ih, -1,
                                            0x5F3759DF, OP.mult, OP.add)
                    nc.vector.tensor_scalar_mul(vh, var, 0.5)
                    for _ in range(2):
                        nc.vector.tensor_tensor(tt, rs, rs, OP.mult)
                        nc.vector.tensor_tensor(tt, tt, vh, OP.mult)
                        nc.vector.tensor_scalar(tt, tt, -1.0, 1.5,
                                                OP.mult, OP.add)
                        nc.vector.tensor_tensor(rs, rs, tt, OP.mult)
                    # nm = -(sum/D)*rstd
                    nc.vector.tensor_tensor(nm, sm, rs, OP.mult)
                    nc.vector.tensor_scalar_mul(nm, nm, -1.0 / D)
                    tnorm = bp.tile([P, D], F32, tag="kb", bufs=2,
                                    name=f"tn{tb}")
                    nc.vector.tensor_scalar(tnorm[:], src[:, tb, :],
                                            rs, nm, OP.mult, OP.add)
                    for db in range(DB):
                        pt = pspool.tile([P, P], F32, tag="tr", bufs=trbufs,
                                         name=f"ptr{pfx}_{tb}_{db}")
                        nc.tensor.transpose(pt[:], tnorm[:, ts(db, P)],
                                            ident[:])
                        nc.vector.tensor_scalar(dst[:, db, ts(tb, P)], pt[:],
                                                gv[:, db:db + 1],
                                                cv[:, db:db + 1],
                                                OP.mult, OP.add)
                    if after_tb is not None:
                        after_tb(tb)

            # ================= Phase A: LN1 + transpose ====================
            HT = bp.tile([P, DB, N], R, tag="ht", name="HT")
            psAB_cm = tc.tile_pool(name="psAB", bufs=1, space="PSUM")
            psAB = psAB_cm.__enter__()
            layernorm_transpose(xsb, HT, g1v, c1v, "a", psAB, 4)

            # ================= Phase B0: V projection ======================
            Vaug = bp.tile([P, NT, H, HS + 1], R, tag="v", name="Vaug")
            nc.vector.tensor_copy(
                Vaug[:, :, :, HS:HS + 1],
                ones_f[:, None, :].to_broadcast([P, NT, H, 1]))
            if True:
                for eh in range(2):
                    wvt = bp.tile([P, DB, 512], R, tag="se", bufs=2,
                                  name=f"wv{eh}")
                    for do in range(DB):
                        nc.sync.dma_start(
                            wvt[:, do].rearrange("p (h s) -> p h s", s=HS),
                            wv_d[eh * 8:(eh + 1) * 8, ds(do * P, P), :]
                            .rearrange("h dp s -> dp h s")
                            .bitcast(R))
                    for tb in range(NT):
                        pv = psAB.tile([P, 512], F32, tag="qkv", bufs=4,
                                       name=f"pv{eh}_{tb}")
                        for db in range(DB):
                            nc.tensor.matmul(pv[:], HT[:, db, ts(tb, P)],
                                             wvt[:, db, :],
                                             start=(db == 0), stop=(db == DB - 1))
                        nc.scalar.activation(
                            Vaug[:, tb, eh * 8:(eh + 1) * 8, 0:HS],
                            pv[:].rearrange("p (h s) -> p h s", s=HS), AF.Copy)

            # ============ Phase BC: Q/K per e-block fused with attention ===
            psAB_cm.__exit__(None, None, None)
            psBC_cm = tc.tile_pool(name="psBC", bufs=1, space="PSUM")
            psBC = psBC_cm.__enter__()
            attnT = bp.tile([P, EBS, N], R, tag="at", name="attnT")
            # prefetch Wproj during attention (4 x 1MB quarter-tiles)
            wp4 = []
            for g4 in range(4):
                wpt = bp.tile([P, 2, D], R, tag="sh", bufs=4, name=f"wp{g4}")
                nc.sync.dma_start(
                    wpt[:], wp_d[ds(g4 * 256, 256)]
                    .rearrange("(eo ep) d -> ep eo d", ep=P)
                    .bitcast(R))
                wp4.append(wpt)

            if True:
                for eb in range(EBS):
                    wqt = bp.tile([P, DB, P], R, tag="wqk", bufs=2,
                                  name=f"wq{eb}")
                    for do in range(DB):
                        nc.sync.dma_start(
                            wqt[:, do].rearrange("p (h s) -> p h s", s=HS),
                            wq_d[2 * eb:2 * eb + 2, ds(do * P, P), :]
                            .rearrange("h dp s -> dp h s")
                            .bitcast(R))
                    wkt = bp.tile([P, DB, P], R, tag="wqk", bufs=2,
                                  name=f"wk{eb}")
                    for do in range(DB):
                        nc.sync.dma_start(
                            wkt[:, do].rearrange("p (h s) -> p h s", s=HS),
                            wk_d[2 * eb:2 * eb + 2, ds(do * P, P), :]
                            .rearrange("h dp s -> dp h s")
                            .bitcast(R))
                    Qb = bp.tile([P, N], R, tag="qb", bufs=2, name=f"Qb{eb}")
                    Kb = bp.tile([P, N], R, tag="kb", bufs=2, name=f"Kb{eb}")
                    for nh in range(NH):
                        pq = psBC.tile([P, 512], F32, tag="qk", bufs=2,
                                       name=f"pq{eb}_{nh}")
                        for db in range(DB):
                            nc.tensor.matmul(pq[:], wqt[:, db, :],
                                             HT[:, db, ds(nh * 512, 512)],
                                             start=(db == 0), stop=(db == DB - 1))
                        nc.vector.tensor_scalar_add(Qb[:, ds(nh * 512, 512)],
                                                    pq[:], bqv[:, eb:eb + 1])
                        pk = psBC.tile([P, 512], F32, tag="qk", bufs=2,
                                       name=f"pk{eb}_{nh}")
                        for db in range(DB):
                            nc.tensor.matmul(pk[:], wkt[:, db, :],
                                             HT[:, db, ds(nh * 512, 512)],
                                             start=(db == 0), stop=(db == DB - 1))
                        nc.vector.tensor_scalar_add(Kb[:, ds(nh * 512, 512)],
                                                    pk[:], bkv[:, eb:eb + 1])

                    # attention for heads 2eb (partitions 0:64) and
                    # 2eb+1 (partitions 64:128), per n-half of 512
                    for nh in range(NH):
                        pts = [bp.tile([P, NT, 512], R, tag="se", bufs=2,
                                       name=f"PT{eb}_{nh}_{i}")
                               for i in range(2)]
                        # scoresT[m, n] = sum_s K[m,s] Q[n,s]; exp via ACT
                        for mt in range(NT):
                            for i in range(2):
                                base = i * HS
                                pss = psBC.tile([P, 512], F32, tag="sc",
                                                bufs=4, name=f"ps{eb}{nh}{mt}{i}")
                                nc.tensor.matmul(
                                    pss[:],
                                    Kb[base:base + HS, ts(mt, P)],
                                    Qb[base:base + HS, ds(nh * 512, 512)],
                                    start=True, stop=True)
                                nc.scalar.activation(pts[i][:, mt, :], pss[:],
                                                     AF.Exp, scale=0.125)
                        pas = [psBC.tile([HS + 1, 512], F32, tag="at65",
                                         bufs=2, name=f"pa{eb}_{nh}_{i}")
                               for i in range(2)]
                        for mb in range(NT):
                            for i in range(2):
                                nc.tensor.matmul(pas[i][:],
                                                 Vaug[:, mb, 2 * eb + i, :],
                                                 pts[i][:, mb, :],
                                                 start=(mb == 0),
                                                 stop=(mb == NT - 1))
                        for i in range(2):
                            base = i * HS
                            rec = bp.tile([1, 512], F32, tag="rb", bufs=2,
                                          name=f"rc{eb}_{nh}_{i}")
                            nc.vector.reciprocal(rec[:],
                                                 pas[i][HS:HS + 1, :])
                            rbs = bp.tile([HS, 512], F32, tag="rb", bufs=2,
                                          name=f"rb{eb}_{nh}_{i}")
                            nc.gpsimd.partition_broadcast(rbs[:], rec[:])
                            dstA = attnT[base:base + HS, eb,
                                         ds(nh * 512, 512)]
                            nc.vector.tensor_tensor(dstA, pas[i][0:HS, :],
                                                    rbs[:], OP.mult)
                            nc.vector.tensor_scalar_add(
                                dstA, dstA, bvv[base:base + HS, eb:eb + 1])

            # w1(ft0) prefetch into "se" (frees at end of attention);
            # high priority so the DMA issues as soon as the slot frees
            w1pre = bp.tile([P, DB, 512], R, tag="se", bufs=2,
                            name="w1pre")
            with tc.high_priority():
                nc.sync.dma_start(
                    w1pre[:],
                    w1_d[:, ds(0, 512)]
                    .rearrange("(do dp) f -> dp do f", dp=P)
                    .bitcast(R))

            # ================= Phase D: proj + residual ====================
            psBC_cm.__exit__(None, None, None)
            psDE_cm = tc.tile_pool(name="psDE", bufs=1, space="PSUM")
            psDE = psDE_cm.__enter__()
            x2 = bp.tile([P, NT, D], F32, tag="v", name="x2")
            xr = bp.tile([P, NT, D], R, tag="ht", name="xrl")
            if True:
                # broadcast bproj -> [P, D]
                bprow = bp.tile([1, D], R, tag="kb", bufs=2, name="bprow")
                nc.sync.dma_start(bprow[:], bp_d[None, :].bitcast(R))
                bpB = bp.tile([P, D], F32, tag="qb", bufs=2, name="bpB")
                for dh in range(2):
                    pbb = psDE.tile([P, 512], F32, tag="trb", bufs=1,
                                    name=f"pbb{dh}")
                    nc.tensor.matmul(pbb[:], onesP[:],
                                     bprow[:, ds(dh * 512, 512)],
                                     start=True, stop=True)
                    nc.vector.tensor_copy(bpB[:, ds(dh * 512, 512)], pbb[:])
                for tb in range(NT):
                    nc.sync.dma_start(xr[:, tb, :],
                                      xr3[:, tb, :].bitcast(R))
                    nc.vector.tensor_tensor(xr[:, tb, :], xr[:, tb, :],
                                            bpB[:], OP.add)
                for tb in range(NT):
                    for dt in range(2):
                        pp = psDE.tile([P, 512], F32, tag="pj", bufs=4,
                                       name=f"pp{tb}_{dt}")
                        for g4 in range(4):
                            for eo in range(2):
                                nc.tensor.matmul(
                                    pp[:], attnT[:, g4 * 2 + eo, ts(tb, P)],
                                    wp4[g4][:, eo, ds(dt * 512, 512)],
                                    start=(g4 == 0 and eo == 0),
                                    stop=False)
                        # residual folded into the PE accumulation
                        nc.tensor.matmul(pp[:], identR[:],
                                         xr[:, tb, ds(dt * 512, 512)],
                                         start=False, stop=True)
                        nc.scalar.activation(x2[:, tb, ds(dt * 512, 512)],
                                             pp[:], AF.Copy)

            # ================= Phase E: LN2, transpose, stash x2+b2 ========
            H2T = bp.tile([P, DB, N], R, tag="ht", name="H2T")
            if True:
                def ffn1_group(nt, ft, fc, _unused, pool, ptag, pbufs,
                               ydst, w1t, w1o):
                    p1 = pool.tile([P, 512], F32, tag=ptag, bufs=pbufs,
                                   name=f"p1_{nt}_{ft}_{fc}")
                    for db in range(DB):
                        nc.tensor.matmul(
                            p1[:], w1t[:, db, ds(w1o, P)],
                            H2T[:, db, ds(nt * 512, 512)],
                            start=(db == 0), stop=(db == DB - 1))
                    bf = ft * 4 + fc
                    if use_lrelu:
                        nc.scalar.activation(ydst, p1[:], AF.Prelu,
                                             bias=b1v[:, bf:bf + 1],
                                             alpha=0.01)
                    else:
                        z = bp.tile([P, 512], F32, tag="qb", bufs=2,
                                    name=f"z{nt}_{bf}")
                        nc.scalar.activation(z[:], p1[:], AF.Identity,
                                             bias=b1v[:, bf:bf + 1])
                        zs = bp.tile([P, 512], F32, tag="rb", bufs=2,
                                     name=f"zs{nt}_{bf}")
                        nc.vector.tensor_scalar_mul(zs[:], z[:], 0.01)
                        nc.vector.tensor_tensor(ydst, z[:], zs[:], OP.max)

                layernorm_transpose(x2, H2T, g2v, c2v, "e", psDE, 3)
                # broadcast b2 -> [P, D]; x2 += b2B; stash to DRAM
                b2row = bp.tile([1, D], R, tag="kb", bufs=2, name="b2row")
                nc.sync.dma_start(b2row[:], b2_d[None, :].bitcast(R))
                b2B = bp.tile([P, D], F32, tag="qb", bufs=2, name="b2B")
                for dh in range(2):
                    pb2 = psDE.tile([P, 512], F32, tag="trb", bufs=1,
                                    name=f"pb2{dh}")
                    nc.tensor.matmul(pb2[:], onesP[:],
                                     b2row[:, ds(dh * 512, 512)],
                                     start=True, stop=True)
                    nc.vector.tensor_copy(b2B[:, ds(dh * 512, 512)], pb2[:])
                for tb in range(NT):
                    nc.vector.tensor_tensor(x2[:, tb, :], x2[:, tb, :],
                                            b2B[:], OP.add)
                    nc.sync.dma_start(x2pb_d[:, tb, :], x2[:, tb, :])

            # ================= Phase F: FFN ================================
            psDE_cm.__exit__(None, None, None)
            psF_cm = tc.tile_pool(name="psF", bufs=1, space="PSUM")
            psF = psF_cm.__enter__()
            if True:
                for nt in range(NH):
                    y1 = [bp.tile([P, 16, 512], R, tag=tg,
                                  name=f"y1{nt}{tg}")
                          for tg in ("at", "v")]
                    xcf = bp.tile([P, 4, D], F32, tag="se", bufs=2,
                                  name=f"xcf{nt}")
                    nc.sync.dma_start(xcf[:], x2pb_d[:, nt * 4:(nt + 1) * 4, :])
                    for ft in range(FF // 512):
                        if ft == 0:
                            w1h = [w1pre, w1pre]
                            w1off = [0, 256]
                        else:
                            w1h = []
                            w1off = [0, 0]
                            for hh in range(2):
                                w1t = bp.tile([P, DB, 256], R, tag="sh",
                                              bufs=4, name=f"w1_{nt}_{ft}_{hh}")
                                nc.sync.dma_start(
                                    w1t[:],
                                    w1_d[:, ds(ft * 512 + hh * 256, 256)]
                                    .rearrange("(do dp) f -> dp do f", dp=P)
                                    .bitcast(R))
                                w1h.append(w1t)
                        for fc in range(4):
                            bf = ft * 4 + fc
                            ffn1_group(nt, ft, fc, None, psF, "fp", 8,
                                       y1[bf // 16][:, bf % 16, :],
                                       w1h[fc // 2],
                                       w1off[fc // 2] + (fc % 2) * P)
                    pf2 = [psF.tile([P, 512], F32, tag="fp", bufs=8,
                                    name=f"p2_{nt}_{j}") for j in range(8)]
                    NFT = FF // 512
                    def w2_halves(nt, ft):
                        hs = []
                        for hh in range(2):
                            w2t = bp.tile([P, 2, D], R, tag="sh", bufs=4,
                                          name=f"w2_{nt}_{ft}_{hh}")
                            nc.sync.dma_start(
                                w2t[:],
                                w2_d[ds(ft * 512 + hh * 256, 256), :]
                                .rearrange("(fo fp) d -> fp fo d", fp=P)
                                .bitcast(R))
                            hs.append(w2t)
                        return hs
                    for ft in range(NFT - 1):
                        w2h = w2_halves(nt, ft)
                        for fc in range(4):
                            bf = ft * 4 + fc
                            ysrc = y1[bf // 16][:, bf % 16, :]
                            for tb in range(4):
                                for dt in range(2):
                                    nc.tensor.matmul(
                                        pf2[tb * 2 + dt][:],
                                        ysrc[:, ts(tb, P)],
                                        w2h[fc // 2][:, fc % 2,
                                                     ds(dt * 512, 512)],
                                        start=(ft == 0 and fc == 0),
                                        stop=False)
                    # last f-tile: close each psum group in turn so its evac
                    # and output DMA overlap the remaining groups' matmuls
                    ftl = NFT - 1
                    w2h = w2_halves(nt, ftl)
                    for tb in range(4):
                        for dt in range(2):
                            for fc in range(4):
                                bf = ftl * 4 + fc
                                ysrc = y1[bf // 16][:, bf % 16, :]
                                nc.tensor.matmul(
                                    pf2[tb * 2 + dt][:],
                                    ysrc[:, ts(tb, P)],
                                    w2h[fc // 2][:, fc % 2,
                                                 ds(dt * 512, 512)],
                                    start=False, stop=(fc == 3))
                            rows = ds(nt * 512 + tb * P, P)
                            og = bp.tile([P, 512], F32, tag="rb", bufs=2,
                                         name=f"og{nt}_{tb}_{dt}")
                            nc.vector.tensor_tensor(og[:], pf2[tb * 2 + dt][:],
                                                    xcf[:, tb, ds(dt * 512, 512)],
                                                    OP.add)
                            nc.sync.dma_start(out_d[rows, ds(dt * 512, 512)],
                                              og[:])
            psF_cm.__exit__(None, None, None)
    nc.compile()
    return nc


def get_nc():
    global _CACHED_NC
    if _CACHED_NC is None:
        _CACHED_NC = build_nc()
    return _CACHED_NC


def kernel(**inputs):
    nc = get_nc()
    x = np.ascontiguousarray(np.asarray(inputs["x"], dtype=np.float32))
    B = x.shape[0]
    weights = {k: np.ascontiguousarray(np.asarray(v, dtype=np.float32))
               for k, v in inputs.items() if k != "x"}
    in_maps = [dict(weights, x=x[b]) for b in range(B)]
    res = run_bass_kernel_spmd(nc, in_maps, list(range(B)))
    return np.stack([res.results[b]["out"] for b in range(B)], axis=0)


# revision 38
# speedup vs baseline: 11956.5033x; 1.0011x over previous
"""Trainium2 Bass kernel for a pre-LN transformer block (MHA + FFN).

Data-parallel over batch: 8 NeuronCores, one batch element each.
All matmuls run as float32r (full PE rate at free-dim>=256), storage fp32.
"""
import sys

for _p in ("/opt/trn_rl_repo", "/root/.axon_site/_ro/trn_rl_repo"):
    if _p not in sys.path:
        sys.path.insert(0, _p)

import numpy as np
import concourse.bass as bass
import concourse.tile as tile
from concourse import bacc, mybir
from concourse.bass import ds, ts
from concourse.bass_utils import run_bass_kernel_spmd
from concourse.masks import make_identity

P = 128
N = 1024          # tokens per core (seq len)
D = 1024          # d_emb
H = 16            # heads
HS = 64           # head size
FF = 4096         # ffn hidden
NT = N // P       # 8 token tiles
DB = D // P       # 8 d blocks
EBS = D // P      # 8 e blocks (qkv out features)
NH = 2            # n halves of 512
LN_EPS = 1e-5

F32 = mybir.dt.float32
R = mybir.dt.float32r
AF = mybir.ActivationFunctionType
OP = mybir.AluOpType

_CACHED_NC = None


def build_nc(use_lrelu=True):
    nc = bacc.Bacc("TRN2", target_bir_lowering=False, debug=False, num_devices=8)

    x_d = nc.dram_tensor("x", [N, D], F32, kind="ExternalInput").ap()
    wq_d = nc.dram_tensor("Wq", [H, D, HS], F32, kind="ExternalInput").ap()
    bq_d = nc.dram_tensor("bq", [H, HS], F32, kind="ExternalInput").ap()
    wk_d = nc.dram_tensor("Wk", [H, D, HS], F32, kind="ExternalInput").ap()
    bk_d = nc.dram_tensor("bk", [H, HS], F32, kind="ExternalInput").ap()
    wv_d = nc.dram_tensor("Wv", [H, D, HS], F32, kind="ExternalInput").ap()
    bv_d = nc.dram_tensor("bv", [H, HS], F32, kind="ExternalInput").ap()
    wp_d = nc.dram_tensor("Wproj", [H * HS, D], F32, kind="ExternalInput").ap()
    bp_d = nc.dram_tensor("bproj", [D], F32, kind="ExternalInput").ap()
    w1_d = nc.dram_tensor("W1", [D, FF], F32, kind="ExternalInput").ap()
    b1_d = nc.dram_tensor("b1", [FF], F32, kind="ExternalInput").ap()
    w2_d = nc.dram_tensor("W2", [FF, D], F32, kind="ExternalInput").ap()
    b2_d = nc.dram_tensor("b2", [D], F32, kind="ExternalInput").ap()
    g1_d = nc.dram_tensor("ln1_g", [D], F32, kind="ExternalInput").ap()
    c1_d = nc.dram_tensor("ln1_b", [D], F32, kind="ExternalInput").ap()
    g2_d = nc.dram_tensor("ln2_g", [D], F32, kind="ExternalInput").ap()
    c2_d = nc.dram_tensor("ln2_b", [D], F32, kind="ExternalInput").ap()
    out_d = nc.dram_tensor("out", [N, D], F32, kind="ExternalOutput").ap()
    x2pb_d = nc.dram_tensor("x2pb_scratch", [P, NT, D], F32).ap()

    with tile.TileContext(nc) as tc:
        with tc.tile_pool(name="cn", bufs=1) as cp, \
             tc.tile_pool(name="big", bufs=1) as bp:
            # ---- constants / bias vectors (persistent, tiny) ----
            ident = cp.tile([P, P], F32)
            make_identity(nc, ident[:])
            ones_f = cp.tile([P, 1], F32)
            nc.vector.memset(ones_f[:], 1.0)
            ones64 = cp.tile([1, HS], R)
            nc.vector.tensor_copy(ones64[:],
                                  ones_f[0:1, :].to_broadcast([1, HS]))
            onesP = cp.tile([1, P], R)
            nc.vector.tensor_copy(onesP[:],
                                  ones_f[0:1, :].to_broadcast([1, P]))
            epsv = cp.tile([P, 1], F32)
            nc.vector.memset(epsv[:], LN_EPS)
            identR = cp.tile([P, P], R)
            nc.vector.tensor_copy(identR[:], ident[:])

            # x load first so the big DMA isn't stuck behind the
            # scattered little bias loads
            xsb = bp.tile([P, NT, D], F32, tag="at", name="xsb")
            xr3 = x_d.rearrange("(t p) d -> p t d", p=P)
            for tb in range(NT):
                nc.sync.dma_start(xsb[:, tb, :], xr3[:, tb, :])

            bqv = cp.tile([P, EBS], F32)
            nc.sync.dma_start(bqv[:], bq_d.rearrange("h s -> (h s)")
                              .rearrange("(b p) -> p b", p=P))
            bkv = cp.tile([P, EBS], F32)
            nc.sync.dma_start(bkv[:], bk_d.rearrange("h s -> (h s)")
                              .rearrange("(b p) -> p b", p=P))
            bvv = cp.tile([P, EBS], F32)
            nc.sync.dma_start(bvv[:], bv_d.rearrange("h s -> (h s)")
                              .rearrange("(b p) -> p b", p=P))
            g1v = cp.tile([P, DB], F32)
            nc.sync.dma_start(g1v[:], g1_d.rearrange("(b p) -> p b", p=P))
            c1v = cp.tile([P, DB], F32)
            nc.sync.dma_start(c1v[:], c1_d.rearrange("(b p) -> p b", p=P))
            g2v = cp.tile([P, DB], F32)
            nc.sync.dma_start(g2v[:], g2_d.rearrange("(b p) -> p b", p=P))
            c2v = cp.tile([P, DB], F32)
            nc.sync.dma_start(c2v[:], c2_d.rearrange("(b p) -> p b", p=P))
            b1v = cp.tile([P, FF // P], F32)
            nc.sync.dma_start(b1v[:], b1_d.rearrange("(b p) -> p b", p=P))

            # LN stats scratch (reused for LN2 by tag)
            st_sum = cp.tile([P, NT], F32)
            st_sq = cp.tile([P, NT], F32)
            st_mu = cp.tile([P, NT], F32)
            st_var = cp.tile([P, NT], F32)
            st_rs = cp.tile([P, NT], F32)
            st_nm = cp.tile([P, NT], F32)
            st_vh = cp.tile([P, NT], F32)
            st_t = cp.tile([P, NT], F32)
            st_ih = cp.tile([P, NT], mybir.dt.int32)

            def layernorm_transpose(src, dst, gv, cv, pfx, pspool, trbufs,
                                    after_tb=None, tbs=None):
                """src: [P, NT, D] token layout (f32) -> dst [P, DB, N] f32r
                feature layout, with affine (gv, cv per-partition) folded into
                the transpose evacuation. Fully per-tb so each token tile
                flows stats -> normalize -> transpose independently."""
                for tb in (range(NT) if tbs is None else tbs):
                    t1 = (tb, tb + 1)
                    nc.vector.reduce_sum(st_sum[:, t1[0]:t1[1]], src[:, tb, :],
                                         axis=mybir.AxisListType.X)
                    sq = bp.tile([P, D], F32, tag="qb", bufs=2,
                                 name=f"sq{tb}")
                    nc.scalar.activation(sq[:], src[:, tb, :], AF.Square,
                                         accum_out=st_sq[:, t1[0]:t1[1]])
                    sm = st_sum[:, t1[0]:t1[1]]
                    var = st_var[:, t1[0]:t1[1]]
                    rs = st_rs[:, t1[0]:t1[1]]
                    nm = st_nm[:, t1[0]:t1[1]]
                    ih = st_ih[:, t1[0]:t1[1]]
                    vh = st_vh[:, t1[0]:t1[1]]
                    tt = st_t[:, t1[0]:t1[1]]
                    i32 = mybir.dt.int32
                    # var = sq/D - (sum/D)^2 + eps   (depth-3 chain)
                    nc.vector.tensor_tensor(tt, sm, sm, OP.mult)
                    nc.vector.tensor_scalar(var, tt, -1.0 / (D * D), LN_EPS,
                                            OP.mult, OP.add)
                    nc.vector.tensor_scalar(var, st_sq[:, t1[0]:t1[1]],
                                            1.0 / D, var, OP.mult, OP.add)
                    # rstd = rsqrt(var), DVE-only (bit hack + 2 Newton steps)
                    # so the ACT engine never needs the sqrt table set
                    nc.vector.tensor_scalar(ih, var.bitcast(i32), 1, None,
                                            OP.arith_shift_right)
                    nc.vector.tensor_scalar(rs.bitcast(i32), ih, -1,
                                            0x5F3759DF, OP.mult, OP.add)
                    nc.vector.tensor_scalar_mul(vh, var, -0.5)
                    for _ in range(2):
                        nc.vector.tensor_tensor(tt, rs, rs, OP.mult)
                        nc.vector.tensor_scalar(tt, tt, vh, 1.5,
                                                OP.mult, OP.add)
                        nc.vector.tensor_tensor(rs, rs, tt, OP.mult)
                    # nm = -(sum/D)*rstd
                    nc.vector.tensor_tensor(nm, sm, rs, OP.mult)
                    nc.vector.tensor_scalar_mul(nm, nm, -1.0 / D)
                    tnorm = bp.tile([P, D], F32, tag="kb", bufs=2,
                                    name=f"tn{tb}")
                    nc.vector.tensor_scalar(tnorm[:], src[:, tb, :],
                                            rs, nm, OP.mult, OP.add)
                    for db in range(DB):
                        pt = pspool.tile([P, P], F32, tag="tr", bufs=trbufs,
                                         name=f"ptr{pfx}_{tb}_{db}")
                        nc.tensor.transpose(pt[:], tnorm[:, ts(db, P)],
                                            ident[:])
                        nc.vector.tensor_scalar(dst[:, db, ts(tb, P)], pt[:],
                                                gv[:, db:db + 1],
                                                cv[:, db:db + 1],
                                                OP.mult, OP.add)
                    if after_tb is not None:
                        after_tb(tb)

            # ================= Phase A: LN1 + transpose ====================
            HT = bp.tile([P, DB, N], R, tag="ht", name="HT")
            psAB_cm = tc.tile_pool(name="psAB", bufs=1, space="PSUM")
            psAB = psAB_cm.__enter__()
            layernorm_transpose(xsb, HT, g1v, c1v, "a", psAB, 4)

            # ================= Phase B0: V projection ======================
            Vaug = bp.tile([P, NT, H, HS + 1], R, tag="v", name="Vaug")
            nc.vector.tensor_copy(
                Vaug[:, :, :, HS:HS + 1],
                ones_f[:, None, :].to_broadcast([P, NT, H, 1]))
            if True:
                for eh in range(2):
                    wvt = bp.tile([P, DB, 512], R, tag="se", bufs=2,
                                  name=f"wv{eh}")
                    for do in range(DB):
                        nc.sync.dma_start(
                            wvt[:, do].rearrange("p (h s) -> p h s", s=HS),
                            wv_d[eh * 8:(eh + 1) * 8, ds(do * P, P), :]
                            .rearrange("h dp s -> dp h s")
                            .bitcast(R))
                    for tb in range(NT):
                        pv = psAB.tile([P, 512], F32, tag="qkv", bufs=4,
                                       name=f"pv{eh}_{tb}")
                        for db in range(DB):
                            nc.tensor.matmul(pv[:], HT[:, db, ts(tb, P)],
                                             wvt[:, db, :],
                                             start=(db == 0), stop=(db == DB - 1))
                        nc.scalar.activation(
                            Vaug[:, tb, eh * 8:(eh + 1) * 8, 0:HS],
                            pv[:].rearrange("p (h s) -> p h s", s=HS), AF.Copy)

            # ============ Phase BC: Q/K per e-block fused with attention ===
            psAB_cm.__exit__(None, None, None)
            psBC_cm = tc.tile_pool(name="psBC", bufs=1, space="PSUM")
            psBC = psBC_cm.__enter__()
            attnT = bp.tile([P, EBS, N], R, tag="at", name="attnT")
            # prefetch Wproj during attention (4 x 1MB quarter-tiles)
            wp4 = []
            for g4 in range(4):
                wpt = bp.tile([P, 2, D], R, tag="sh", bufs=4, name=f"wp{g4}")
                nc.sync.dma_start(
                    wpt[:], wp_d[ds(g4 * 256, 256)]
                    .rearrange("(eo ep) d -> ep eo d", ep=P)
                    .bitcast(R))
                wp4.append(wpt)

            if True:
                for eb in range(EBS):
                    wqt = bp.tile([P, DB, P], R, tag="wqk", bufs=2,
                                  name=f"wq{eb}")
                    for do in range(DB):
                        nc.sync.dma_start(
                            wqt[:, do].rearrange("p (h s) -> p h s", s=HS),
                            wq_d[2 * eb:2 * eb + 2, ds(do * P, P), :]
                            .rearrange("h dp s -> dp h s")
                            .bitcast(R))
                    wkt = bp.tile([P, DB, P], R, tag="wqk", bufs=2,
                                  name=f"wk{eb}")
                    for do in range(DB):
                        nc.sync.dma_start(
                            wkt[:, do].rearrange("p (h s) -> p h s", s=HS),
                            wk_d[2 * eb:2 * eb + 2, ds(do * P, P), :]
                            .rearrange("h dp s -> dp h s")
                            .bitcast(R))
                    Qb = bp.tile([P, N], R, tag="qb", bufs=2, name=f"Qb{eb}")
                    Kb = bp.tile([P, N], R, tag="kb", bufs=2, name=f"Kb{eb}")
                    for nh in range(NH):
                        pq = psBC.tile([P, 512], F32, tag="qk", bufs=2,
                                       name=f"pq{eb}_{nh}")
                        for db in range(DB):
                            nc.tensor.matmul(pq[:], wqt[:, db, :],
                                             HT[:, db, ds(nh * 512, 512)],
                                             start=(db == 0), stop=(db == DB - 1))
                        nc.vector.tensor_scalar_add(Qb[:, ds(nh * 512, 512)],
                                                    pq[:], bqv[:, eb:eb + 1])
                        pk = psBC.tile([P, 512], F32, tag="qk", bufs=2,
                                       name=f"pk{eb}_{nh}")
                        for db in range(DB):
                            nc.tensor.matmul(pk[:], wkt[:, db, :],
                                             HT[:, db, ds(nh * 512, 512)],
                                             start=(db == 0), stop=(db == DB - 1))
                        nc.vector.tensor_scalar_add(Kb[:, ds(nh * 512, 512)],
                                                    pk[:], bkv[:, eb:eb + 1])

                    # attention for heads 2eb (partitions 0:64) and
                    # 2eb+1 (partitions 64:128), per n-half of 512
                    for nh in range(NH):
                        pts = [bp.tile([P, NT, 512], R, tag="se", bufs=2,
                                       name=f"PT{eb}_{nh}_{i}")
                               for i in range(2)]
                        # scoresT[m, n] = sum_s K[m,s] Q[n,s]; exp via ACT
                        for mt in range(NT):
                            for i in range(2):
                                base = i * HS
                                pss = psBC.tile([P, 512], F32, tag="sc",
                                                bufs=4, name=f"ps{eb}{nh}{mt}{i}")
                                nc.tensor.matmul(
                                    pss[:],
                                    Kb[base:base + HS, ts(mt, P)],
                                    Qb[base:base + HS, ds(nh * 512, 512)],
                                    start=True, stop=True)
                                nc.scalar.activation(pts[i][:, mt, :], pss[:],
                                                     AF.Exp, scale=0.125)
                        pas = [psBC.tile([HS + 1, 512], F32, tag="at65",
                                         bufs=2, name=f"pa{eb}_{nh}_{i}")
                               for i in range(2)]
                        for mb in range(NT):
                            for i in range(2):
                                nc.tensor.matmul(pas[i][:],
                                                 Vaug[:, mb, 2 * eb + i, :],
                                                 pts[i][:, mb, :],
                                                 start=(mb == 0),
                                                 stop=(mb == NT - 1))
                        for i in range(2):
                            base = i * HS
                            rec = bp.tile([1, 512], F32, tag="rb", bufs=2,
                                          name=f"rc{eb}_{nh}_{i}")
                            nc.vector.reciprocal(rec[:],
                                                 pas[i][HS:HS + 1, :])
                            rbs = bp.tile([HS, 512], F32, tag="rb", bufs=2,
                                          name=f"rb{eb}_{nh}_{i}")
                            nc.gpsimd.partition_broadcast(rbs[:], rec[:])
                            dstA = attnT[base:base + HS, eb,
                                         ds(nh * 512, 512)]
                            nc.vector.tensor_tensor(dstA, pas[i][0:HS, :],
                                                    rbs[:], OP.mult)
                            nc.vector.tensor_scalar_add(
                                dstA, dstA, bvv[base:base + HS, eb:eb + 1])

            # w1(ft0) prefetch into "se" (frees at end of attention);
            # high priority so the DMA issues as soon as the slot frees
            w1pre = bp.tile([P, DB, 512], R, tag="se", bufs=2,
                            name="w1pre")
            with tc.high_priority():
                nc.sync.dma_start(
                    w1pre[:],
                    w1_d[:, ds(0, 512)]
                    .rearrange("(do dp) f -> dp do f", dp=P)
                    .bitcast(R))

            # ================= Phase D: proj + residual ====================
            psBC_cm.__exit__(None, None, None)
            psDE_cm = tc.tile_pool(name="psDE", bufs=1, space="PSUM")
            psDE = psDE_cm.__enter__()
            x2 = bp.tile([P, NT, D], F32, tag="v", name="x2")
            xr = bp.tile([P, NT, D], R, tag="ht", name="xrl")
            if True:
                # broadcast bproj -> [P, D]
                bprow = bp.tile([1, D], R, tag="kb", bufs=2, name="bprow")
                nc.sync.dma_start(bprow[:], bp_d[None, :].bitcast(R))
                bpB = bp.tile([P, D], F32, tag="qb", bufs=2, name="bpB")
                for dh in range(2):
                    pbb = psDE.tile([P, 512], F32, tag="trb", bufs=1,
                                    name=f"pbb{dh}")
                    nc.tensor.matmul(pbb[:], onesP[:],
                                     bprow[:, ds(dh * 512, 512)],
                                     start=True, stop=True)
                    nc.vector.tensor_copy(bpB[:, ds(dh * 512, 512)], pbb[:])
                for tb in range(NT):
                    nc.sync.dma_start(xr[:, tb, :],
                                      xr3[:, tb, :].bitcast(R))
                    nc.vector.tensor_tensor(xr[:, tb, :], xr[:, tb, :],
                                            bpB[:], OP.add)
                for tb in range(NT):
                    for dt in range(2):
                        pp = psDE.tile([P, 512], F32, tag="pj", bufs=4,
                                       name=f"pp{tb}_{dt}")
                        for g4 in range(4):
                            for eo in range(2):
                                nc.tensor.matmul(
                                    pp[:], attnT[:, g4 * 2 + eo, ts(tb, P)],
                                    wp4[g4][:, eo, ds(dt * 512, 512)],
                                    start=(g4 == 0 and eo == 0),
                                    stop=False)
                        # residual folded into the PE accumulation
                        nc.tensor.matmul(pp[:], identR[:],
                                         xr[:, tb, ds(dt * 512, 512)],
                                         start=False, stop=True)
                        nc.scalar.activation(x2[:, tb, ds(dt * 512, 512)],
                                             pp[:], AF.Copy)

            # ================= Phase E: LN2, transpose, stash x2+b2 ========
            H2T = bp.tile([P, DB, N], R, tag="ht", name="H2T")
            if True:
                def ffn1_group(nt, ft, fc, _unused, pool, ptag, pbufs,
                               ydst, w1t, w1o):
                    p1 = pool.tile([P, 512], F32, tag=ptag, bufs=pbufs,
                                   name=f"p1_{nt}_{ft}_{fc}")
                    for db in range(DB):
                        nc.tensor.matmul(
                            p1[:], w1t[:, db, ds(w1o, P)],
                            H2T[:, db, ds(nt * 512, 512)],
                            start=(db == 0), stop=(db == DB - 1))
                    bf = ft * 4 + fc
                    if use_lrelu:
                        nc.scalar.activation(ydst, p1[:], AF.Prelu,
                                             bias=b1v[:, bf:bf + 1],
                                             alpha=0.01)
                    else:
                        z = bp.tile([P, 512], F32, tag="qb", bufs=2,
                                    name=f"z{nt}_{bf}")
                        nc.scalar.activation(z[:], p1[:], AF.Identity,
                                             bias=b1v[:, bf:bf + 1])
                        zs = bp.tile([P, 512], F32, tag="rb", bufs=2,
                                     name=f"zs{nt}_{bf}")
                        nc.vector.tensor_scalar_mul(zs[:], z[:], 0.01)
                        nc.vector.tensor_tensor(ydst, z[:], zs[:], OP.max)

                layernorm_transpose(x2, H2T, g2v, c2v, "e", psDE, 3)
                # broadcast b2 -> [P, D]; x2 += b2B; stash to DRAM
                b2row = bp.tile([1, D], R, tag="kb", bufs=2, name="b2row")
                nc.sync.dma_start(b2row[:], b2_d[None, :].bitcast(R))
                b2B = bp.tile([P, D], F32, tag="qb", bufs=2, name="b2B")
                for dh in range(2):
                    pb2 = psDE.tile([P, 512], F32, tag="trb", bufs=1,
                                    name=f"pb2{dh}")
                    nc.tensor.matmul(pb2[:], onesP[:],
                                     b2row[:, ds(dh * 512, 512)],
                                     start=True, stop=True)
                    nc.vector.tensor_copy(b2B[:, ds(dh * 512, 512)], pb2[:])
                for tb in range(NT):
                    nc.vector.tensor_tensor(x2[:, tb, :], x2[:, tb, :],
                                            b2B[:], OP.add)
                    nc.sync.dma_start(x2pb_d[:, tb, :], x2[:, tb, :])

            # ================= Phase F: FFN ================================
            psDE_cm.__exit__(None, None, None)
            psF_cm = tc.tile_pool(name="psF", bufs=1, space="PSUM")
            psF = psF_cm.__enter__()
            if True:
                for nt in range(NH):
                    y1 = [bp.tile([P, 16, 512], R, tag=tg,
                                  name=f"y1{nt}{tg}")
                          for tg in ("at", "v")]
                    xcf = bp.tile([P, 4, D], F32, tag="se", bufs=2,
                                  name=f"xcf{nt}")
                    nc.sync.dma_start(xcf[:], x2pb_d[:, nt * 4:(nt + 1) * 4, :])
                    for ft in range(FF // 512):
                        if ft == 0:
                            w1h = [w1pre, w1pre]
                            w1off = [0, 256]
                        else:
                            w1h = []
                            w1off = [0, 0]
                            for hh in range(2):
                                w1t = bp.tile([P, DB, 256], R, tag="sh",
                                              bufs=4, name=f"w1_{nt}_{ft}_{hh}")
                                nc.sync.dma_start(
                                    w1t[:],
                                    w1_d[:, ds(ft * 512 + hh * 256, 256)]
                                    .rearrange("(do dp) f -> dp do f", dp=P)
                                    .bitcast(R))
                                w1h.append(w1t)
                        for fc in range(4):
                            bf = ft * 4 + fc
                            ffn1_group(nt, ft, fc, None, psF, "fp", 8,
                                       y1[bf // 16][:, bf % 16, :],
                                       w1h[fc // 2],
                                       w1off[fc // 2] + (fc % 2) * P)
                    pf2 = [psF.tile([P, 512], F32, tag="fp", bufs=8,
                                    name=f"p2_{nt}_{j}") for j in range(8)]
                    NFT = FF // 512
                    def w2_halves(nt, ft):
                        hs = []
                        for hh in range(2):
                            w2t = bp.tile([P, 2, D], R, tag="sh", bufs=4,
                                          name=f"w2_{nt}_{ft}_{hh}")
                            nc.sync.dma_start(
                                w2t[:],
                                w2_d[ds(ft * 512 + hh * 256, 256), :]
                                .rearrange("(fo fp) d -> fp fo d", fp=P)
                                .bitcast(R))
                            hs.append(w2t)
                        return hs
                    for ft in range(NFT - 1):
                        w2h = w2_halves(nt, ft)
                        for fc in range(4):
                            bf = ft * 4 + fc
                            ysrc = y1[bf // 16][:, bf % 16, :]
                            for tb in range(4):
                                for dt in range(2):
                                    nc.tensor.matmul(
                                        pf2[tb * 2 + dt][:],
                                        ysrc[:, ts(tb, P)],
                                        w2h[fc // 2][:, fc % 2,
                                                     ds(dt * 512, 512)],
                                        start=(ft == 0 and fc == 0),
                                        stop=False)
                    # last f-tile: close each psum group in turn so its evac
                    # and output DMA overlap the remaining groups' matmuls
                    ftl = NFT - 1
                    w2h = w2_halves(nt, ftl)
                    for tb in range(4):
                        for dt in range(2):
                            for fc in range(4):
                                bf = ftl * 4 + fc
                                ysrc = y1[bf // 16][:, bf % 16, :]
                                nc.tensor.matmul(
                                    pf2[tb * 2 + dt][:],
                                    ysrc[:, ts(tb, P)],
                                    w2h[fc // 2][:, fc % 2,
                                                 ds(dt * 512, 512)],
                                    start=False, stop=(fc == 3))
                            rows = ds(nt * 512 + tb * P, P)
                            og = bp.tile([P, 512], F32, tag="rb", bufs=2,
                                         name=f"og{nt}_{tb}_{dt}")
                            nc.vector.tensor_tensor(og[:], pf2[tb * 2 + dt][:],
                                                    xcf[:, tb, ds(dt * 512, 512)],
                                                    OP.add)
                            nc.sync.dma_start(out_d[rows, ds(dt * 512, 512)],
                                              og[:])
            psF_cm.__exit__(None, None, None)
    nc.compile()
    return nc


def get_nc():
    global _CACHED_NC
    if _CACHED_NC is None:
        _CACHED_NC = build_nc()
    return _CACHED_NC


def kernel(**inputs):
    nc = get_nc()
    x = np.ascontiguousarray(np.asarray(inputs["x"], dtype=np.float32))
    B = x.shape[0]
    weights = {k: np.ascontiguousarray(np.asarray(v, dtype=np.float32))
               for k, v in inputs.items() if k != "x"}
    in_maps = [dict(weights, x=x[b]) for b in range(B)]
    res = run_bass_kernel_spmd(nc, in_maps, list(range(B)))
    return np.stack([res.results[b]["out"] for b in range(B)], axis=0)
